# revision 54
# baseline (speedup 1.0000x reference)
"""Trainium2 Bass kernel for a pre-LN transformer decoder block.

Shapes (hardcoded): B=4, S_TGT=S_SRC=2048, D=512, H=8, DK=64, FF=2048, fp32.

Sharding: 8 cores; core c handles batch c//2. The two cores of a batch split
the 2048 query rows into two causal-balanced groups of 2x512 rows:
  r0: global q-blocks [0:512) and [1536:2048)
  r1: global q-blocks [512:1024) and [1024:1536)
All cores run one identical SPMD program. Keys (and the x rows feeding K/V)
are PERMUTED per core so that the own q-blocks land at canonical positions:
  pi = [own0 | filler0 | own1 | filler1]   (4 blocks of 512 rows)
With this order both ranks see SA extents of 8 k-tiles (pos0) and 16 (pos1),
diagonal mask tiles align, and Q^T is just columns {block0, block2} of the
transposed/normalized x. Per-core visibility is carried by mask DATA built
on the host. Cross-attention is unmasked full-extent.

Precision/layout strategy:
 - Projections / FFN / Wo run as fp8e4m3 DoubleRow matmuls (2 contraction
   rows per pass over e-tile pairs, 4x PE rate vs f32r). Weights are scaled
   x64 and activations x16 into fp8; every PSUM result is descaled by
   2^-10 in its PSUM->SBUF copy (engine-alternated between DVE and Act).
 - K^T/Q^T are bf16 (scores at full PE rate); P is 16*exp(score/8), stored
   fp8 on unmasked k-tile pairs (DoubleRow PV) and bf16 on masked tiles.
   The ones-column of V (=16) carries the softmax denominator; the x16
   cancels in the division.
 - exp alternates between Act (native Exp) and DVE (Schraudolph bit-trick:
   bits = int(A*score + B) reinterpreted as bf16/e4m3), balancing the
   otherwise Act-bound attention spans.
"""

import numpy as np
import ml_dtypes

import concourse.bass as bass
import concourse.bacc as bacc
import concourse.mybir as mybir
import concourse.tile as tile
from concourse.bass_utils import run_bass_kernel_spmd

F32 = mybir.dt.float32
F32R = mybir.dt.float32r
BF16 = mybir.dt.bfloat16
FP8 = mybir.dt.float8e4
I8 = mybir.dt.int8
I16 = mybir.dt.int16
AFT = mybir.ActivationFunctionType
ALU = mybir.AluOpType
AXL = mybir.AxisListType
DR = mybir.MatmulPerfMode.DoubleRow

B, S, D, H, DK, FF = 4, 2048, 512, 8, 64, 2048
P = 128            # partitions
ET = D // P        # 4 e-tiles of 128 over the model dim
EP = ET // 2       # e-tile pairs for DoubleRow
SQ = 1024          # own query rows per core
NKT0, NKT1 = 8, 16  # uniform k-tile extents for SA pos0 / pos1
EPS = 1e-6

WS = 64.0          # fp8 weight scale
XS = 16.0          # fp8 activation scale
DS = 1.0 / (WS * XS)   # descale after a DoubleRow matmul
LN16 = float(np.log(16.0))
LOG2E = 1.4426950408889634
# Schraudolph exp: bits = trunc(score*A + B); B includes the x16 bias
SCH_A_BF = 128.0 * LOG2E / 8.0
SCH_B_BF = (127.0 + 4.0) * 128.0 - 8.0
SCH_A_F8 = 8.0 * LOG2E / 8.0
SCH_B_F8 = (7.0 + 4.0) * 8.0

OWN_BLOCKS = {0: (0, 3), 1: (1, 2)}
PERM_BLOCKS = {0: (0, 1, 3, 2), 1: (1, 0, 2, 3)}
Q_SRC_QBS = (0, 2)
# combined 4-in-1 transpose PSUM (HW-proven); CoreSim's checker rejects it,
# so debugging scripts can flip this off before build.
COMBINED_TP = True
USE_SCH = True     # DVE Schraudolph exp offload
USE_PAIRS = True   # fp8 DoubleRow PV on unmasked k-tile pairs
DEBUG_TAPS = False  # dump intermediates to DRAM for debugging


def _r(ap, pattern, **kw):
    return ap.rearrange(pattern, **kw)


def build_program(bias_flags):
    """Build the SPMD Bass program. bias_flags: dict of bools saying which
    folded biases are nonzero (uniform across cores)."""
    nc = bacc.Bacc("TRN2", target_bir_lowering=False, debug=False, num_devices=8)

    def din(name, shape, dt=F32):
        return nc.dram_tensor(name, shape, dt, kind="ExternalInput").ap()

    xkv_d = din("xkv", [S, D])
    enc_d = din("enc", [S, D])
    masks_d = din("masks", [16, P, 512], BF16)
    msrc_d = din("msrc", [P, S // P])
    ident_d = din("ident", [P, P])
    # fp8 weights, pre-scaled x64
    w_sa = {k: din(f"sa_{k}", [D, D], FP8) for k in ("wq", "wk", "wv", "wo")}
    w_ca = {k: din(f"ca_{k}", [D, D], FP8) for k in ("wq", "wk", "wv", "wo")}
    w1_d = {k: din(f"w1{k}", [D, FF], FP8) for k in "ab"}
    w2_d = {k: din(f"w2{k}", [FF, D], FP8) for k in "ab"}
    # folded biases (pre-scaled x1024), [1, D] / [1, FF]
    b_sa = {k: din(f"bsa_{k}", [1, D]) for k in ("q", "k", "v", "o")}
    b_ca = {k: din(f"bca_{k}", [1, D]) for k in ("q", "k", "v", "o")}
    b1_d = din("b1", [1, FF])
    b2_d = din("b2", [1, D])
    out_d = nc.dram_tensor("out", [SQ, D], F32, kind="ExternalOutput").ap()
    taps = {}
    taps_live = {}
    if DEBUG_TAPS:
        for nm, shape, dt in [
                ("dbg_kvT8", [P, ET * S], FP8),
                ("dbg_ktT", [P, ET * S], BF16),
                ("dbg_qtT", [P, ET * SQ], BF16),
                ("dbg_va", [P, 16 * H * 66], FP8),
                ("dbg_atT8", [P, ET * SQ], FP8),
                ("dbg_x1", [P, (SQ // P) * D], F32),
                ("dbg_x2", [P, (SQ // P) * D], F32)]:
            taps[nm] = nc.dram_tensor(nm, shape, dt,
                                      kind="ExternalOutput").ap()

    with tile.TileContext(nc) as tc:
        with (
            tc.tile_pool(name="persist", bufs=1) as pp,
            tc.tile_pool(name="ln_sb", bufs=5) as lnp,
            tc.tile_pool(name="p_sb", bufs=6) as psb,
            tc.tile_pool(name="ln_st", bufs=4) as stp,
            tc.tile_pool(name="sc_ps", bufs=3, space="PSUM") as scp,
            tc.tile_pool(name="acc_ps", bufs=1, space="PSUM") as accp,
        ):
            ident = pp.tile([P, P], F32R, tag="ident")
            nc.sync.dma_start(ident[:], ident_d[:].bitcast(F32R))
            ones1f = pp.tile([1, P], F32, tag="ones1f")
            nc.vector.memset(ones1f[:], 1.0)
            ones1 = pp.tile([1, P], F32R, tag="ones1")
            nc.vector.tensor_copy(ones1[:], ones1f[:])
            c16_bf = pp.tile([1, P], BF16, tag="c16_bf")
            nc.vector.memset(c16_bf[:], 16.0)
            ln16 = pp.tile([P, 1], F32, tag="ln16")
            nc.vector.memset(ln16[:], LN16)
            msrc = pp.tile([P, S // P], F32, tag="msrc")
            nc.sync.dma_start(msrc[:], msrc_d[:])
            masks_sb = pp.tile([P, 16 * 512], BF16, tag="masks")

            def load_w(dram, name, cols=D):
                # [cin, cols] -> sbuf [128, ET, cols] fp8
                t = pp.tile([P, ET * cols], FP8, tag=name)
                nc.sync.dma_start(
                    _r(t[:], "p (e d) -> p e d", e=ET),
                    _r(dram[:], "(e p) d -> p e d", p=P))
                return t

            def load_bias(dram, name, flag, n=D):
                if not flag:
                    return None
                t = pp.tile([1, n], F32R, tag=name)
                nc.sync.dma_start(t[:], dram[:].bitcast(F32R))
                return t

            def pv8(t8):
                return _r(t8[:], "p (e s) -> p e s", e=ET)

            def copy_scaled(dst, src, c, on_act):
                """PSUM->SBUF copy with scale, engine-balanced."""
                with nc.allow_low_precision("fp8/bf16 staging"):
                    if on_act:
                        nc.scalar.activation(dst, src, AFT.Copy, scale=c)
                    else:
                        nc.vector.tensor_scalar_mul(dst, src, c)

            def ln_stats(x_t):
                """x_t: [128, 512] f32 sbuf -> (scale, bias) per-row [128,1]."""
                sx = stp.tile([P, 1], F32, tag="sx")
                dump = lnp.tile([P, D], F32, tag="ln_xn")
                sq = stp.tile([P, 1], F32, tag="sq")
                nc.scalar.activation(dump[:], x_t[:], AFT.Square,
                                     accum_out=sq[:])
                nc.vector.reduce_sum(sx[:], x_t[:], axis=AXL.X)
                mu = stp.tile([P, 1], F32, tag="mu")
                nc.vector.tensor_scalar_mul(mu[:], sx[:], 1.0 / D)
                m2 = stp.tile([P, 1], F32, tag="m2")
                nc.vector.tensor_mul(m2[:], mu[:], mu[:])
                v1 = stp.tile([P, 1], F32, tag="v1")
                nc.vector.tensor_scalar(v1[:], m2[:], -float(D), None,
                                        op0=ALU.mult)
                nc.vector.tensor_add(v1[:], v1[:], sq[:])
                std = stp.tile([P, 1], F32, tag="std")
                nc.scalar.activation(std[:], v1[:], AFT.Sqrt,
                                     scale=1.0 / (D - 1))
                nc.vector.tensor_scalar_add(std[:], std[:], EPS)
                s = stp.tile([P, 1], F32, tag="s")
                nc.vector.reciprocal(s[:], std[:])
                nb = stp.tile([P, 1], F32, tag="nb")
                nc.vector.tensor_mul(nb[:], mu[:], s[:])
                nc.vector.tensor_scalar_mul(nb[:], nb[:], -1.0)
                return s, nb

            def transpose4(xn, dstT8, rows, t, on_act):
                """Transpose [128, 512] f32r sbuf tile t into dstT8
                [128, ET*rows] fp8 (x16): 4 e-transposes, 1 scaled copy."""
                if COMBINED_TP:
                    ps = scp.tile([P, 1024], F32R, tag="score",
                                  name="tp")[:, 0:512]
                    for e in range(ET):
                        nc.tensor.matmul(
                            ps[:, e * P:(e + 1) * P],
                            xn[:, e * P:(e + 1) * P], ident[:],
                            start=(e == 0), stop=(e == ET - 1),
                            is_transpose=True, skip_group_check=(e != 0))
                    copy_scaled(
                        _r(dstT8[:], "p (e s) -> p e s", e=ET)[
                            :, :, t * P:(t + 1) * P],
                        _r(ps[:], "p (e c) -> p e c", e=ET), XS, on_act)
                else:
                    for e in range(ET):
                        ps = scp.tile([P, 1024], F32R, tag="score",
                                      name="tp")[:, 0:P]
                        nc.tensor.matmul(
                            ps[:], xn[:, e * P:(e + 1) * P], ident[:],
                            start=True, stop=True, is_transpose=True)
                        copy_scaled(
                            _r(dstT8[:], "p (e s) -> p e s", e=ET)[
                                :, e, t * P:(t + 1) * P],
                            ps[:], XS, on_act)

            def ln_transpose(src_d, rows, dstT8, do_ln=True):
                nt = rows // P
                for t in range(nt):
                    if do_ln:
                        x_t = lnp.tile([P, D], F32, tag="ln_x")
                        nc.sync.dma_start(x_t[:], src_d[t * P:(t + 1) * P, :])
                        s, nb = ln_stats(x_t)
                        xn = lnp.tile([P, D], F32R, tag="ln_xn")
                        nc.vector.tensor_scalar(xn[:], x_t[:], s[:], nb[:],
                                                op0=ALU.mult, op1=ALU.add)
                        transpose4(xn[:], dstT8, rows, t,
                                   on_act=(t % 2 == 0))
                    else:
                        xn = lnp.tile([P, D], F32R, tag="ln_x")
                        nc.sync.dma_start(
                            xn[:], src_d[t * P:(t + 1) * P, :].bitcast(F32R))
                        transpose4(xn[:], dstT8, rows, t, on_act=(t % 2 == 0))

            def ln_transpose_sbuf(xsb, dstT8, tiles=None):
                for t in (tiles if tiles is not None else range(SQ // P)):
                    x_t = xsb[:, t * D:(t + 1) * D]
                    s, nb = ln_stats(x_t)
                    xn = lnp.tile([P, D], F32R, tag="ln_xn")
                    nc.vector.tensor_scalar(xn[:], x_t, s[:], nb[:],
                                            op0=ALU.mult, op1=ALU.add)
                    transpose4(xn[:], dstT8, SQ, t, on_act=(t % 2 == 0))

            def projT(wt8, bt, has_b, srcT8, src_rows, dstT, src_qbs=None):
                """dstT[:, dt, :] = bf16 (W.T @ xn.T)-slice via DoubleRow."""
                if src_qbs is None:
                    src_qbs = list(range(src_rows // 512))
                nqb = len(src_qbs)
                for dt in range(ET):
                    for dqb, qb in enumerate(src_qbs):
                        ps = scp.tile([P, 1024], F32, tag="score",
                                      name="ps")[:, 0:512]
                        for ep in range(EP):
                            nc.tensor.matmul(
                                ps[:],
                                pv8(wt8)[:, 2 * ep:2 * ep + 2,
                                         dt * P:(dt + 1) * P],
                                pv8(srcT8)[:, 2 * ep:2 * ep + 2,
                                           qb * 512:(qb + 1) * 512],
                                start=(ep == 0),
                                stop=(ep == EP - 1 and not has_b),
                                perf_mode=DR)
                            if ep == EP - 1 and has_b:
                                nc.tensor.matmul(
                                    ps[:], bt[:, dt * P:(dt + 1) * P],
                                    ones1[:, 0:512].bitcast(F32R),
                                    start=False, stop=True)
                        copy_scaled(
                            dstT[:, dt * nqb * 512 + dqb * 512:
                                 dt * nqb * 512 + (dqb + 1) * 512],
                            ps[:], DS, on_act=((dt + dqb) % 2 == 0))

            def proj_va(wt8, bt, has_b, srcT8, src_rows, va):
                """V projection, token-major: va [128, nkt*8*66] fp8 = 16*V
                (+ src-mask row scaling), ones column = 16."""
                nkt = src_rows // P
                for kt in range(nkt):
                    ps = scp.tile([P, 1024], F32, tag="score",
                                  name="ps")[:, 0:512]
                    for ep in range(EP):
                        nc.tensor.matmul(
                            ps[:],
                            pv8(srcT8)[:, 2 * ep:2 * ep + 2,
                                       kt * P:(kt + 1) * P],
                            pv8(wt8)[:, 2 * ep:2 * ep + 2, 0:D],
                            start=(ep == 0),
                            stop=(ep == EP - 1 and not has_b),
                            perf_mode=DR)
                        if ep == EP - 1 and has_b:
                            nc.tensor.matmul(
                                ps[:], ones1[:, 0:P], bt[:],
                                start=False, stop=True)
                    dst = _r(va[:], "p (t h c) -> p t h c", t=nkt, h=H)
                    if bias_flags.get("msrc1"):
                        copy_scaled(dst[:, kt, :, 0:DK],
                                    _r(ps[:], "p (h c) -> p h c", h=H),
                                    XS * DS, on_act=(kt % 2 == 0))
                    else:
                        with nc.allow_low_precision("fp8 va"):
                            nc.vector.tensor_scalar(
                                dst[:, kt, :, 0:DK],
                                _r(ps[:], "p (h c) -> p h c", h=H),
                                msrc[:, kt:kt + 1], XS * DS,
                                op0=ALU.mult, op1=ALU.mult)

            def attention(ktT, va, qtT, nkts, masked, exp_dve, wo8, bo,
                          has_bo, resid, x_out, on_pos_done=None):
                """ktT [128, 4*S_k] bf16; va [128, nkt*8*66] fp8 (16*V);
                qtT [128, 4*1024] bf16; masked: fn(pos,kt)-> mask idx|None;
                exp_dve: fn(pos,kt)->bool; x_out [128,8*512] f32 resid+attn."""
                skmax = max(nkts) * P
                atT8 = pp.tile([P, ET * SQ], FP8, tag="attnT")
                taps_live["atT8"] = atT8
                for pos, nkt in enumerate(nkts):
                    units = []
                    kt = 0
                    while kt < nkt:
                        if (USE_PAIRS and masked(pos, kt) is None
                                and kt + 1 < nkt
                                and masked(pos, kt + 1) is None):
                            units.append((kt, kt + 1))
                            kt += 2
                        else:
                            units.append((kt,))
                            kt += 1
                    for hp in range(H // 2):
                        acc = [accp.tile([P, 512], F32, tag=f"acc{i}",
                                         name=f"acc{i}")
                               for i in range(2)]

                        def emit_score(kt):
                            st = scp.tile([P, 1024], F32, tag="score")
                            for i in range(2):
                                nc.tensor.matmul(
                                    st[:, i * 512:(i + 1) * 512],
                                    ktT[i * DK:(i + 1) * DK,
                                        hp * skmax + kt * P:
                                        hp * skmax + (kt + 1) * P],
                                    qtT[i * DK:(i + 1) * DK,
                                        hp * SQ + pos * 512:
                                        hp * SQ + (pos + 1) * 512],
                                    start=True, stop=True)
                            return st

                        def do_exp(pos, kt, st, pair=None):
                            """pair: ([128,2048] int8 tile, j) for fp8 pair
                            halves; None -> bf16 p_t (masked path)."""
                            on_dve = USE_SCH and exp_dve(pos, kt)
                            if pair is not None:
                                t8, j = pair
                                dst = t8[:, j * 1024:(j + 1) * 1024]
                                if on_dve:
                                    with nc.allow_low_precision("sch exp"):
                                        nc.vector.tensor_scalar(
                                            dst, st[:], SCH_A_F8, SCH_B_F8,
                                            op0=ALU.mult, op1=ALU.add)
                                else:
                                    nc.scalar.activation(
                                        dst.bitcast(FP8), st[:], AFT.Exp,
                                        bias=ln16[:], scale=1.0 / 8.0)
                                return None
                            p_t = psb.tile([P, 1024], I16, tag="p")
                            if on_dve:
                                with nc.allow_low_precision("sch exp"):
                                    nc.vector.tensor_scalar(
                                        p_t[:], st[:], SCH_A_BF, SCH_B_BF,
                                        op0=ALU.mult, op1=ALU.add)
                            else:
                                nc.scalar.activation(
                                    p_t[:].bitcast(BF16), st[:], AFT.Exp,
                                    bias=ln16[:], scale=1.0 / 8.0)
                            return p_t

                        flat = [kt for u in units for kt in u]
                        sts = {flat[0]: emit_score(flat[0])}

                        def prefetch(kt):
                            fi = flat.index(kt) + 1
                            if fi < len(flat):
                                sts[flat[fi]] = emit_score(flat[fi])

                        vat = _r(va[:], "p (t h c) -> p t h c",
                                 t=S // P, h=H)
                        for u in units:
                            if len(u) == 2:
                                k0, k1 = u
                                p2 = psb.tile([P, 2048], I8, tag="p2")
                                for j, kt in enumerate(u):
                                    st = sts.pop(kt)
                                    prefetch(kt)
                                    do_exp(pos, kt, st, pair=(p2, j))
                                p8 = p2[:].bitcast(FP8)
                                for i in range(2):
                                    h = 2 * hp + i
                                    nc.tensor.matmul(
                                        acc[i][0:DK + 2, :],
                                        vat[:, k0:k0 + 2, h, 0:66],
                                        _r(p8, "p (j x) -> p j x", j=2)[
                                            :, :, i * 512:(i + 1) * 512],
                                        start=(k0 == 0),
                                        stop=(k1 == nkt - 1),
                                        perf_mode=DR)
                            else:
                                kt = u[0]
                                st = sts.pop(kt)
                                prefetch(kt)
                                p_t = do_exp(pos, kt, st)
                                pb = p_t[:].bitcast(BF16)
                                mi = masked(pos, kt)
                                if mi is not None:
                                    mt = masks_sb[:, mi * 512:(mi + 1) * 512]
                                    for i in range(2):
                                        nc.vector.tensor_mul(
                                            pb[:, i * 512:(i + 1) * 512],
                                            pb[:, i * 512:(i + 1) * 512],
                                            mt)
                                for i in range(2):
                                    h = 2 * hp + i
                                    nc.tensor.matmul(
                                        acc[i][0:DK + 2, :],
                                        vat[:, kt, h, 0:66],
                                        pb[:, i * 512:(i + 1) * 512],
                                        start=(kt == 0), stop=(kt == nkt - 1))
                        # epilogue: atT8 = 16 * acc/denom (fp8)
                        rcl = []
                        for i in range(2):
                            rc = lnp.tile([1, 512], BF16, tag="ln_xn",
                                          name="rc")
                            with nc.allow_low_precision("softmax denom"):
                                nc.vector.reciprocal(
                                    rc[:], acc[i][DK:DK + 1, :])
                            rcl.append(rc)
                        rbl = []
                        for i in range(2):
                            rb = scp.tile([P, 1024], F32, tag="score",
                                          name="rb")[:, 0:512]
                            nc.tensor.matmul(
                                rb[0:DK, :], c16_bf[:, 0:DK],
                                rcl[i][:], start=True, stop=True)
                            rbs = lnp.tile([DK, 512], F32, tag="ln_xn",
                                           name="rbs")
                            copy_scaled(rbs[:], rb[0:DK, :], 1.0,
                                        on_act=True)
                            rbl.append(rbs)
                        for i in range(2):
                            with nc.allow_low_precision("fp8 attn out"):
                                nc.vector.tensor_tensor(
                                    atT8[i * DK:(i + 1) * DK,
                                         hp * SQ + pos * 512:
                                         hp * SQ + (pos + 1) * 512],
                                    acc[i][0:DK, :], rbl[i][:],
                                    op=ALU.mult)
                    # output projection + residual for this pos block
                    for qt in range(pos * 4, (pos + 1) * 4):
                        res = resid(qt)
                        ps = scp.tile([P, 1024], F32, tag="score",
                                      name="ps")[:, 0:512]
                        for ep in range(EP):
                            nc.tensor.matmul(
                                ps[:],
                                pv8(atT8)[:, 2 * ep:2 * ep + 2,
                                          qt * P:(qt + 1) * P],
                                pv8(wo8)[:, 2 * ep:2 * ep + 2, 0:D],
                                start=(ep == 0),
                                stop=(ep == EP - 1 and not has_bo),
                                perf_mode=DR)
                            if ep == EP - 1 and has_bo:
                                nc.tensor.matmul(
                                    ps[:], ones1[:, 0:P], bo[:],
                                    start=False, stop=True)
                        tmp = lnp.tile([P, D], F32, tag="ln_xn", name="wot")
                        nc.scalar.activation(tmp[:], ps[:], AFT.Copy,
                                             scale=DS)
                        nc.vector.tensor_tensor(
                            x_out[:, qt * D:(qt + 1) * D], tmp[:],
                            res, op=ALU.add)
                    if on_pos_done is not None:
                        on_pos_done(pos)

            # ---------------- stage A: LN0 + transposes ----------------
            kvT8 = pp.tile([P, ET * S], FP8, tag="kvT")
            ln_transpose(xkv_d, S, kvT8, do_ln=True)

            # ---------------- weights (all upfront, Pool DGE queue) -------
            wk8 = load_w(w_sa["wk"], "w_a")
            wq8 = load_w(w_sa["wq"], "w_b")
            wv8 = load_w(w_sa["wv"], "w_c")
            wo8 = load_w(w_sa["wo"], "w_d")
            ck8 = load_w(w_ca["wk"], "w_e")
            cq8 = load_w(w_ca["wq"], "w_f")
            cv8 = load_w(w_ca["wv"], "w_g")
            co8 = load_w(w_ca["wo"], "w_h")
            w18 = [load_w(w1_d[k], f"w1{k}", cols=FF) for k in "ab"]
            w28 = []
            for k in "ab":
                t = pp.tile([P, (FF // P) * D], FP8, tag=f"w2{k}")
                nc.sync.dma_start(
                    _r(t[:], "p (t d) -> p t d", t=FF // P),
                    _r(w2_d[k][:], "(t p) d -> p t d", p=P))
                w28.append(t)
            bk = load_bias(b_sa["k"], "b_a", bias_flags["sa_k"])
            bq = load_bias(b_sa["q"], "b_b", bias_flags["sa_q"])
            bv = load_bias(b_sa["v"], "b_c", bias_flags["sa_v"])
            bo = load_bias(b_sa["o"], "b_d", bias_flags["sa_o"])
            cbk = load_bias(b_ca["k"], "b_e", bias_flags["ca_k"])
            cbq = load_bias(b_ca["q"], "b_f", bias_flags["ca_q"])
            cbv = load_bias(b_ca["v"], "b_g", bias_flags["ca_v"])
            cbo = load_bias(b_ca["o"], "b_h", bias_flags["ca_o"])
            b1_sb = load_bias(b1_d, "b1", bias_flags["ff1"], n=FF)
            b2_sb = load_bias(b2_d, "b2", bias_flags["ff2"])
            nc.sync.dma_start(
                _r(masks_sb[:], "p (t c) -> p t c", t=16),
                _r(masks_d[:], "t p c -> p t c"))

            # ---------------- stage B: SA projections ----------------
            ktT_sa = pp.tile([P, ET * S], BF16, tag="ktT")
            qtT_sa = pp.tile([P, ET * SQ], BF16, tag="qtT")
            va_sa = pp.tile([P, 16 * H * 66], FP8, tag="va")
            nc.vector.memset(
                _r(va_sa[:], "p (t h c) -> p t h c", t=16, h=H)[:, :, :, DK:DK + 1],
                16.0)
            nc.vector.memset(
                _r(va_sa[:], "p (t h c) -> p t h c", t=16, h=H)[:, :, :, DK + 1:],
                0.0)
            if DEBUG_TAPS:
                nc.sync.dma_start(taps["dbg_kvT8"][:], kvT8[:])
            projT(wk8, bk, bias_flags["sa_k"], kvT8, S, ktT_sa)
            projT(wq8, bq, bias_flags["sa_q"], kvT8, S, qtT_sa,
                  src_qbs=list(Q_SRC_QBS))
            proj_va(wv8, bv, bias_flags["sa_v"], kvT8, S, va_sa)

            if DEBUG_TAPS:
                nc.sync.dma_start(taps["dbg_ktT"][:], ktT_sa[:])
                nc.sync.dma_start(taps["dbg_qtT"][:], qtT_sa[:])
                nc.sync.dma_start(taps["dbg_va"][:], va_sa[:])
            # ---------------- stage C/D: SA attention + Wo ----------------
            x1 = pp.tile([P, (SQ // P) * D], F32, tag="x1")

            def sa_masked(pos, kt):
                return kt if (pos == 0 or kt >= 8) else None

            def sa_exp_dve(pos, kt):
                if sa_masked(pos, kt) is None:
                    return kt % 4 == 1
                return (kt % 8) == 2

            def q_src_row(qt):
                pos, j = divmod(qt, 4)
                return Q_SRC_QBS[pos] * 512 + j * P

            def sa_resid(qt):
                rt = lnp.tile([P, D], F32, tag="ln_x", name="sa_resid")
                r0 = q_src_row(qt)
                nc.sync.dma_start(rt[:], xkv_d[r0:r0 + P, :])
                return rt[:]

            # hoisted CA prep: encoder transpose overlaps SA attention
            encT8 = pp.tile([P, ET * S], FP8, tag="kvT")  # reuse kvT slot
            ln_transpose(enc_d, S, encT8, do_ln=False)

            attention(ktT_sa, va_sa, qtT_sa, [NKT0, NKT1], sa_masked,
                      sa_exp_dve, wo8, bo, bias_flags["sa_o"], sa_resid, x1)

            if DEBUG_TAPS:
                nc.sync.dma_start(taps["dbg_x1"][:], x1[:])
                nc.sync.dma_start(taps["dbg_atT8"][:],
                                  taps_live["atT8"][:])
            # ---------------- stage E/F: CA ----------------
            ktT_ca = pp.tile([P, ET * S], BF16, tag="ktT")
            qtT_ca = pp.tile([P, ET * SQ], BF16, tag="qtT")
            va_ca = pp.tile([P, 16 * H * 66], FP8, tag="va")
            nc.vector.memset(
                _r(va_ca[:], "p (t h c) -> p t h c", t=16, h=H)[:, :, :, DK:DK + 1],
                16.0)
            nc.vector.memset(
                _r(va_ca[:], "p (t h c) -> p t h c", t=16, h=H)[:, :, :, DK + 1:],
                0.0)
            projT(ck8, cbk, bias_flags["ca_k"], encT8, S, ktT_ca)
            proj_va(cv8, cbv, bias_flags["ca_v"], encT8, S, va_ca)

            h1T8 = pp.tile([P, ET * SQ], FP8, tag="hT")
            ln_transpose_sbuf(x1, h1T8)
            projT(cq8, cbq, bias_flags["ca_q"], h1T8, SQ, qtT_ca)

            x2 = pp.tile([P, (SQ // P) * D], F32, tag="x2")

            attention(ktT_ca, va_ca, qtT_ca, [16, 16], lambda p, k: None,
                      lambda p, k: (k % 2 == 1) and (k % 16 != 15), co8,
                      cbo,
                      bias_flags["ca_o"],
                      lambda qt: x1[:, qt * D:(qt + 1) * D], x2)

            if DEBUG_TAPS:
                nc.sync.dma_start(taps["dbg_x2"][:], x2[:])
            # ---------------- stage G: LN2 + FFN ----------------
            h2T8 = pp.tile([P, ET * SQ], FP8, tag="hT")
            ln_transpose_sbuf(x2, h2T8)

            ffT8 = pp.tile([P, (FF // P) * SQ], FP8, tag="ffT")
            for ft in range(FF // P):
                for qb in range(SQ // 512):
                    ps = scp.tile([P, 1024], F32, tag="score",
                                  name="ps")[:, 0:512]
                    for wi, wt in enumerate(w18):
                        for ep in range(EP):
                            last = (wi == 1 and ep == EP - 1)
                            nc.tensor.matmul(
                                ps[:],
                                _r(wt[:], "p (e f) -> p e f", e=ET)[
                                    :, 2 * ep:2 * ep + 2,
                                    ft * P:(ft + 1) * P],
                                pv8(h2T8)[:, 2 * ep:2 * ep + 2,
                                          qb * 512:(qb + 1) * 512],
                                start=(wi == 0 and ep == 0),
                                stop=(last and not bias_flags["ff1"]),
                                perf_mode=DR)
                            if last and bias_flags["ff1"]:
                                nc.tensor.matmul(
                                    ps[:], b1_sb[:, ft * P:(ft + 1) * P],
                                    ones1[:, 0:512].bitcast(F32R),
                                    start=False, stop=True)
                    # ffT8 = 16*relu(z); psum holds 1024*z
                    nc.scalar.activation(
                        ffT8[:, ft * SQ + qb * 512:ft * SQ + (qb + 1) * 512],
                        ps[:], AFT.Relu, scale=XS * DS)
            for qt in range(SQ // P):
                ps = scp.tile([P, 1024], F32, tag="score",
                              name="ps")[:, 0:512]
                nfp = FF // P // 2
                for wi, wt in enumerate(w28):
                    for fp in range(nfp):
                        nc.tensor.matmul(
                            ps[:],
                            _r(ffT8[:], "p (t s) -> p t s", t=FF // P)[
                                :, 2 * fp:2 * fp + 2, qt * P:(qt + 1) * P],
                            _r(wt[:], "p (t d) -> p t d", t=FF // P)[
                                :, 2 * fp:2 * fp + 2, :],
                            start=(wi == 0 and fp == 0),
                            stop=(wi == 1 and fp == nfp - 1
                                  and not bias_flags["ff2"]),
                            perf_mode=DR)
                if bias_flags["ff2"]:
                    nc.tensor.matmul(
                        ps[:], ones1[:, 0:P], b2_sb[:],
                        start=False, stop=True)
                tmp = lnp.tile([P, D], F32, tag="ln_xn", name="ff2t")
                copy_scaled(tmp[:], ps[:], DS, on_act=(qt % 2 == 0))
                o_t = lnp.tile([P, D], F32, tag="ln_xn", name="o_t")
                nc.vector.tensor_tensor(
                    o_t[:], tmp[:], x2[:, qt * D:(qt + 1) * D], op=ALU.add)
                nc.sync.dma_start(out_d[qt * P:(qt + 1) * P, :], o_t[:])

    nc.finalize()
    return nc


_CACHE = {}
LAST_EXEC_NS = None


def kernel(**inputs):
    x = np.asarray(inputs["x"], np.float32)
    enc = np.asarray(inputs["encoder_output"], np.float32)
    src_mask = np.asarray(inputs["src_mask"]).reshape(S)
    tgt_mask = np.asarray(inputs["tgt_mask"]).reshape(S, S)

    def fold(w, g, b, extra_b):
        w = np.asarray(w, np.float32)
        wf = np.asarray(g, np.float32)[:, None] * w
        bf = np.asarray(b, np.float32) @ w + np.asarray(extra_b, np.float32)
        return wf, bf

    def q8(w):
        return np.asarray(w * WS, np.float32).astype(ml_dtypes.float8_e4m3)

    def q8r(w):
        ws = np.asarray(w * WS, np.float32)
        return (ws - ws.astype(ml_dtypes.float8_e4m3)
                .astype(np.float32)).astype(ml_dtypes.float8_e4m3)

    z = np.zeros(D, np.float32)
    sa_wq, bsa_q = fold(inputs["sa_wq"], inputs["ln0_g"], inputs["ln0_b"], z)
    sa_wk, bsa_k = fold(inputs["sa_wk"], inputs["ln0_g"], inputs["ln0_b"], z)
    sa_wv, bsa_v = fold(inputs["sa_wv"], inputs["ln0_g"], inputs["ln0_b"], z)
    sa_wo = np.asarray(inputs["sa_wo"], np.float32)
    bsa_o = np.asarray(inputs["sa_bo"], np.float32)
    ca_wq, bca_q = fold(inputs["ca_wq"], inputs["ln1_g"], inputs["ln1_b"], z)
    ca_wk = np.asarray(inputs["ca_wk"], np.float32)
    bca_k = np.zeros(D, np.float32)
    ca_wv = np.asarray(inputs["ca_wv"], np.float32)
    bca_v = np.zeros(D, np.float32)
    ca_wo = np.asarray(inputs["ca_wo"], np.float32)
    bca_o = np.asarray(inputs["ca_bo"], np.float32)
    w1, b1 = fold(inputs["ff_w1"], inputs["ln2_g"], inputs["ln2_b"],
                  np.asarray(inputs["ff_b1"], np.float32))
    w2 = np.asarray(inputs["ff_w2"], np.float32)
    b2 = np.asarray(inputs["ff_b2"], np.float32)

    bias_flags = {
        "sa_q": bool(np.any(bsa_q)), "sa_k": bool(np.any(bsa_k)),
        "sa_v": bool(np.any(bsa_v)), "sa_o": bool(np.any(bsa_o)),
        "ca_q": bool(np.any(bca_q)), "ca_k": bool(np.any(bca_k)),
        "ca_v": bool(np.any(bca_v)), "ca_o": bool(np.any(bca_o)),
        "ff1": bool(np.any(b1)), "ff2": bool(np.any(b2)),
        "msrc1": bool(np.all(src_mask == 1)),
    }

    key = tuple(sorted(bias_flags.items()))
    if key not in _CACHE:
        _CACHE[key] = build_program(bias_flags)
    nc = _CACHE[key]

    ident = np.eye(P, dtype=np.float32)
    msrc = src_mask.astype(np.float32).reshape(S // P, P).T.copy()
    BS = WS * XS  # bias pre-scale (descaled by DS in the psum copy)

    shared = {
        "ident": ident, "msrc": msrc,
        "sa_wq": q8(sa_wq), "sa_wk": q8(sa_wk), "sa_wv": q8(sa_wv),
        "sa_wo": q8(sa_wo),
        "ca_wq": q8(ca_wq), "ca_wk": q8(ca_wk), "ca_wv": q8(ca_wv),
        "ca_wo": q8(ca_wo),
        "w1a": q8(w1), "w1b": q8r(w1), "w2a": q8(w2), "w2b": q8r(w2),
        "bsa_q": bsa_q[None] * BS, "bsa_k": bsa_k[None] * BS,
        "bsa_v": bsa_v[None] * BS, "bsa_o": bsa_o[None] * BS,
        "bca_q": bca_q[None] * BS, "bca_k": bca_k[None] * BS,
        "bca_v": bca_v[None] * BS, "bca_o": bca_o[None] * BS,
        "b1": b1[None] * BS, "b2": b2[None] * BS,
    }

    in_maps = []
    for c in range(8):
        b, r = divmod(c, 2)
        perm = PERM_BLOCKS[r]
        rows = np.concatenate(
            [np.arange(gb * 512, (gb + 1) * 512) for gb in perm])
        gb0, gb1 = OWN_BLOCKS[r]
        assert perm[0] == gb0 and perm[2] == gb1
        mk = np.zeros((16, P, 512), np.float32)
        for pos, gb in enumerate((gb0, gb1)):
            qs = slice(gb * 512, (gb + 1) * 512)
            mrow = tgt_mask[qs][:, rows]
            for j in range(8):
                kt = j if pos == 0 else 8 + j
                ks = slice(kt * P, (kt + 1) * P)
                mk[pos * 8 + j] = mrow[:, ks].T
            ext = (NKT0 if pos == 0 else NKT1) * P
            assert not np.any(mrow[:, ext:]), "tgt_mask beyond extent"
        im = dict(shared)
        im["xkv"] = np.ascontiguousarray(x[b][rows])
        im["enc"] = np.ascontiguousarray(enc[b])
        im["masks"] = mk.astype(ml_dtypes.bfloat16)
        in_maps.append(im)

    res = run_bass_kernel_spmd(nc, in_maps, core_ids=list(range(8)))
    global LAST_EXEC_NS
    LAST_EXEC_NS = res.exec_time_ns

    out = np.empty((B, S, D), np.float32)
    for c in range(8):
        b, r = divmod(c, 2)
        gb0, gb1 = OWN_BLOCKS[r]
        o = res.results[c]["out"]
        out[b, gb0 * 512:(gb0 + 1) * 512] = o[0:512]
        out[b, gb1 * 512:(gb1 + 1) * 512] = o[512:1024]
    return out


# revision 55
# speedup vs baseline: 1.0321x; 1.0321x over previous
"""Trainium2 Bass kernel for a pre-LN transformer decoder block.

Shapes (hardcoded): B=4, S_TGT=S_SRC=2048, D=512, H=8, DK=64, FF=2048, fp32.

Sharding: 8 cores; core c handles batch c//2. The two cores of a batch split
the 2048 query rows into two causal-balanced groups of 2x512 rows:
  r0: global q-blocks [0:512) and [1536:2048)
  r1: global q-blocks [512:1024) and [1024:1536)
All cores run one identical SPMD program. Keys (and the x rows feeding K/V)
are PERMUTED per core so that the own q-blocks land at canonical positions:
  pi = [own0 | filler0 | own1 | filler1]   (4 blocks of 512 rows)
With this order both ranks see SA extents of 8 k-tiles (pos0) and 16 (pos1),
diagonal mask tiles align, and Q^T is just columns {block0, block2} of the
transposed/normalized x. Per-core visibility is carried by mask DATA built
on the host. Cross-attention is unmasked full-extent.

Precision/layout strategy:
 - Projections / FFN / Wo run as fp8e4m3 DoubleRow matmuls (2 contraction
   rows per pass over e-tile pairs, 4x PE rate vs f32r). Weights are scaled
   x64 and activations x16 into fp8; every PSUM result is descaled by
   2^-10 in its PSUM->SBUF copy (engine-alternated between DVE and Act).
 - K^T/Q^T are bf16 (scores at full PE rate); P is 16*exp(score/8), stored
   fp8 on unmasked k-tile pairs (DoubleRow PV) and bf16 on masked tiles.
   The ones-column of V (=16) carries the softmax denominator; the x16
   cancels in the division.
 - exp alternates between Act (native Exp) and DVE (Schraudolph bit-trick:
   bits = int(A*score + B) reinterpreted as bf16/e4m3), balancing the
   otherwise Act-bound attention spans.
"""

import numpy as np
import ml_dtypes

import concourse.bass as bass
import concourse.bacc as bacc
import concourse.mybir as mybir
import concourse.tile as tile
from concourse.bass_utils import run_bass_kernel_spmd

F32 = mybir.dt.float32
F32R = mybir.dt.float32r
BF16 = mybir.dt.bfloat16
FP8 = mybir.dt.float8e4
I8 = mybir.dt.int8
I16 = mybir.dt.int16
AFT = mybir.ActivationFunctionType
ALU = mybir.AluOpType
AXL = mybir.AxisListType
DR = mybir.MatmulPerfMode.DoubleRow

B, S, D, H, DK, FF = 4, 2048, 512, 8, 64, 2048
P = 128            # partitions
ET = D // P        # 4 e-tiles of 128 over the model dim
EP = ET // 2       # e-tile pairs for DoubleRow
SQ = 1024          # own query rows per core
NKT0, NKT1 = 8, 16  # uniform k-tile extents for SA pos0 / pos1
EPS = 1e-6

WS = 64.0          # fp8 weight scale
XS = 16.0          # fp8 activation scale
DS = 1.0 / (WS * XS)   # descale after a DoubleRow matmul
LN16 = float(np.log(16.0))
LOG2E = 1.4426950408889634
# Schraudolph exp: bits = trunc(score*A + B); B includes the x16 bias
SCH_A_BF = 128.0 * LOG2E / 8.0
SCH_B_BF = (127.0 + 4.0) * 128.0 - 8.0
SCH_A_F8 = 8.0 * LOG2E / 8.0
SCH_B_F8 = (7.0 + 4.0) * 8.0

OWN_BLOCKS = {0: (0, 3), 1: (1, 2)}
PERM_BLOCKS = {0: (0, 1, 3, 2), 1: (1, 0, 2, 3)}
Q_SRC_QBS = (0, 2)
# combined 4-in-1 transpose PSUM (HW-proven); CoreSim's checker rejects it,
# so debugging scripts can flip this off before build.
COMBINED_TP = True
USE_SCH = True     # DVE Schraudolph exp offload
USE_PAIRS = True   # fp8 DoubleRow PV on unmasked k-tile pairs
DEBUG_TAPS = False  # dump intermediates to DRAM for debugging


def _r(ap, pattern, **kw):
    return ap.rearrange(pattern, **kw)


def build_program(bias_flags):
    """Build the SPMD Bass program. bias_flags: dict of bools saying which
    folded biases are nonzero (uniform across cores)."""
    nc = bacc.Bacc("TRN2", target_bir_lowering=False, debug=False, num_devices=8)

    def din(name, shape, dt=F32):
        return nc.dram_tensor(name, shape, dt, kind="ExternalInput").ap()

    xkv_d = din("xkv", [S, D])
    enc_d = din("enc", [S, D])
    masks_d = din("masks", [16, P, 512], BF16)
    msrc_d = din("msrc", [P, S // P])
    ident_d = din("ident", [P, P])
    # fp8 weights, pre-scaled x64
    w_sa = {k: din(f"sa_{k}", [D, D], FP8) for k in ("wq", "wk", "wv", "wo")}
    w_ca = {k: din(f"ca_{k}", [D, D], FP8) for k in ("wq", "wk", "wv", "wo")}
    w1_d = {k: din(f"w1{k}", [D, FF], FP8) for k in "ab"}
    w2_d = {k: din(f"w2{k}", [FF, D], FP8) for k in "ab"}
    # folded biases (pre-scaled x1024), [1, D] / [1, FF]
    b_sa = {k: din(f"bsa_{k}", [1, D]) for k in ("q", "k", "v", "o")}
    b_ca = {k: din(f"bca_{k}", [1, D]) for k in ("q", "k", "v", "o")}
    b1_d = din("b1", [1, FF])
    b2_d = din("b2", [1, D])
    out_d = nc.dram_tensor("out", [SQ, D], F32, kind="ExternalOutput").ap()
    taps = {}
    taps_live = {}
    if DEBUG_TAPS:
        for nm, shape, dt in [
                ("dbg_kvT8", [P, ET * S], FP8),
                ("dbg_ktT", [P, ET * S], BF16),
                ("dbg_qtT", [P, ET * SQ], BF16),
                ("dbg_va", [P, 16 * H * 66], FP8),
                ("dbg_atT8", [P, ET * SQ], FP8),
                ("dbg_x1", [P, (SQ // P) * D], F32),
                ("dbg_x2", [P, (SQ // P) * D], F32)]:
            taps[nm] = nc.dram_tensor(nm, shape, dt,
                                      kind="ExternalOutput").ap()

    with tile.TileContext(nc) as tc:
        with (
            tc.tile_pool(name="persist", bufs=1) as pp,
            tc.tile_pool(name="ln_sb", bufs=5) as lnp,
            tc.tile_pool(name="p_sb", bufs=6) as psb,
            tc.tile_pool(name="ln_st", bufs=4) as stp,
            tc.tile_pool(name="sc_ps", bufs=3, space="PSUM") as scp,
            tc.tile_pool(name="acc_ps", bufs=1, space="PSUM") as accp,
        ):
            ident = pp.tile([P, P], F32R, tag="ident")
            nc.sync.dma_start(ident[:], ident_d[:].bitcast(F32R))
            ones1f = pp.tile([1, P], F32, tag="ones1f")
            nc.vector.memset(ones1f[:], 1.0)
            ones1 = pp.tile([1, P], F32R, tag="ones1")
            nc.vector.tensor_copy(ones1[:], ones1f[:])
            c16_bf = pp.tile([1, P], BF16, tag="c16_bf")
            nc.vector.memset(c16_bf[:], 16.0)
            ln16 = pp.tile([P, 1], F32, tag="ln16")
            nc.vector.memset(ln16[:], LN16)
            msrc = pp.tile([P, S // P], F32, tag="msrc")
            nc.sync.dma_start(msrc[:], msrc_d[:])
            masks_sb = pp.tile([P, 16 * 512], BF16, tag="masks")

            def load_w(dram, name, cols=D):
                # [cin, cols] -> sbuf [128, ET, cols] fp8
                t = pp.tile([P, ET * cols], FP8, tag=name)
                nc.sync.dma_start(
                    _r(t[:], "p (e d) -> p e d", e=ET),
                    _r(dram[:], "(e p) d -> p e d", p=P))
                return t

            def load_bias(dram, name, flag, n=D):
                if not flag:
                    return None
                t = pp.tile([1, n], F32R, tag=name)
                nc.sync.dma_start(t[:], dram[:].bitcast(F32R))
                return t

            def pv8(t8):
                return _r(t8[:], "p (e s) -> p e s", e=ET)

            def copy_scaled(dst, src, c, on_act):
                """PSUM->SBUF copy with scale, engine-balanced."""
                with nc.allow_low_precision("fp8/bf16 staging"):
                    if on_act:
                        nc.scalar.activation(dst, src, AFT.Copy, scale=c)
                    else:
                        nc.vector.tensor_scalar_mul(dst, src, c)

            def ln_stats(x_t):
                """x_t: [128, 512] f32 sbuf -> (scale, bias) per-row [128,1]."""
                sx = stp.tile([P, 1], F32, tag="sx")
                dump = lnp.tile([P, D], F32, tag="ln_xn")
                sq = stp.tile([P, 1], F32, tag="sq")
                nc.scalar.activation(dump[:], x_t[:], AFT.Square,
                                     accum_out=sq[:])
                nc.vector.reduce_sum(sx[:], x_t[:], axis=AXL.X)
                mu = stp.tile([P, 1], F32, tag="mu")
                nc.vector.tensor_scalar_mul(mu[:], sx[:], 1.0 / D)
                m2 = stp.tile([P, 1], F32, tag="m2")
                nc.vector.tensor_mul(m2[:], mu[:], mu[:])
                v1 = stp.tile([P, 1], F32, tag="v1")
                nc.vector.tensor_scalar(v1[:], m2[:], -float(D), None,
                                        op0=ALU.mult)
                nc.vector.tensor_add(v1[:], v1[:], sq[:])
                std = stp.tile([P, 1], F32, tag="std")
                nc.scalar.activation(std[:], v1[:], AFT.Sqrt,
                                     scale=1.0 / (D - 1))
                nc.vector.tensor_scalar_add(std[:], std[:], EPS)
                s = stp.tile([P, 1], F32, tag="s")
                nc.vector.reciprocal(s[:], std[:])
                nb = stp.tile([P, 1], F32, tag="nb")
                nc.vector.tensor_mul(nb[:], mu[:], s[:])
                nc.vector.tensor_scalar_mul(nb[:], nb[:], -1.0)
                return s, nb

            def transpose4(xn, dstT8, rows, t, on_act):
                """Transpose [128, 512] f32r sbuf tile t into dstT8
                [128, ET*rows] fp8 (x16): 4 e-transposes, 1 scaled copy."""
                if COMBINED_TP:
                    ps = scp.tile([P, 1024], F32R, tag="score",
                                  name="tp")[:, 0:512]
                    for e in range(ET):
                        nc.tensor.matmul(
                            ps[:, e * P:(e + 1) * P],
                            xn[:, e * P:(e + 1) * P], ident[:],
                            start=(e == 0), stop=(e == ET - 1),
                            is_transpose=True, skip_group_check=(e != 0))
                    copy_scaled(
                        _r(dstT8[:], "p (e s) -> p e s", e=ET)[
                            :, :, t * P:(t + 1) * P],
                        _r(ps[:], "p (e c) -> p e c", e=ET), XS, on_act)
                else:
                    for e in range(ET):
                        ps = scp.tile([P, 1024], F32R, tag="score",
                                      name="tp")[:, 0:P]
                        nc.tensor.matmul(
                            ps[:], xn[:, e * P:(e + 1) * P], ident[:],
                            start=True, stop=True, is_transpose=True)
                        copy_scaled(
                            _r(dstT8[:], "p (e s) -> p e s", e=ET)[
                                :, e, t * P:(t + 1) * P],
                            ps[:], XS, on_act)

            def ln_transpose(src_d, rows, dstT8, do_ln=True):
                nt = rows // P
                for t in range(nt):
                    if do_ln:
                        x_t = lnp.tile([P, D], F32, tag="ln_x")
                        nc.sync.dma_start(x_t[:], src_d[t * P:(t + 1) * P, :])
                        s, nb = ln_stats(x_t)
                        xn = lnp.tile([P, D], F32R, tag="ln_xn")
                        nc.vector.tensor_scalar(xn[:], x_t[:], s[:], nb[:],
                                                op0=ALU.mult, op1=ALU.add)
                        transpose4(xn[:], dstT8, rows, t,
                                   on_act=(t % 2 == 0))
                    else:
                        xn = lnp.tile([P, D], F32R, tag="ln_x")
                        nc.sync.dma_start(
                            xn[:], src_d[t * P:(t + 1) * P, :].bitcast(F32R))
                        transpose4(xn[:], dstT8, rows, t, on_act=(t % 2 == 0))

            def ln_transpose_sbuf(xsb, dstT8, tiles=None):
                for t in (tiles if tiles is not None else range(SQ // P)):
                    x_t = xsb[:, t * D:(t + 1) * D]
                    s, nb = ln_stats(x_t)
                    xn = lnp.tile([P, D], F32R, tag="ln_xn")
                    nc.vector.tensor_scalar(xn[:], x_t, s[:], nb[:],
                                            op0=ALU.mult, op1=ALU.add)
                    transpose4(xn[:], dstT8, SQ, t, on_act=(t % 2 == 0))

            def projT(wt8, bt, has_b, srcT8, src_rows, dstT, src_qbs=None):
                """dstT[:, dt, :] = bf16 (W.T @ xn.T)-slice via DoubleRow."""
                if src_qbs is None:
                    src_qbs = list(range(src_rows // 512))
                nqb = len(src_qbs)
                for dt in range(ET):
                    for dqb, qb in enumerate(src_qbs):
                        ps = scp.tile([P, 1024], F32, tag="score",
                                      name="ps")[:, 0:512]
                        for ep in range(EP):
                            nc.tensor.matmul(
                                ps[:],
                                pv8(wt8)[:, 2 * ep:2 * ep + 2,
                                         dt * P:(dt + 1) * P],
                                pv8(srcT8)[:, 2 * ep:2 * ep + 2,
                                           qb * 512:(qb + 1) * 512],
                                start=(ep == 0),
                                stop=(ep == EP - 1 and not has_b),
                                perf_mode=DR)
                            if ep == EP - 1 and has_b:
                                nc.tensor.matmul(
                                    ps[:], bt[:, dt * P:(dt + 1) * P],
                                    ones1[:, 0:512].bitcast(F32R),
                                    start=False, stop=True)
                        copy_scaled(
                            dstT[:, dt * nqb * 512 + dqb * 512:
                                 dt * nqb * 512 + (dqb + 1) * 512],
                            ps[:], DS, on_act=((dt + dqb) % 2 == 0))

            def proj_va(wt8, bt, has_b, srcT8, src_rows, va):
                """V projection, token-major: va [128, nkt*8*66] fp8 = 16*V
                (+ src-mask row scaling), ones column = 16."""
                nkt = src_rows // P
                for kt in range(nkt):
                    ps = scp.tile([P, 1024], F32, tag="score",
                                  name="ps")[:, 0:512]
                    for ep in range(EP):
                        nc.tensor.matmul(
                            ps[:],
                            pv8(srcT8)[:, 2 * ep:2 * ep + 2,
                                       kt * P:(kt + 1) * P],
                            pv8(wt8)[:, 2 * ep:2 * ep + 2, 0:D],
                            start=(ep == 0),
                            stop=(ep == EP - 1 and not has_b),
                            perf_mode=DR)
                        if ep == EP - 1 and has_b:
                            nc.tensor.matmul(
                                ps[:], ones1[:, 0:P], bt[:],
                                start=False, stop=True)
                    dst = _r(va[:], "p (t h c) -> p t h c", t=nkt, h=H)
                    if bias_flags.get("msrc1"):
                        copy_scaled(dst[:, kt, :, 0:DK],
                                    _r(ps[:], "p (h c) -> p h c", h=H),
                                    XS * DS, on_act=(kt % 2 == 0))
                    else:
                        with nc.allow_low_precision("fp8 va"):
                            nc.vector.tensor_scalar(
                                dst[:, kt, :, 0:DK],
                                _r(ps[:], "p (h c) -> p h c", h=H),
                                msrc[:, kt:kt + 1], XS * DS,
                                op0=ALU.mult, op1=ALU.mult)

            def attention(ktT, va, qtT, nkts, masked, exp_dve, wo8, bo,
                          has_bo, resid, x_out, on_pos_done=None):
                """ktT [128, 4*S_k] bf16; va [128, nkt*8*66] fp8 (16*V);
                qtT [128, 4*1024] bf16; masked: fn(pos,kt)-> mask idx|None;
                exp_dve: fn(pos,kt)->bool; x_out [128,8*512] f32 resid+attn."""
                skmax = max(nkts) * P
                atT8 = pp.tile([P, ET * SQ], FP8, tag="attnT")
                taps_live["atT8"] = atT8
                for pos, nkt in enumerate(nkts):
                    units = []
                    kt = 0
                    while kt < nkt:
                        if (USE_PAIRS and masked(pos, kt) is None
                                and kt + 1 < nkt
                                and masked(pos, kt + 1) is None):
                            units.append((kt, kt + 1))
                            kt += 2
                        else:
                            units.append((kt,))
                            kt += 1
                    for hp in range(H // 2):
                        acc = [accp.tile([P, 512], F32, tag=f"acc{i}",
                                         name=f"acc{i}")
                               for i in range(2)]

                        def emit_score(kt):
                            st = scp.tile([P, 1024], F32, tag="score")
                            for i in range(2):
                                nc.tensor.matmul(
                                    st[:, i * 512:(i + 1) * 512],
                                    ktT[i * DK:(i + 1) * DK,
                                        hp * skmax + kt * P:
                                        hp * skmax + (kt + 1) * P],
                                    qtT[i * DK:(i + 1) * DK,
                                        hp * SQ + pos * 512:
                                        hp * SQ + (pos + 1) * 512],
                                    start=True, stop=True)
                            return st

                        def do_exp(pos, kt, st, pair=None):
                            """pair: ([128,2048] int8 tile, j) for fp8 pair
                            halves; None -> bf16 p_t (masked path)."""
                            on_dve = USE_SCH and exp_dve(pos, kt)
                            if pair is not None:
                                t8, j = pair
                                dst = t8[:, j * 1024:(j + 1) * 1024]
                                if on_dve:
                                    with nc.allow_low_precision("sch exp"):
                                        nc.vector.tensor_scalar(
                                            dst, st[:], SCH_A_F8, SCH_B_F8,
                                            op0=ALU.mult, op1=ALU.add)
                                else:
                                    nc.scalar.activation(
                                        dst.bitcast(FP8), st[:], AFT.Exp,
                                        bias=ln16[:], scale=1.0 / 8.0)
                                return None
                            p_t = psb.tile([P, 1024], I16, tag="p")
                            if on_dve:
                                with nc.allow_low_precision("sch exp"):
                                    nc.vector.tensor_scalar(
                                        p_t[:], st[:], SCH_A_BF, SCH_B_BF,
                                        op0=ALU.mult, op1=ALU.add)
                            else:
                                nc.scalar.activation(
                                    p_t[:].bitcast(BF16), st[:], AFT.Exp,
                                    bias=ln16[:], scale=1.0 / 8.0)
                            return p_t

                        flat = [kt for u in units for kt in u]
                        sts = {flat[0]: emit_score(flat[0])}

                        def prefetch(kt):
                            fi = flat.index(kt) + 1
                            if fi < len(flat):
                                sts[flat[fi]] = emit_score(flat[fi])

                        vat = _r(va[:], "p (t h c) -> p t h c",
                                 t=S // P, h=H)
                        for u in units:
                            if len(u) == 2:
                                k0, k1 = u
                                p2 = psb.tile([P, 2048], I8, tag="p2")
                                for j, kt in enumerate(u):
                                    st = sts.pop(kt)
                                    prefetch(kt)
                                    do_exp(pos, kt, st, pair=(p2, j))
                                p8 = p2[:].bitcast(FP8)
                                for i in range(2):
                                    h = 2 * hp + i
                                    nc.tensor.matmul(
                                        acc[i][0:DK + 2, :],
                                        vat[:, k0:k0 + 2, h, 0:66],
                                        _r(p8, "p (j x) -> p j x", j=2)[
                                            :, :, i * 512:(i + 1) * 512],
                                        start=(k0 == 0),
                                        stop=(k1 == nkt - 1),
                                        perf_mode=DR)
                            else:
                                kt = u[0]
                                st = sts.pop(kt)
                                prefetch(kt)
                                p_t = do_exp(pos, kt, st)
                                pb = p_t[:].bitcast(BF16)
                                mi = masked(pos, kt)
                                if mi is not None:
                                    mt = masks_sb[:, mi * 512:(mi + 1) * 512]
                                    for i in range(2):
                                        nc.vector.tensor_mul(
                                            pb[:, i * 512:(i + 1) * 512],
                                            pb[:, i * 512:(i + 1) * 512],
                                            mt)
                                for i in range(2):
                                    h = 2 * hp + i
                                    nc.tensor.matmul(
                                        acc[i][0:DK + 2, :],
                                        vat[:, kt, h, 0:66],
                                        pb[:, i * 512:(i + 1) * 512],
                                        start=(kt == 0), stop=(kt == nkt - 1))
                        # epilogue: atT8 = 16 * acc/denom (fp8)
                        rcl = []
                        for i in range(2):
                            rc = lnp.tile([1, 512], BF16, tag="ln_xn",
                                          name="rc")
                            with nc.allow_low_precision("softmax denom"):
                                nc.vector.reciprocal(
                                    rc[:], acc[i][DK:DK + 1, :])
                            rcl.append(rc)
                        rbl = []
                        for i in range(2):
                            rbs = lnp.tile([DK, 512], BF16, tag="ln_xn",
                                           name="rbs")
                            nc.gpsimd.partition_broadcast(rbs[:], rcl[i][:])
                            rbl.append(rbs)
                        for i in range(2):
                            with nc.allow_low_precision("fp8 attn out"):
                                nc.vector.tensor_tensor(
                                    atT8[i * DK:(i + 1) * DK,
                                         hp * SQ + pos * 512:
                                         hp * SQ + (pos + 1) * 512],
                                    acc[i][0:DK, :], rbl[i][:],
                                    op=ALU.mult)
                    # output projection + residual for this pos block
                    for qt in range(pos * 4, (pos + 1) * 4):
                        res = resid(qt)
                        ps = scp.tile([P, 1024], F32, tag="score",
                                      name="ps")[:, 0:512]
                        for ep in range(EP):
                            nc.tensor.matmul(
                                ps[:],
                                pv8(atT8)[:, 2 * ep:2 * ep + 2,
                                          qt * P:(qt + 1) * P],
                                pv8(wo8)[:, 2 * ep:2 * ep + 2, 0:D],
                                start=(ep == 0),
                                stop=(ep == EP - 1 and not has_bo),
                                perf_mode=DR)
                            if ep == EP - 1 and has_bo:
                                nc.tensor.matmul(
                                    ps[:], ones1[:, 0:P], bo[:],
                                    start=False, stop=True)
                        tmp = lnp.tile([P, D], F32, tag="ln_xn", name="wot")
                        nc.scalar.activation(tmp[:], ps[:], AFT.Copy,
                                             scale=DS)
                        nc.vector.tensor_tensor(
                            x_out[:, qt * D:(qt + 1) * D], tmp[:],
                            res, op=ALU.add)
                    if on_pos_done is not None:
                        on_pos_done(pos)

            # ---------------- stage A: LN0 + transposes ----------------
            kvT8 = pp.tile([P, ET * S], FP8, tag="kvT")
            ln_transpose(xkv_d, S, kvT8, do_ln=True)

            # ---------------- weights (all upfront, Pool DGE queue) -------
            wk8 = load_w(w_sa["wk"], "w_a")
            wq8 = load_w(w_sa["wq"], "w_b")
            wv8 = load_w(w_sa["wv"], "w_c")
            wo8 = load_w(w_sa["wo"], "w_d")
            ck8 = load_w(w_ca["wk"], "w_e")
            cq8 = load_w(w_ca["wq"], "w_f")
            cv8 = load_w(w_ca["wv"], "w_g")
            co8 = load_w(w_ca["wo"], "w_h")
            w18 = [load_w(w1_d[k], f"w1{k}", cols=FF) for k in "ab"]
            w28 = []
            for k in "ab":
                t = pp.tile([P, (FF // P) * D], FP8, tag=f"w2{k}")
                nc.sync.dma_start(
                    _r(t[:], "p (t d) -> p t d", t=FF // P),
                    _r(w2_d[k][:], "(t p) d -> p t d", p=P))
                w28.append(t)
            bk = load_bias(b_sa["k"], "b_a", bias_flags["sa_k"])
            bq = load_bias(b_sa["q"], "b_b", bias_flags["sa_q"])
            bv = load_bias(b_sa["v"], "b_c", bias_flags["sa_v"])
            bo = load_bias(b_sa["o"], "b_d", bias_flags["sa_o"])
            cbk = load_bias(b_ca["k"], "b_e", bias_flags["ca_k"])
            cbq = load_bias(b_ca["q"], "b_f", bias_flags["ca_q"])
            cbv = load_bias(b_ca["v"], "b_g", bias_flags["ca_v"])
            cbo = load_bias(b_ca["o"], "b_h", bias_flags["ca_o"])
            b1_sb = load_bias(b1_d, "b1", bias_flags["ff1"], n=FF)
            b2_sb = load_bias(b2_d, "b2", bias_flags["ff2"])
            nc.sync.dma_start(
                _r(masks_sb[:], "p (t c) -> p t c", t=16),
                _r(masks_d[:], "t p c -> p t c"))

            # ---------------- stage B: SA projections ----------------
            ktT_sa = pp.tile([P, ET * S], BF16, tag="ktT")
            qtT_sa = pp.tile([P, ET * SQ], BF16, tag="qtT")
            va_sa = pp.tile([P, 16 * H * 66], FP8, tag="va")
            nc.vector.memset(
                _r(va_sa[:], "p (t h c) -> p t h c", t=16, h=H)[:, :, :, DK:DK + 1],
                1.0)
            nc.vector.memset(
                _r(va_sa[:], "p (t h c) -> p t h c", t=16, h=H)[:, :, :, DK + 1:],
                0.0)
            if DEBUG_TAPS:
                nc.sync.dma_start(taps["dbg_kvT8"][:], kvT8[:])
            projT(wk8, bk, bias_flags["sa_k"], kvT8, S, ktT_sa)
            projT(wq8, bq, bias_flags["sa_q"], kvT8, S, qtT_sa,
                  src_qbs=list(Q_SRC_QBS))
            proj_va(wv8, bv, bias_flags["sa_v"], kvT8, S, va_sa)

            if DEBUG_TAPS:
                nc.sync.dma_start(taps["dbg_ktT"][:], ktT_sa[:])
                nc.sync.dma_start(taps["dbg_qtT"][:], qtT_sa[:])
                nc.sync.dma_start(taps["dbg_va"][:], va_sa[:])
            # ---------------- stage C/D: SA attention + Wo ----------------
            x1 = pp.tile([P, (SQ // P) * D], F32, tag="x1")

            def sa_masked(pos, kt):
                return kt if (pos == 0 or kt >= 8) else None

            def sa_exp_dve(pos, kt):
                if sa_masked(pos, kt) is None:
                    return kt % 4 == 1
                return (kt % 8) == 2

            def q_src_row(qt):
                pos, j = divmod(qt, 4)
                return Q_SRC_QBS[pos] * 512 + j * P

            def sa_resid(qt):
                rt = lnp.tile([P, D], F32, tag="ln_x", name="sa_resid")
                r0 = q_src_row(qt)
                nc.sync.dma_start(rt[:], xkv_d[r0:r0 + P, :])
                return rt[:]

            # hoisted CA prep: encoder transpose overlaps SA attention
            encT8 = pp.tile([P, ET * S], FP8, tag="kvT")  # reuse kvT slot
            ln_transpose(enc_d, S, encT8, do_ln=False)

            attention(ktT_sa, va_sa, qtT_sa, [NKT0, NKT1], sa_masked,
                      sa_exp_dve, wo8, bo, bias_flags["sa_o"], sa_resid, x1)

            if DEBUG_TAPS:
                nc.sync.dma_start(taps["dbg_x1"][:], x1[:])
                nc.sync.dma_start(taps["dbg_atT8"][:],
                                  taps_live["atT8"][:])
            # ---------------- stage E/F: CA ----------------
            ktT_ca = pp.tile([P, ET * S], BF16, tag="ktT")
            qtT_ca = pp.tile([P, ET * SQ], BF16, tag="qtT")
            va_ca = pp.tile([P, 16 * H * 66], FP8, tag="va")
            nc.vector.memset(
                _r(va_ca[:], "p (t h c) -> p t h c", t=16, h=H)[:, :, :, DK:DK + 1],
                1.0)
            nc.vector.memset(
                _r(va_ca[:], "p (t h c) -> p t h c", t=16, h=H)[:, :, :, DK + 1:],
                0.0)
            projT(ck8, cbk, bias_flags["ca_k"], encT8, S, ktT_ca)
            proj_va(cv8, cbv, bias_flags["ca_v"], encT8, S, va_ca)

            h1T8 = pp.tile([P, ET * SQ], FP8, tag="hT")
            ln_transpose_sbuf(x1, h1T8)
            projT(cq8, cbq, bias_flags["ca_q"], h1T8, SQ, qtT_ca)

            x2 = pp.tile([P, (SQ // P) * D], F32, tag="x2")

            attention(ktT_ca, va_ca, qtT_ca, [16, 16], lambda p, k: None,
                      lambda p, k: (k % 2 == 1) and (k % 16 != 15), co8,
                      cbo,
                      bias_flags["ca_o"],
                      lambda qt: x1[:, qt * D:(qt + 1) * D], x2)

            if DEBUG_TAPS:
                nc.sync.dma_start(taps["dbg_x2"][:], x2[:])
            # ---------------- stage G: LN2 + FFN ----------------
            h2T8 = pp.tile([P, ET * SQ], FP8, tag="hT")
            ln_transpose_sbuf(x2, h2T8)

            ffT8 = pp.tile([P, (FF // P) * SQ], FP8, tag="ffT")
            for ft in range(FF // P):
                for qb in range(SQ // 512):
                    ps = scp.tile([P, 1024], F32, tag="score",
                                  name="ps")[:, 0:512]
                    for wi, wt in enumerate(w18):
                        for ep in range(EP):
                            last = (wi == 1 and ep == EP - 1)
                            nc.tensor.matmul(
                                ps[:],
                                _r(wt[:], "p (e f) -> p e f", e=ET)[
                                    :, 2 * ep:2 * ep + 2,
                                    ft * P:(ft + 1) * P],
                                pv8(h2T8)[:, 2 * ep:2 * ep + 2,
                                          qb * 512:(qb + 1) * 512],
                                start=(wi == 0 and ep == 0),
                                stop=(last and not bias_flags["ff1"]),
                                perf_mode=DR)
                            if last and bias_flags["ff1"]:
                                nc.tensor.matmul(
                                    ps[:], b1_sb[:, ft * P:(ft + 1) * P],
                                    ones1[:, 0:512].bitcast(F32R),
                                    start=False, stop=True)
                    # ffT8 = 16*relu(z); psum holds 1024*z
                    nc.scalar.activation(
                        ffT8[:, ft * SQ + qb * 512:ft * SQ + (qb + 1) * 512],
                        ps[:], AFT.Relu, scale=XS * DS)
            for qt in range(SQ // P):
                ps = scp.tile([P, 1024], F32, tag="score",
                              name="ps")[:, 0:512]
                nfp = FF // P // 2
                for wi, wt in enumerate(w28):
                    for fp in range(nfp):
                        nc.tensor.matmul(
                            ps[:],
                            _r(ffT8[:], "p (t s) -> p t s", t=FF // P)[
                                :, 2 * fp:2 * fp + 2, qt * P:(qt + 1) * P],
                            _r(wt[:], "p (t d) -> p t d", t=FF // P)[
                                :, 2 * fp:2 * fp + 2, :],
                            start=(wi == 0 and fp == 0),
                            stop=(wi == 1 and fp == nfp - 1
                                  and not bias_flags["ff2"]),
                            perf_mode=DR)
                if bias_flags["ff2"]:
                    nc.tensor.matmul(
                        ps[:], ones1[:, 0:P], b2_sb[:],
                        start=False, stop=True)
                tmp = lnp.tile([P, D], F32, tag="ln_xn", name="ff2t")
                copy_scaled(tmp[:], ps[:], DS, on_act=(qt % 2 == 0))
                o_t = lnp.tile([P, D], F32, tag="ln_xn", name="o_t")
                nc.vector.tensor_tensor(
                    o_t[:], tmp[:], x2[:, qt * D:(qt + 1) * D], op=ALU.add)
                nc.sync.dma_start(out_d[qt * P:(qt + 1) * P, :], o_t[:])

    nc.finalize()
    return nc


_CACHE = {}
LAST_EXEC_NS = None


def kernel(**inputs):
    x = np.asarray(inputs["x"], np.float32)
    enc = np.asarray(inputs["encoder_output"], np.float32)
    src_mask = np.asarray(inputs["src_mask"]).reshape(S)
    tgt_mask = np.asarray(inputs["tgt_mask"]).reshape(S, S)

    def fold(w, g, b, extra_b):
        w = np.asarray(w, np.float32)
        wf = np.asarray(g, np.float32)[:, None] * w
        bf = np.asarray(b, np.float32) @ w + np.asarray(extra_b, np.float32)
        return wf, bf

    def q8(w):
        return np.asarray(w * WS, np.float32).astype(ml_dtypes.float8_e4m3)

    def q8r(w):
        ws = np.asarray(w * WS, np.float32)
        return (ws - ws.astype(ml_dtypes.float8_e4m3)
                .astype(np.float32)).astype(ml_dtypes.float8_e4m3)

    z = np.zeros(D, np.float32)
    sa_wq, bsa_q = fold(inputs["sa_wq"], inputs["ln0_g"], inputs["ln0_b"], z)
    sa_wk, bsa_k = fold(inputs["sa_wk"], inputs["ln0_g"], inputs["ln0_b"], z)
    sa_wv, bsa_v = fold(inputs["sa_wv"], inputs["ln0_g"], inputs["ln0_b"], z)
    sa_wo = np.asarray(inputs["sa_wo"], np.float32)
    bsa_o = np.asarray(inputs["sa_bo"], np.float32)
    ca_wq, bca_q = fold(inputs["ca_wq"], inputs["ln1_g"], inputs["ln1_b"], z)
    ca_wk = np.asarray(inputs["ca_wk"], np.float32)
    bca_k = np.zeros(D, np.float32)
    ca_wv = np.asarray(inputs["ca_wv"], np.float32)
    bca_v = np.zeros(D, np.float32)
    ca_wo = np.asarray(inputs["ca_wo"], np.float32)
    bca_o = np.asarray(inputs["ca_bo"], np.float32)
    w1, b1 = fold(inputs["ff_w1"], inputs["ln2_g"], inputs["ln2_b"],
                  np.asarray(inputs["ff_b1"], np.float32))
    w2 = np.asarray(inputs["ff_w2"], np.float32)
    b2 = np.asarray(inputs["ff_b2"], np.float32)

    bias_flags = {
        "sa_q": bool(np.any(bsa_q)), "sa_k": bool(np.any(bsa_k)),
        "sa_v": bool(np.any(bsa_v)), "sa_o": bool(np.any(bsa_o)),
        "ca_q": bool(np.any(bca_q)), "ca_k": bool(np.any(bca_k)),
        "ca_v": bool(np.any(bca_v)), "ca_o": bool(np.any(bca_o)),
        "ff1": bool(np.any(b1)), "ff2": bool(np.any(b2)),
        "msrc1": bool(np.all(src_mask == 1)),
    }

    key = tuple(sorted(bias_flags.items()))
    if key not in _CACHE:
        _CACHE[key] = build_program(bias_flags)
    nc = _CACHE[key]

    ident = np.eye(P, dtype=np.float32)
    msrc = src_mask.astype(np.float32).reshape(S // P, P).T.copy()
    BS = WS * XS  # bias pre-scale (descaled by DS in the psum copy)

    shared = {
        "ident": ident, "msrc": msrc,
        "sa_wq": q8(sa_wq), "sa_wk": q8(sa_wk), "sa_wv": q8(sa_wv),
        "sa_wo": q8(sa_wo),
        "ca_wq": q8(ca_wq), "ca_wk": q8(ca_wk), "ca_wv": q8(ca_wv),
        "ca_wo": q8(ca_wo),
        "w1a": q8(w1), "w1b": q8r(w1), "w2a": q8(w2), "w2b": q8r(w2),
        "bsa_q": bsa_q[None] * BS, "bsa_k": bsa_k[None] * BS,
        "bsa_v": bsa_v[None] * BS, "bsa_o": bsa_o[None] * BS,
        "bca_q": bca_q[None] * BS, "bca_k": bca_k[None] * BS,
        "bca_v": bca_v[None] * BS, "bca_o": bca_o[None] * BS,
        "b1": b1[None] * BS, "b2": b2[None] * BS,
    }

    in_maps = []
    for c in range(8):
        b, r = divmod(c, 2)
        perm = PERM_BLOCKS[r]
        rows = np.concatenate(
            [np.arange(gb * 512, (gb + 1) * 512) for gb in perm])
        gb0, gb1 = OWN_BLOCKS[r]
        assert perm[0] == gb0 and perm[2] == gb1
        mk = np.zeros((16, P, 512), np.float32)
        for pos, gb in enumerate((gb0, gb1)):
            qs = slice(gb * 512, (gb + 1) * 512)
            mrow = tgt_mask[qs][:, rows]
            for j in range(8):
                kt = j if pos == 0 else 8 + j
                ks = slice(kt * P, (kt + 1) * P)
                mk[pos * 8 + j] = mrow[:, ks].T
            ext = (NKT0 if pos == 0 else NKT1) * P
            assert not np.any(mrow[:, ext:]), "tgt_mask beyond extent"
        im = dict(shared)
        im["xkv"] = np.ascontiguousarray(x[b][rows])
        im["enc"] = np.ascontiguousarray(enc[b])
        im["masks"] = mk.astype(ml_dtypes.bfloat16)
        in_maps.append(im)

    res = run_bass_kernel_spmd(nc, in_maps, core_ids=list(range(8)))
    global LAST_EXEC_NS
    LAST_EXEC_NS = res.exec_time_ns

    out = np.empty((B, S, D), np.float32)
    for c in range(8):
        b, r = divmod(c, 2)
        gb0, gb1 = OWN_BLOCKS[r]
        o = res.results[c]["out"]
        out[b, gb0 * 512:(gb0 + 1) * 512] = o[0:512]
        out[b, gb1 * 512:(gb1 + 1) * 512] = o[512:1024]
    return out


# revision 62
# speedup vs baseline: 1.0343x; 1.0021x over previous
"""Trainium2 Bass kernel for a pre-LN transformer decoder block.

Shapes (hardcoded): B=4, S_TGT=S_SRC=2048, D=512, H=8, DK=64, FF=2048, fp32.

Sharding: 8 cores; core c handles batch c//2. The two cores of a batch split
the 2048 query rows into two causal-balanced groups of 2x512 rows:
  r0: global q-blocks [0:512) and [1536:2048)
  r1: global q-blocks [512:1024) and [1024:1536)
All cores run one identical SPMD program. Keys (and the x rows feeding K/V)
are PERMUTED per core so that the own q-blocks land at canonical positions:
  pi = [own0 | filler0 | own1 | filler1]   (4 blocks of 512 rows)
With this order both ranks see SA extents of 8 k-tiles (pos0) and 16 (pos1),
diagonal mask tiles align, and Q^T is just columns {block0, block2} of the
transposed/normalized x. Per-core visibility is carried by mask DATA built
on the host. Cross-attention is unmasked full-extent.

Precision/layout strategy:
 - Projections / FFN / Wo run as fp8e4m3 DoubleRow matmuls (2 contraction
   rows per pass over e-tile pairs, 4x PE rate vs f32r). Weights are scaled
   x64 and activations x16 into fp8; every PSUM result is descaled by
   2^-10 in its PSUM->SBUF copy (engine-alternated between DVE and Act).
 - K^T/Q^T are bf16 (scores at full PE rate); P is 16*exp(score/8), stored
   fp8 on unmasked k-tile pairs (DoubleRow PV) and bf16 on masked tiles.
   The ones-column of V (=16) carries the softmax denominator; the x16
   cancels in the division.
 - exp alternates between Act (native Exp) and DVE (Schraudolph bit-trick:
   bits = int(A*score + B) reinterpreted as bf16/e4m3), balancing the
   otherwise Act-bound attention spans.
"""

import numpy as np
import ml_dtypes

import concourse.bass as bass
import concourse.bacc as bacc
import concourse.mybir as mybir
import concourse.tile as tile
from concourse.bass_utils import run_bass_kernel_spmd

F32 = mybir.dt.float32
F32R = mybir.dt.float32r
BF16 = mybir.dt.bfloat16
FP8 = mybir.dt.float8e4
I8 = mybir.dt.int8
I16 = mybir.dt.int16
AFT = mybir.ActivationFunctionType
ALU = mybir.AluOpType
AXL = mybir.AxisListType
DR = mybir.MatmulPerfMode.DoubleRow

B, S, D, H, DK, FF = 4, 2048, 512, 8, 64, 2048
P = 128            # partitions
ET = D // P        # 4 e-tiles of 128 over the model dim
EP = ET // 2       # e-tile pairs for DoubleRow
SQ = 1024          # own query rows per core
NKT0, NKT1 = 8, 16  # uniform k-tile extents for SA pos0 / pos1
EPS = 1e-6

WS = 64.0          # fp8 weight scale
XS = 16.0          # fp8 activation scale
DS = 1.0 / (WS * XS)   # descale after a DoubleRow matmul
LN16 = float(np.log(16.0))
LOG2E = 1.4426950408889634
# Schraudolph exp: bits = trunc(score*A + B); B includes the x16 bias
SCH_A_BF = 128.0 * LOG2E / 8.0
SCH_B_BF = (127.0 + 4.0) * 128.0 - 8.0
SCH_A_F8 = 8.0 * LOG2E / 8.0
SCH_B_F8 = (7.0 + 4.0) * 8.0

OWN_BLOCKS = {0: (0, 3), 1: (1, 2)}
PERM_BLOCKS = {0: (0, 1, 3, 2), 1: (1, 0, 2, 3)}
Q_SRC_QBS = (0, 2)
# combined 4-in-1 transpose PSUM (HW-proven); CoreSim's checker rejects it,
# so debugging scripts can flip this off before build.
COMBINED_TP = True
USE_SCH = True     # DVE Schraudolph exp offload
USE_PAIRS = True   # fp8 DoubleRow PV on unmasked k-tile pairs
DEBUG_TAPS = False  # dump intermediates to DRAM for debugging


def _r(ap, pattern, **kw):
    return ap.rearrange(pattern, **kw)


def build_program(bias_flags):
    """Build the SPMD Bass program. bias_flags: dict of bools saying which
    folded biases are nonzero (uniform across cores)."""
    nc = bacc.Bacc("TRN2", target_bir_lowering=False, debug=False, num_devices=8)

    def din(name, shape, dt=F32):
        return nc.dram_tensor(name, shape, dt, kind="ExternalInput").ap()

    xkv_d = din("xkv", [S, D])
    enc_d = din("enc", [S, D])
    masks_d = din("masks", [16, P, 512], BF16)
    msrc_d = din("msrc", [P, S // P])
    ident_d = din("ident", [P, P])
    # fp8 weights, pre-scaled x64
    w_sa = {k: din(f"sa_{k}", [D, D], FP8) for k in ("wq", "wk", "wv", "wo")}
    w_ca = {k: din(f"ca_{k}", [D, D], FP8) for k in ("wq", "wk", "wv", "wo")}
    w1_d = {k: din(f"w1{k}", [D, FF], FP8) for k in "ab"}
    w2_d = {k: din(f"w2{k}", [FF, D], FP8) for k in "ab"}
    # folded biases (pre-scaled x1024), [1, D] / [1, FF]
    b_sa = {k: din(f"bsa_{k}", [1, D]) for k in ("q", "k", "v", "o")}
    b_ca = {k: din(f"bca_{k}", [1, D]) for k in ("q", "k", "v", "o")}
    b1_d = din("b1", [1, FF])
    b2_d = din("b2", [1, D])
    out_d = nc.dram_tensor("out", [SQ, D], F32, kind="ExternalOutput").ap()
    taps = {}
    taps_live = {}
    if DEBUG_TAPS:
        for nm, shape, dt in [
                ("dbg_kvT8", [P, ET * S], FP8),
                ("dbg_ktT", [P, ET * S], BF16),
                ("dbg_qtT", [P, ET * SQ], BF16),
                ("dbg_va", [P, 16 * H * 66], FP8),
                ("dbg_atT8", [P, ET * SQ], FP8),
                ("dbg_x1", [P, (SQ // P) * D], F32),
                ("dbg_x2", [P, (SQ // P) * D], F32)]:
            taps[nm] = nc.dram_tensor(nm, shape, dt,
                                      kind="ExternalOutput").ap()

    with tile.TileContext(nc) as tc:
        with (
            tc.tile_pool(name="persist", bufs=1) as pp,
            tc.tile_pool(name="ln_sb", bufs=5) as lnp,
            tc.tile_pool(name="p_sb", bufs=6) as psb,
            tc.tile_pool(name="ln_st", bufs=4) as stp,
            tc.tile_pool(name="sc_ps", bufs=3, space="PSUM") as scp,
            tc.tile_pool(name="acc_ps", bufs=1, space="PSUM") as accp,
        ):
            ident = pp.tile([P, P], F32R, tag="ident")
            nc.sync.dma_start(ident[:], ident_d[:].bitcast(F32R))
            ones1f = pp.tile([1, P], F32, tag="ones1f")
            nc.vector.memset(ones1f[:], 1.0)
            ones1 = pp.tile([1, P], F32R, tag="ones1")
            nc.vector.tensor_copy(ones1[:], ones1f[:])
            c16_bf = pp.tile([1, P], BF16, tag="c16_bf")
            nc.vector.memset(c16_bf[:], 16.0)
            ln16 = pp.tile([P, 1], F32, tag="ln16")
            nc.vector.memset(ln16[:], LN16)
            msrc = pp.tile([P, S // P], F32, tag="msrc")
            nc.sync.dma_start(msrc[:], msrc_d[:])
            masks_sb = pp.tile([P, 16 * 512], BF16, tag="masks")

            def load_w(dram, name, cols=D):
                # [cin, cols] -> sbuf [128, ET, cols] fp8
                t = pp.tile([P, ET * cols], FP8, tag=name)
                nc.sync.dma_start(
                    _r(t[:], "p (e d) -> p e d", e=ET),
                    _r(dram[:], "(e p) d -> p e d", p=P))
                return t

            def load_bias(dram, name, flag, n=D):
                if not flag:
                    return None
                t = pp.tile([1, n], F32R, tag=name)
                nc.sync.dma_start(t[:], dram[:].bitcast(F32R))
                return t

            def pv8(t8):
                return _r(t8[:], "p (e s) -> p e s", e=ET)

            def copy_scaled(dst, src, c, on_act):
                """PSUM->SBUF copy with scale, engine-balanced."""
                with nc.allow_low_precision("fp8/bf16 staging"):
                    if on_act:
                        nc.scalar.activation(dst, src, AFT.Copy, scale=c)
                    else:
                        nc.vector.tensor_scalar_mul(dst, src, c)

            def ln_stats(x_t):
                """x_t: [128, 512] f32 sbuf -> (scale, bias) per-row [128,1]."""
                sx = stp.tile([P, 1], F32, tag="sx")
                dump = lnp.tile([P, D], F32, tag="ln_xn")
                sq = stp.tile([P, 1], F32, tag="sq")
                nc.scalar.activation(dump[:], x_t[:], AFT.Square,
                                     accum_out=sq[:])
                nc.vector.reduce_sum(sx[:], x_t[:], axis=AXL.X)
                mu = stp.tile([P, 1], F32, tag="mu")
                nc.vector.tensor_scalar_mul(mu[:], sx[:], 1.0 / D)
                m2 = stp.tile([P, 1], F32, tag="m2")
                nc.vector.tensor_mul(m2[:], mu[:], mu[:])
                v1 = stp.tile([P, 1], F32, tag="v1")
                nc.vector.tensor_scalar(v1[:], m2[:], -float(D), None,
                                        op0=ALU.mult)
                nc.vector.tensor_add(v1[:], v1[:], sq[:])
                std = stp.tile([P, 1], F32, tag="std")
                nc.scalar.activation(std[:], v1[:], AFT.Sqrt,
                                     scale=1.0 / (D - 1))
                nc.vector.tensor_scalar_add(std[:], std[:], EPS)
                s = stp.tile([P, 1], F32, tag="s")
                nc.vector.reciprocal(s[:], std[:])
                nb = stp.tile([P, 1], F32, tag="nb")
                nc.vector.tensor_mul(nb[:], mu[:], s[:])
                nc.vector.tensor_scalar_mul(nb[:], nb[:], -1.0)
                return s, nb

            def transpose4(xn, dstT8, rows, t, on_act):
                """Transpose [128, 512] f32r sbuf tile t into dstT8
                [128, ET*rows] fp8 (x16): 4 e-transposes, 1 scaled copy."""
                if COMBINED_TP:
                    ps = scp.tile([P, 1024], F32R, tag="score",
                                  name="tp")[:, 0:512]
                    for e in range(ET):
                        nc.tensor.matmul(
                            ps[:, e * P:(e + 1) * P],
                            xn[:, e * P:(e + 1) * P], ident[:],
                            start=(e == 0), stop=(e == ET - 1),
                            is_transpose=True, skip_group_check=(e != 0))
                    copy_scaled(
                        _r(dstT8[:], "p (e s) -> p e s", e=ET)[
                            :, :, t * P:(t + 1) * P],
                        _r(ps[:], "p (e c) -> p e c", e=ET), XS, on_act)
                else:
                    for e in range(ET):
                        ps = scp.tile([P, 1024], F32R, tag="score",
                                      name="tp")[:, 0:P]
                        nc.tensor.matmul(
                            ps[:], xn[:, e * P:(e + 1) * P], ident[:],
                            start=True, stop=True, is_transpose=True)
                        copy_scaled(
                            _r(dstT8[:], "p (e s) -> p e s", e=ET)[
                                :, e, t * P:(t + 1) * P],
                            ps[:], XS, on_act)

            def ln_transpose(src_d, rows, dstT8, do_ln=True):
                nt = rows // P
                for t in range(nt):
                    if do_ln:
                        x_t = lnp.tile([P, D], F32, tag="ln_x")
                        nc.sync.dma_start(x_t[:], src_d[t * P:(t + 1) * P, :])
                        s, nb = ln_stats(x_t)
                        xn = lnp.tile([P, D], F32R, tag="ln_xn")
                        nc.vector.tensor_scalar(xn[:], x_t[:], s[:], nb[:],
                                                op0=ALU.mult, op1=ALU.add)
                        transpose4(xn[:], dstT8, rows, t,
                                   on_act=(t % 2 == 0))
                    else:
                        xn = lnp.tile([P, D], F32R, tag="ln_x")
                        nc.sync.dma_start(
                            xn[:], src_d[t * P:(t + 1) * P, :].bitcast(F32R))
                        transpose4(xn[:], dstT8, rows, t, on_act=(t % 2 == 0))

            def ln_transpose_sbuf(xsb, dstT8, tiles=None):
                for t in (tiles if tiles is not None else range(SQ // P)):
                    x_t = xsb[:, t * D:(t + 1) * D]
                    s, nb = ln_stats(x_t)
                    xn = lnp.tile([P, D], F32R, tag="ln_xn")
                    nc.vector.tensor_scalar(xn[:], x_t, s[:], nb[:],
                                            op0=ALU.mult, op1=ALU.add)
                    transpose4(xn[:], dstT8, SQ, t, on_act=(t % 2 == 0))

            def projT(wt8, bt, has_b, srcT8, src_rows, dstT, src_qbs=None):
                """dstT[:, dt, :] = bf16 (W.T @ xn.T)-slice via DoubleRow."""
                if src_qbs is None:
                    src_qbs = list(range(src_rows // 512))
                nqb = len(src_qbs)
                for dt in range(ET):
                    for dqb, qb in enumerate(src_qbs):
                        ps = scp.tile([P, 1024], F32, tag="score",
                                      name="ps")[:, 0:512]
                        for ep in range(EP):
                            nc.tensor.matmul(
                                ps[:],
                                pv8(wt8)[:, 2 * ep:2 * ep + 2,
                                         dt * P:(dt + 1) * P],
                                pv8(srcT8)[:, 2 * ep:2 * ep + 2,
                                           qb * 512:(qb + 1) * 512],
                                start=(ep == 0),
                                stop=(ep == EP - 1 and not has_b),
                                perf_mode=DR)
                            if ep == EP - 1 and has_b:
                                nc.tensor.matmul(
                                    ps[:], bt[:, dt * P:(dt + 1) * P],
                                    ones1[:, 0:512].bitcast(F32R),
                                    start=False, stop=True)
                        copy_scaled(
                            dstT[:, dt * nqb * 512 + dqb * 512:
                                 dt * nqb * 512 + (dqb + 1) * 512],
                            ps[:], DS, on_act=((dt + dqb) % 2 == 0))

            def proj_va(wt8, bt, has_b, srcT8, src_rows, va):
                """V projection, token-major: va [128, nkt*8*66] fp8 = 16*V
                (+ src-mask row scaling), ones column = 16."""
                nkt = src_rows // P
                for kt in range(nkt):
                    ps = scp.tile([P, 1024], F32, tag="score",
                                  name="ps")[:, 0:512]
                    for ep in range(EP):
                        nc.tensor.matmul(
                            ps[:],
                            pv8(srcT8)[:, 2 * ep:2 * ep + 2,
                                       kt * P:(kt + 1) * P],
                            pv8(wt8)[:, 2 * ep:2 * ep + 2, 0:D],
                            start=(ep == 0),
                            stop=(ep == EP - 1 and not has_b),
                            perf_mode=DR)
                        if ep == EP - 1 and has_b:
                            nc.tensor.matmul(
                                ps[:], ones1[:, 0:P], bt[:],
                                start=False, stop=True)
                    dst = _r(va[:], "p (t h c) -> p t h c", t=nkt, h=H)
                    if bias_flags.get("msrc1"):
                        copy_scaled(dst[:, kt, :, 0:DK],
                                    _r(ps[:], "p (h c) -> p h c", h=H),
                                    XS * DS, on_act=(kt % 2 == 0))
                    else:
                        with nc.allow_low_precision("fp8 va"):
                            nc.vector.tensor_scalar(
                                dst[:, kt, :, 0:DK],
                                _r(ps[:], "p (h c) -> p h c", h=H),
                                msrc[:, kt:kt + 1], XS * DS,
                                op0=ALU.mult, op1=ALU.mult)

            def attention(ktT, va, qtT, nkts, masked, exp_dve, wo8, bo,
                          has_bo, resid, x_out, on_pos_done=None):
                """ktT [128, 4*S_k] bf16; va [128, nkt*8*66] fp8 (16*V);
                qtT [128, 4*1024] bf16; masked: fn(pos,kt)-> mask idx|None;
                exp_dve: fn(pos,kt)->bool; x_out [128,8*512] f32 resid+attn."""
                skmax = max(nkts) * P
                atT8 = pp.tile([P, ET * SQ], FP8, tag="attnT")
                taps_live["atT8"] = atT8
                for pos, nkt in enumerate(nkts):
                    units = []
                    kt = 0
                    while kt < nkt:
                        if (USE_PAIRS and masked(pos, kt) is None
                                and kt + 1 < nkt
                                and masked(pos, kt + 1) is None):
                            units.append((kt, kt + 1))
                            kt += 2
                        else:
                            units.append((kt,))
                            kt += 1
                    for hp in range(H // 2):
                        acc = [accp.tile([P, 512], F32, tag=f"acc{i}",
                                         name=f"acc{i}")
                               for i in range(2)]

                        def emit_score(kt):
                            st = scp.tile([P, 1024], F32, tag="score")
                            for i in range(2):
                                nc.tensor.matmul(
                                    st[:, i * 512:(i + 1) * 512],
                                    ktT[i * DK:(i + 1) * DK,
                                        hp * skmax + kt * P:
                                        hp * skmax + (kt + 1) * P],
                                    qtT[i * DK:(i + 1) * DK,
                                        hp * SQ + pos * 512:
                                        hp * SQ + (pos + 1) * 512],
                                    start=True, stop=True)
                            return st

                        def do_exp(pos, kt, st, pair=None):
                            """pair: ([128,2048] int8 tile, j) for fp8 pair
                            halves; None -> bf16 p_t (masked path)."""
                            on_dve = USE_SCH and exp_dve(pos, kt)
                            if pair is not None:
                                t8, j = pair
                                dst = t8[:, j * 1024:(j + 1) * 1024]
                                if on_dve:
                                    with nc.allow_low_precision("sch exp"):
                                        nc.vector.tensor_scalar(
                                            dst, st[:], SCH_A_F8, SCH_B_F8,
                                            op0=ALU.mult, op1=ALU.add)
                                else:
                                    nc.scalar.activation(
                                        dst.bitcast(FP8), st[:], AFT.Exp,
                                        bias=ln16[:], scale=1.0 / 8.0)
                                return None
                            p_t = psb.tile([P, 1024], I16, tag="p")
                            if on_dve:
                                with nc.allow_low_precision("sch exp"):
                                    nc.vector.tensor_scalar(
                                        p_t[:], st[:], SCH_A_BF, SCH_B_BF,
                                        op0=ALU.mult, op1=ALU.add)
                            else:
                                nc.scalar.activation(
                                    p_t[:].bitcast(BF16), st[:], AFT.Exp,
                                    bias=ln16[:], scale=1.0 / 8.0)
                            return p_t

                        flat = [kt for u in units for kt in u]
                        sts = {flat[0]: emit_score(flat[0])}

                        def prefetch(kt):
                            fi = flat.index(kt) + 1
                            if fi < len(flat):
                                sts[flat[fi]] = emit_score(flat[fi])

                        vat = _r(va[:], "p (t h c) -> p t h c",
                                 t=S // P, h=H)
                        for u in units:
                            if len(u) == 2:
                                k0, k1 = u
                                p2 = psb.tile([P, 2048], I8, tag="p2")
                                for j, kt in enumerate(u):
                                    st = sts.pop(kt)
                                    prefetch(kt)
                                    do_exp(pos, kt, st, pair=(p2, j))
                                p8 = p2[:].bitcast(FP8)
                                for i in range(2):
                                    h = 2 * hp + i
                                    nc.tensor.matmul(
                                        acc[i][0:DK + 2, :],
                                        vat[:, k0:k0 + 2, h, 0:66],
                                        _r(p8, "p (j x) -> p j x", j=2)[
                                            :, :, i * 512:(i + 1) * 512],
                                        start=(k0 == 0),
                                        stop=(k1 == nkt - 1),
                                        perf_mode=DR)
                            else:
                                kt = u[0]
                                st = sts.pop(kt)
                                prefetch(kt)
                                p_t = do_exp(pos, kt, st)
                                pb = p_t[:].bitcast(BF16)
                                mi = masked(pos, kt)
                                if mi is not None:
                                    mt = masks_sb[:, mi * 512:(mi + 1) * 512]
                                    for i in range(2):
                                        nc.vector.tensor_mul(
                                            pb[:, i * 512:(i + 1) * 512],
                                            pb[:, i * 512:(i + 1) * 512],
                                            mt)
                                for i in range(2):
                                    h = 2 * hp + i
                                    nc.tensor.matmul(
                                        acc[i][0:DK + 2, :],
                                        vat[:, kt, h, 0:66],
                                        pb[:, i * 512:(i + 1) * 512],
                                        start=(kt == 0), stop=(kt == nkt - 1))
                        # epilogue: atT8 = 16 * acc/denom (fp8)
                        rcl = []
                        for i in range(2):
                            rc = lnp.tile([1, 512], BF16, tag="ln_xn",
                                          name="rc")
                            with nc.allow_low_precision("softmax denom"):
                                nc.vector.reciprocal(
                                    rc[:], acc[i][DK:DK + 1, :])
                            rcl.append(rc)
                        rbl = []
                        for i in range(2):
                            rbs = lnp.tile([DK, 512], BF16, tag="ln_xn",
                                           name="rbs")
                            nc.gpsimd.partition_broadcast(rbs[:], rcl[i][:])
                            rbl.append(rbs)
                        for i in range(2):
                            with nc.allow_low_precision("fp8 attn out"):
                                nc.vector.tensor_tensor(
                                    atT8[i * DK:(i + 1) * DK,
                                         hp * SQ + pos * 512:
                                         hp * SQ + (pos + 1) * 512],
                                    acc[i][0:DK, :], rbl[i][:],
                                    op=ALU.mult)
                    # output projection + residual for this pos block
                    for qt in range(pos * 4, (pos + 1) * 4):
                        res = resid(qt)
                        ps = scp.tile([P, 1024], F32, tag="score",
                                      name="ps")[:, 0:512]
                        for ep in range(EP):
                            nc.tensor.matmul(
                                ps[:],
                                pv8(atT8)[:, 2 * ep:2 * ep + 2,
                                          qt * P:(qt + 1) * P],
                                pv8(wo8)[:, 2 * ep:2 * ep + 2, 0:D],
                                start=(ep == 0),
                                stop=(ep == EP - 1 and not has_bo),
                                perf_mode=DR)
                            if ep == EP - 1 and has_bo:
                                nc.tensor.matmul(
                                    ps[:], ones1[:, 0:P], bo[:],
                                    start=False, stop=True)
                        tmp = lnp.tile([P, D], F32, tag="ln_xn", name="wot")
                        nc.scalar.activation(tmp[:], ps[:], AFT.Copy,
                                             scale=DS)
                        nc.vector.tensor_tensor(
                            x_out[:, qt * D:(qt + 1) * D], tmp[:],
                            res, op=ALU.add)
                    if on_pos_done is not None:
                        on_pos_done(pos)

            # ---------------- stage A: LN0 + transposes ----------------
            kvT8 = pp.tile([P, ET * S], FP8, tag="kvT")
            ln_transpose(xkv_d, S, kvT8, do_ln=True)

            # ---------------- weights (all upfront, Pool DGE queue) -------
            wk8 = load_w(w_sa["wk"], "w_a")
            wq8 = load_w(w_sa["wq"], "w_b")
            wv8 = load_w(w_sa["wv"], "w_c")
            wo8 = load_w(w_sa["wo"], "w_d")
            ck8 = load_w(w_ca["wk"], "w_e")
            cq8 = load_w(w_ca["wq"], "w_f")
            cv8 = load_w(w_ca["wv"], "w_g")
            co8 = load_w(w_ca["wo"], "w_h")
            w18 = [load_w(w1_d[k], f"w1{k}", cols=FF) for k in "ab"]
            w28 = []
            for k in "ab":
                t = pp.tile([P, (FF // P) * D], FP8, tag=f"w2{k}")
                nc.sync.dma_start(
                    _r(t[:], "p (t d) -> p t d", t=FF // P),
                    _r(w2_d[k][:], "(t p) d -> p t d", p=P))
                w28.append(t)
            bk = load_bias(b_sa["k"], "b_a", bias_flags["sa_k"])
            bq = load_bias(b_sa["q"], "b_b", bias_flags["sa_q"])
            bv = load_bias(b_sa["v"], "b_c", bias_flags["sa_v"])
            bo = load_bias(b_sa["o"], "b_d", bias_flags["sa_o"])
            cbk = load_bias(b_ca["k"], "b_e", bias_flags["ca_k"])
            cbq = load_bias(b_ca["q"], "b_f", bias_flags["ca_q"])
            cbv = load_bias(b_ca["v"], "b_g", bias_flags["ca_v"])
            cbo = load_bias(b_ca["o"], "b_h", bias_flags["ca_o"])
            b1_sb = load_bias(b1_d, "b1", bias_flags["ff1"], n=FF)
            b2_sb = load_bias(b2_d, "b2", bias_flags["ff2"])
            nc.sync.dma_start(
                _r(masks_sb[:], "p (t c) -> p t c", t=16),
                _r(masks_d[:], "t p c -> p t c"))

            # ---------------- stage B: SA projections ----------------
            ktT_sa = pp.tile([P, ET * S], BF16, tag="ktT")
            qtT_sa = pp.tile([P, ET * SQ], BF16, tag="qtT")
            va_sa = pp.tile([P, 16 * H * 66], FP8, tag="va")
            nc.vector.memset(
                _r(va_sa[:], "p (t h c) -> p t h c", t=16, h=H)[:, :, :, DK:DK + 1],
                1.0)
            nc.vector.memset(
                _r(va_sa[:], "p (t h c) -> p t h c", t=16, h=H)[:, :, :, DK + 1:],
                0.0)
            if DEBUG_TAPS:
                nc.sync.dma_start(taps["dbg_kvT8"][:], kvT8[:])
            projT(wk8, bk, bias_flags["sa_k"], kvT8, S, ktT_sa)
            projT(wq8, bq, bias_flags["sa_q"], kvT8, S, qtT_sa,
                  src_qbs=list(Q_SRC_QBS))
            proj_va(wv8, bv, bias_flags["sa_v"], kvT8, S, va_sa)

            if DEBUG_TAPS:
                nc.sync.dma_start(taps["dbg_ktT"][:], ktT_sa[:])
                nc.sync.dma_start(taps["dbg_qtT"][:], qtT_sa[:])
                nc.sync.dma_start(taps["dbg_va"][:], va_sa[:])
            # ---------------- stage C/D: SA attention + Wo ----------------
            x1 = pp.tile([P, (SQ // P) * D], F32, tag="x1")

            def sa_masked(pos, kt):
                return kt if (pos == 0 or kt >= 8) else None

            def sa_exp_dve(pos, kt):
                if sa_masked(pos, kt) is None:
                    return kt % 4 == 1
                return (kt % 8) == 2

            def q_src_row(qt):
                pos, j = divmod(qt, 4)
                return Q_SRC_QBS[pos] * 512 + j * P

            def sa_resid(qt):
                rt = lnp.tile([P, D], F32, tag="ln_x", name="sa_resid")
                r0 = q_src_row(qt)
                nc.sync.dma_start(rt[:], xkv_d[r0:r0 + P, :])
                return rt[:]

            # hoisted CA prep: encoder transpose overlaps SA attention
            encT8 = pp.tile([P, ET * S], FP8, tag="kvT")  # reuse kvT slot
            ln_transpose(enc_d, S, encT8, do_ln=False)

            attention(ktT_sa, va_sa, qtT_sa, [NKT0, NKT1], sa_masked,
                      sa_exp_dve, wo8, bo, bias_flags["sa_o"], sa_resid, x1)

            if DEBUG_TAPS:
                nc.sync.dma_start(taps["dbg_x1"][:], x1[:])
                nc.sync.dma_start(taps["dbg_atT8"][:],
                                  taps_live["atT8"][:])
            # ---------------- stage E/F: CA ----------------
            ktT_ca = pp.tile([P, ET * S], BF16, tag="ktT")
            qtT_ca = pp.tile([P, ET * SQ], BF16, tag="qtT")
            va_ca = pp.tile([P, 16 * H * 66], FP8, tag="va")
            nc.vector.memset(
                _r(va_ca[:], "p (t h c) -> p t h c", t=16, h=H)[:, :, :, DK:DK + 1],
                1.0)
            nc.vector.memset(
                _r(va_ca[:], "p (t h c) -> p t h c", t=16, h=H)[:, :, :, DK + 1:],
                0.0)
            projT(ck8, cbk, bias_flags["ca_k"], encT8, S, ktT_ca)
            proj_va(cv8, cbv, bias_flags["ca_v"], encT8, S, va_ca)

            h1T8 = pp.tile([P, ET * SQ], FP8, tag="hT")
            ln_transpose_sbuf(x1, h1T8)
            projT(cq8, cbq, bias_flags["ca_q"], h1T8, SQ, qtT_ca)

            x2 = pp.tile([P, (SQ // P) * D], F32, tag="x2")

            attention(ktT_ca, va_ca, qtT_ca, [16, 16], lambda p, k: None,
                      lambda p, k: (k % 2 == 1) and (k % 16 != 15), co8,
                      cbo,
                      bias_flags["ca_o"],
                      lambda qt: x1[:, qt * D:(qt + 1) * D], x2)

            if DEBUG_TAPS:
                nc.sync.dma_start(taps["dbg_x2"][:], x2[:])
            # ---------------- stage G: LN2 + FFN ----------------
            h2T8 = pp.tile([P, ET * SQ], FP8, tag="hT")
            ln_transpose_sbuf(x2, h2T8)

            ffT8 = pp.tile([P, (FF // P) * SQ], FP8, tag="ffT")

            def ffn2_qt(qt):
                ps = scp.tile([P, 1024], F32, tag="score",
                              name="ps")[:, 0:512]
                nfp = FF // P // 2
                for wi, wt in enumerate(w28):
                    for fp in range(nfp):
                        nc.tensor.matmul(
                            ps[:],
                            _r(ffT8[:], "p (t s) -> p t s", t=FF // P)[
                                :, 2 * fp:2 * fp + 2, qt * P:(qt + 1) * P],
                            _r(wt[:], "p (t d) -> p t d", t=FF // P)[
                                :, 2 * fp:2 * fp + 2, :],
                            start=(wi == 0 and fp == 0),
                            stop=(wi == 1 and fp == nfp - 1
                                  and not bias_flags["ff2"]),
                            perf_mode=DR)
                if bias_flags["ff2"]:
                    nc.tensor.matmul(
                        ps[:], ones1[:, 0:P], b2_sb[:],
                        start=False, stop=True)
                tmp = lnp.tile([P, D], F32, tag="ln_xn", name="ff2t")
                copy_scaled(tmp[:], ps[:], DS, on_act=(qt % 2 == 0))
                o_t = lnp.tile([P, D], F32, tag="ln_xn", name="o_t")
                nc.vector.tensor_tensor(
                    o_t[:], tmp[:], x2[:, qt * D:(qt + 1) * D], op=ALU.add)
                nc.sync.dma_start(out_d[qt * P:(qt + 1) * P, :], o_t[:])

            for qb in range(SQ // 512):
                if qb == 1:
                    for qt in range(4):
                        ffn2_qt(qt)
                for ft in range(FF // P):
                    ps = scp.tile([P, 1024], F32, tag="score",
                                  name="ps")[:, 0:512]
                    for wi, wt in enumerate(w18):
                        for ep in range(EP):
                            last = (wi == 1 and ep == EP - 1)
                            nc.tensor.matmul(
                                ps[:],
                                _r(wt[:], "p (e f) -> p e f", e=ET)[
                                    :, 2 * ep:2 * ep + 2,
                                    ft * P:(ft + 1) * P],
                                pv8(h2T8)[:, 2 * ep:2 * ep + 2,
                                          qb * 512:(qb + 1) * 512],
                                start=(wi == 0 and ep == 0),
                                stop=(last and not bias_flags["ff1"]),
                                perf_mode=DR)
                            if last and bias_flags["ff1"]:
                                nc.tensor.matmul(
                                    ps[:], b1_sb[:, ft * P:(ft + 1) * P],
                                    ones1[:, 0:512].bitcast(F32R),
                                    start=False, stop=True)
                    # ffT8 = 16*relu(z); psum holds 1024*z
                    nc.scalar.activation(
                        ffT8[:, ft * SQ + qb * 512:ft * SQ + (qb + 1) * 512],
                        ps[:], AFT.Relu, scale=XS * DS)
            for qt in range(4, SQ // P):
                ffn2_qt(qt)

    nc.finalize()
    return nc


_CACHE = {}
LAST_EXEC_NS = None


def kernel(**inputs):
    x = np.asarray(inputs["x"], np.float32)
    enc = np.asarray(inputs["encoder_output"], np.float32)
    src_mask = np.asarray(inputs["src_mask"]).reshape(S)
    tgt_mask = np.asarray(inputs["tgt_mask"]).reshape(S, S)

    def fold(w, g, b, extra_b):
        w = np.asarray(w, np.float32)
        wf = np.asarray(g, np.float32)[:, None] * w
        bf = np.asarray(b, np.float32) @ w + np.asarray(extra_b, np.float32)
        return wf, bf

    def q8(w):
        return np.asarray(w * WS, np.float32).astype(ml_dtypes.float8_e4m3)

    def q8r(w):
        ws = np.asarray(w * WS, np.float32)
        return (ws - ws.astype(ml_dtypes.float8_e4m3)
                .astype(np.float32)).astype(ml_dtypes.float8_e4m3)

    z = np.zeros(D, np.float32)
    sa_wq, bsa_q = fold(inputs["sa_wq"], inputs["ln0_g"], inputs["ln0_b"], z)
    sa_wk, bsa_k = fold(inputs["sa_wk"], inputs["ln0_g"], inputs["ln0_b"], z)
    sa_wv, bsa_v = fold(inputs["sa_wv"], inputs["ln0_g"], inputs["ln0_b"], z)
    sa_wo = np.asarray(inputs["sa_wo"], np.float32)
    bsa_o = np.asarray(inputs["sa_bo"], np.float32)
    ca_wq, bca_q = fold(inputs["ca_wq"], inputs["ln1_g"], inputs["ln1_b"], z)
    ca_wk = np.asarray(inputs["ca_wk"], np.float32)
    bca_k = np.zeros(D, np.float32)
    ca_wv = np.asarray(inputs["ca_wv"], np.float32)
    bca_v = np.zeros(D, np.float32)
    ca_wo = np.asarray(inputs["ca_wo"], np.float32)
    bca_o = np.asarray(inputs["ca_bo"], np.float32)
    w1, b1 = fold(inputs["ff_w1"], inputs["ln2_g"], inputs["ln2_b"],
                  np.asarray(inputs["ff_b1"], np.float32))
    w2 = np.asarray(inputs["ff_w2"], np.float32)
    b2 = np.asarray(inputs["ff_b2"], np.float32)

    bias_flags = {
        "sa_q": bool(np.any(bsa_q)), "sa_k": bool(np.any(bsa_k)),
        "sa_v": bool(np.any(bsa_v)), "sa_o": bool(np.any(bsa_o)),
        "ca_q": bool(np.any(bca_q)), "ca_k": bool(np.any(bca_k)),
        "ca_v": bool(np.any(bca_v)), "ca_o": bool(np.any(bca_o)),
        "ff1": bool(np.any(b1)), "ff2": bool(np.any(b2)),
        "msrc1": bool(np.all(src_mask == 1)),
    }

    key = tuple(sorted(bias_flags.items()))
    if key not in _CACHE:
        _CACHE[key] = build_program(bias_flags)
    nc = _CACHE[key]

    ident = np.eye(P, dtype=np.float32)
    msrc = src_mask.astype(np.float32).reshape(S // P, P).T.copy()
    BS = WS * XS  # bias pre-scale (descaled by DS in the psum copy)

    shared = {
        "ident": ident, "msrc": msrc,
        "sa_wq": q8(sa_wq), "sa_wk": q8(sa_wk), "sa_wv": q8(sa_wv),
        "sa_wo": q8(sa_wo),
        "ca_wq": q8(ca_wq), "ca_wk": q8(ca_wk), "ca_wv": q8(ca_wv),
        "ca_wo": q8(ca_wo),
        "w1a": q8(w1), "w1b": q8r(w1), "w2a": q8(w2), "w2b": q8r(w2),
        "bsa_q": bsa_q[None] * BS, "bsa_k": bsa_k[None] * BS,
        "bsa_v": bsa_v[None] * BS, "bsa_o": bsa_o[None] * BS,
        "bca_q": bca_q[None] * BS, "bca_k": bca_k[None] * BS,
        "bca_v": bca_v[None] * BS, "bca_o": bca_o[None] * BS,
        "b1": b1[None] * BS, "b2": b2[None] * BS,
    }

    in_maps = []
    for c in range(8):
        b, r = divmod(c, 2)
        perm = PERM_BLOCKS[r]
        rows = np.concatenate(
            [np.arange(gb * 512, (gb + 1) * 512) for gb in perm])
        gb0, gb1 = OWN_BLOCKS[r]
        assert perm[0] == gb0 and perm[2] == gb1
        mk = np.zeros((16, P, 512), np.float32)
        for pos, gb in enumerate((gb0, gb1)):
            qs = slice(gb * 512, (gb + 1) * 512)
            mrow = tgt_mask[qs][:, rows]
            for j in range(8):
                kt = j if pos == 0 else 8 + j
                ks = slice(kt * P, (kt + 1) * P)
                mk[pos * 8 + j] = mrow[:, ks].T
            ext = (NKT0 if pos == 0 else NKT1) * P
            assert not np.any(mrow[:, ext:]), "tgt_mask beyond extent"
        im = dict(shared)
        im["xkv"] = np.ascontiguousarray(x[b][rows])
        im["enc"] = np.ascontiguousarray(enc[b])
        im["masks"] = mk.astype(ml_dtypes.bfloat16)
        in_maps.append(im)

    res = run_bass_kernel_spmd(nc, in_maps, core_ids=list(range(8)))
    global LAST_EXEC_NS
    LAST_EXEC_NS = res.exec_time_ns

    out = np.empty((B, S, D), np.float32)
    for c in range(8):
        b, r = divmod(c, 2)
        gb0, gb1 = OWN_BLOCKS[r]
        o = res.results[c]["out"]
        out[b, gb0 * 512:(gb0 + 1) * 512] = o[0:512]
        out[b, gb1 * 512:(gb1 + 1) * 512] = o[512:1024]
    return out


# revision 74
# speedup vs baseline: 1.0438x; 1.0091x over previous
"""Trainium2 Bass kernel for a pre-LN transformer decoder block.

Shapes (hardcoded): B=4, S_TGT=S_SRC=2048, D=512, H=8, DK=64, FF=2048, fp32.

Sharding: 8 cores; core c handles batch c//2. The two cores of a batch split
the 2048 query rows into two causal-balanced groups of 2x512 rows:
  r0: global q-blocks [0:512) and [1536:2048)
  r1: global q-blocks [512:1024) and [1024:1536)
All cores run one identical SPMD program. Keys (and the x rows feeding K/V)
are PERMUTED per core so that the own q-blocks land at canonical positions:
  pi = [own0 | filler0 | own1 | filler1]   (4 blocks of 512 rows)
With this order both ranks see SA extents of 8 k-tiles (pos0) and 16 (pos1),
diagonal mask tiles align, and Q^T is just columns {block0, block2} of the
transposed/normalized x. Per-core visibility is carried by mask DATA built
on the host. Cross-attention is unmasked full-extent.

Precision/layout strategy:
 - Projections / FFN / Wo run as fp8e4m3 DoubleRow matmuls (2 contraction
   rows per pass over e-tile pairs, 4x PE rate vs f32r). Weights are scaled
   x64 and activations x16 into fp8; every PSUM result is descaled by
   2^-10 in its PSUM->SBUF copy (engine-alternated between DVE and Act).
 - K^T/Q^T are bf16 (scores at full PE rate); P is 16*exp(score/8), stored
   fp8 on unmasked k-tile pairs (DoubleRow PV) and bf16 on masked tiles.
   The ones-column of V (=16) carries the softmax denominator; the x16
   cancels in the division.
 - exp alternates between Act (native Exp) and DVE (Schraudolph bit-trick:
   bits = int(A*score + B) reinterpreted as bf16/e4m3), balancing the
   otherwise Act-bound attention spans.
"""

import numpy as np
import ml_dtypes

import concourse.bass as bass
import concourse.bacc as bacc
import concourse.mybir as mybir
import concourse.tile as tile
from concourse.bass_utils import run_bass_kernel_spmd

F32 = mybir.dt.float32
F32R = mybir.dt.float32r
BF16 = mybir.dt.bfloat16
FP8 = mybir.dt.float8e4
I8 = mybir.dt.int8
I16 = mybir.dt.int16
AFT = mybir.ActivationFunctionType
ALU = mybir.AluOpType
AXL = mybir.AxisListType
DR = mybir.MatmulPerfMode.DoubleRow

B, S, D, H, DK, FF = 4, 2048, 512, 8, 64, 2048
P = 128            # partitions
ET = D // P        # 4 e-tiles of 128 over the model dim
EP = ET // 2       # e-tile pairs for DoubleRow
SQ = 1024          # own query rows per core
NKT0, NKT1 = 8, 16  # uniform k-tile extents for SA pos0 / pos1
EPS = 1e-6

WS = 64.0          # fp8 weight scale
XS = 16.0          # fp8 activation scale
DS = 1.0 / (WS * XS)   # descale after a DoubleRow matmul
LN16 = float(np.log(16.0))
LOG2E = 1.4426950408889634
# Schraudolph exp: bits = trunc(score*A + B); B includes the x16 bias
SCH_A_BF = 128.0 * LOG2E / 8.0
SCH_B_BF = (127.0 + 4.0) * 128.0 - 8.0
SCH_A_F8 = 8.0 * LOG2E / 8.0
SCH_B_F8 = (7.0 + 4.0) * 8.0

OWN_BLOCKS = {0: (0, 3), 1: (1, 2)}
PERM_BLOCKS = {0: (0, 1, 3, 2), 1: (1, 0, 2, 3)}
Q_SRC_QBS = (0, 2)
# combined 4-in-1 transpose PSUM (HW-proven); CoreSim's checker rejects it,
# so debugging scripts can flip this off before build.
COMBINED_TP = True
USE_SCH = True     # DVE Schraudolph exp offload
USE_PAIRS = True   # fp8 DoubleRow PV on unmasked k-tile pairs
DEBUG_TAPS = False  # dump intermediates to DRAM for debugging


def _r(ap, pattern, **kw):
    return ap.rearrange(pattern, **kw)


def build_program(bias_flags):
    """Build the SPMD Bass program. bias_flags: dict of bools saying which
    folded biases are nonzero (uniform across cores)."""
    nc = bacc.Bacc("TRN2", target_bir_lowering=False, debug=False, num_devices=8)

    def din(name, shape, dt=F32):
        return nc.dram_tensor(name, shape, dt, kind="ExternalInput").ap()

    xkv_d = din("xkv", [S, D])
    enc_d = din("enc", [S, D])
    masks_d = din("masks", [16, P, 512], BF16)
    msrc_d = din("msrc", [P, S // P])
    ident_d = din("ident", [P, P])
    # fp8 weights, pre-scaled x64
    w_sa = {k: din(f"sa_{k}", [D, D], FP8) for k in ("wq", "wk", "wv", "wo")}
    w_ca = {k: din(f"ca_{k}", [D, D], FP8) for k in ("wq", "wk", "wv", "wo")}
    w1_d = {k: din(f"w1{k}", [D, FF], FP8) for k in "ab"}
    w2_d = {k: din(f"w2{k}", [FF, D], FP8) for k in "ab"}
    # folded biases (pre-scaled x1024), [1, D] / [1, FF]
    b_sa = {k: din(f"bsa_{k}", [1, D]) for k in ("q", "k", "v", "o")}
    b_ca = {k: din(f"bca_{k}", [1, D]) for k in ("q", "k", "v", "o")}
    b1_d = din("b1", [1, FF])
    b2_d = din("b2", [1, D])
    out_d = nc.dram_tensor("out", [SQ, D], F32, kind="ExternalOutput").ap()
    taps = {}
    taps_live = {}
    if DEBUG_TAPS:
        for nm, shape, dt in [
                ("dbg_kvT8", [P, ET * S], FP8),
                ("dbg_ktT", [P, ET * S], BF16),
                ("dbg_qtT", [P, ET * SQ], BF16),
                ("dbg_va", [P, 16 * H * 66], FP8),
                ("dbg_atT8", [P, ET * SQ], FP8),
                ("dbg_x1", [P, (SQ // P) * D], F32),
                ("dbg_x2", [P, (SQ // P) * D], F32)]:
            taps[nm] = nc.dram_tensor(nm, shape, dt,
                                      kind="ExternalOutput").ap()

    with tile.TileContext(nc) as tc:
        with (
            tc.tile_pool(name="persist", bufs=1) as pp,
            tc.tile_pool(name="ln_sb", bufs=5) as lnp,
            tc.tile_pool(name="p_sb", bufs=6) as psb,
            tc.tile_pool(name="ln_st", bufs=4) as stp,
            tc.tile_pool(name="sc_ps", bufs=3, space="PSUM") as scp,
            tc.tile_pool(name="acc_ps", bufs=1, space="PSUM") as accp,
        ):
            ident = pp.tile([P, P], F32R, tag="ident")
            nc.sync.dma_start(ident[:], ident_d[:].bitcast(F32R))
            ones1f = pp.tile([1, P], F32, tag="ones1f")
            nc.vector.memset(ones1f[:], 1.0)
            ones1 = pp.tile([1, P], F32R, tag="ones1")
            nc.vector.tensor_copy(ones1[:], ones1f[:])
            c16_bf = pp.tile([1, P], BF16, tag="c16_bf")
            nc.vector.memset(c16_bf[:], 16.0)
            ln16 = pp.tile([P, 1], F32, tag="ln16")
            nc.vector.memset(ln16[:], LN16)
            msrc = pp.tile([P, S // P], F32, tag="msrc")
            nc.sync.dma_start(msrc[:], msrc_d[:])
            masks_sb = pp.tile([P, 16 * 512], BF16, tag="masks")

            def load_w(dram, name, cols=D):
                # [cin, cols] -> sbuf [128, ET, cols] fp8
                t = pp.tile([P, ET * cols], FP8, tag=name)
                nc.sync.dma_start(
                    _r(t[:], "p (e d) -> p e d", e=ET),
                    _r(dram[:], "(e p) d -> p e d", p=P))
                return t

            def load_bias(dram, name, flag, n=D):
                if not flag:
                    return None
                t = pp.tile([1, n], F32R, tag=name)
                nc.sync.dma_start(t[:], dram[:].bitcast(F32R))
                return t

            def pv8(t8):
                return _r(t8[:], "p (e s) -> p e s", e=ET)

            def copy_scaled(dst, src, c, on_act):
                """PSUM->SBUF copy with scale, engine-balanced."""
                with nc.allow_low_precision("fp8/bf16 staging"):
                    if on_act:
                        nc.scalar.activation(dst, src, AFT.Copy, scale=c)
                    else:
                        nc.vector.tensor_scalar_mul(dst, src, c)

            def ln_stats(x_t):
                """x_t: [128, 512] f32 sbuf -> (scale, bias) per-row [128,1]."""
                sx = stp.tile([P, 1], F32, tag="sx")
                dump = lnp.tile([P, D], F32, tag="ln_xn")
                sq = stp.tile([P, 1], F32, tag="sq")
                nc.scalar.activation(dump[:], x_t[:], AFT.Square,
                                     accum_out=sq[:])
                nc.vector.reduce_sum(sx[:], x_t[:], axis=AXL.X)
                mu = stp.tile([P, 1], F32, tag="mu")
                nc.vector.tensor_scalar_mul(mu[:], sx[:], 1.0 / D)
                m2 = stp.tile([P, 1], F32, tag="m2")
                nc.vector.tensor_mul(m2[:], mu[:], mu[:])
                v1 = stp.tile([P, 1], F32, tag="v1")
                nc.vector.tensor_scalar(v1[:], m2[:], -float(D), None,
                                        op0=ALU.mult)
                nc.vector.tensor_add(v1[:], v1[:], sq[:])
                std = stp.tile([P, 1], F32, tag="std")
                nc.scalar.activation(std[:], v1[:], AFT.Sqrt,
                                     scale=1.0 / (D - 1))
                nc.vector.tensor_scalar_add(std[:], std[:], EPS)
                s = stp.tile([P, 1], F32, tag="s")
                nc.vector.reciprocal(s[:], std[:])
                nb = stp.tile([P, 1], F32, tag="nb")
                nc.vector.tensor_mul(nb[:], mu[:], s[:])
                nc.vector.tensor_scalar_mul(nb[:], nb[:], -1.0)
                return s, nb

            def transpose4(xn, dstT8, rows, t, on_act):
                """Transpose [128, 512] f32r sbuf tile t into dstT8
                [128, ET*rows] fp8 (x16): 4 e-transposes, 1 scaled copy."""
                if COMBINED_TP:
                    ps = scp.tile([P, 1024], F32R, tag="score",
                                  name="tp")[:, 0:512]
                    for e in range(ET):
                        nc.tensor.matmul(
                            ps[:, e * P:(e + 1) * P],
                            xn[:, e * P:(e + 1) * P], ident[:],
                            start=(e == 0), stop=(e == ET - 1),
                            is_transpose=True, skip_group_check=(e != 0))
                    copy_scaled(
                        _r(dstT8[:], "p (e s) -> p e s", e=ET)[
                            :, :, t * P:(t + 1) * P],
                        _r(ps[:], "p (e c) -> p e c", e=ET), XS, on_act)
                else:
                    for e in range(ET):
                        ps = scp.tile([P, 1024], F32R, tag="score",
                                      name="tp")[:, 0:P]
                        nc.tensor.matmul(
                            ps[:], xn[:, e * P:(e + 1) * P], ident[:],
                            start=True, stop=True, is_transpose=True)
                        copy_scaled(
                            _r(dstT8[:], "p (e s) -> p e s", e=ET)[
                                :, e, t * P:(t + 1) * P],
                            ps[:], XS, on_act)

            def ln_transpose(src_d, rows, dstT8, do_ln=True):
                nt = rows // P
                for t in range(nt):
                    if do_ln:
                        x_t = lnp.tile([P, D], F32, tag="ln_x")
                        nc.sync.dma_start(x_t[:], src_d[t * P:(t + 1) * P, :])
                        s, nb = ln_stats(x_t)
                        xn = lnp.tile([P, D], F32R, tag="ln_xn")
                        nc.vector.tensor_scalar(xn[:], x_t[:], s[:], nb[:],
                                                op0=ALU.mult, op1=ALU.add)
                        transpose4(xn[:], dstT8, rows, t,
                                   on_act=(t % 2 == 0))
                    else:
                        xn = lnp.tile([P, D], F32R, tag="ln_x")
                        nc.sync.dma_start(
                            xn[:], src_d[t * P:(t + 1) * P, :].bitcast(F32R))
                        transpose4(xn[:], dstT8, rows, t, on_act=(t % 2 == 0))

            def ln_transpose_sbuf(xsb, dstT8, tiles=None):
                for t in (tiles if tiles is not None else range(SQ // P)):
                    x_t = xsb[:, t * D:(t + 1) * D]
                    s, nb = ln_stats(x_t)
                    xn = lnp.tile([P, D], F32R, tag="ln_xn")
                    nc.vector.tensor_scalar(xn[:], x_t, s[:], nb[:],
                                            op0=ALU.mult, op1=ALU.add)
                    transpose4(xn[:], dstT8, SQ, t, on_act=(t % 2 == 0))

            def projT(wt8, bt, has_b, srcT8, src_rows, dstT, src_qbs=None):
                """dstT[:, dt, :] = bf16 (W.T @ xn.T)-slice via DoubleRow."""
                if src_qbs is None:
                    src_qbs = list(range(src_rows // 512))
                nqb = len(src_qbs)
                for dt in range(ET):
                    for dqb, qb in enumerate(src_qbs):
                        ps = scp.tile([P, 1024], F32, tag="score",
                                      name="ps")[:, 0:512]
                        for ep in range(EP):
                            nc.tensor.matmul(
                                ps[:],
                                pv8(wt8)[:, 2 * ep:2 * ep + 2,
                                         dt * P:(dt + 1) * P],
                                pv8(srcT8)[:, 2 * ep:2 * ep + 2,
                                           qb * 512:(qb + 1) * 512],
                                start=(ep == 0),
                                stop=(ep == EP - 1 and not has_b),
                                perf_mode=DR)
                            if ep == EP - 1 and has_b:
                                nc.tensor.matmul(
                                    ps[:], bt[:, dt * P:(dt + 1) * P],
                                    ones1[:, 0:512].bitcast(F32R),
                                    start=False, stop=True)
                        copy_scaled(
                            dstT[:, dt * nqb * 512 + dqb * 512:
                                 dt * nqb * 512 + (dqb + 1) * 512],
                            ps[:], DS, on_act=((dt + dqb) % 2 == 0))

            def proj_va(wt8, bt, has_b, srcT8, src_rows, va):
                """V projection, token-major: va [128, nkt*8*66] fp8 = 16*V
                (+ src-mask row scaling), ones column = 16."""
                nkt = src_rows // P
                for kt in range(nkt):
                    ps = scp.tile([P, 1024], F32, tag="score",
                                  name="ps")[:, 0:512]
                    for ep in range(EP):
                        nc.tensor.matmul(
                            ps[:],
                            pv8(srcT8)[:, 2 * ep:2 * ep + 2,
                                       kt * P:(kt + 1) * P],
                            pv8(wt8)[:, 2 * ep:2 * ep + 2, 0:D],
                            start=(ep == 0),
                            stop=(ep == EP - 1 and not has_b),
                            perf_mode=DR)
                        if ep == EP - 1 and has_b:
                            nc.tensor.matmul(
                                ps[:], ones1[:, 0:P], bt[:],
                                start=False, stop=True)
                    dst = _r(va[:], "p (t h c) -> p t h c", t=nkt, h=H)
                    if bias_flags.get("msrc1"):
                        copy_scaled(dst[:, kt, :, 0:DK],
                                    _r(ps[:], "p (h c) -> p h c", h=H),
                                    XS * DS, on_act=(kt % 2 == 0))
                    else:
                        with nc.allow_low_precision("fp8 va"):
                            nc.vector.tensor_scalar(
                                dst[:, kt, :, 0:DK],
                                _r(ps[:], "p (h c) -> p h c", h=H),
                                msrc[:, kt:kt + 1], XS * DS,
                                op0=ALU.mult, op1=ALU.mult)

            def attention(ktT, va, qtT, nkts, masked, exp_dve, wo8, bo,
                          has_bo, resid, x_out, on_pos_done=None):
                """ktT [128, 4*S_k] bf16; va [128, nkt*8*66] fp8 (16*V);
                qtT [128, 4*1024] bf16; masked: fn(pos,kt)-> mask idx|None;
                exp_dve: fn(pos,kt)->bool; x_out [128,8*512] f32 resid+attn."""
                skmax = max(nkts) * P
                atT8 = pp.tile([P, ET * SQ], FP8, tag="attnT")
                taps_live["atT8"] = atT8
                for pos, nkt in enumerate(nkts):
                    units = []
                    kt = 0
                    while kt < nkt:
                        if (USE_PAIRS and masked(pos, kt) is None
                                and kt + 1 < nkt
                                and masked(pos, kt + 1) is None):
                            units.append((kt, kt + 1))
                            kt += 2
                        else:
                            units.append((kt,))
                            kt += 1
                    for hp in range(H // 2):
                        acc = [accp.tile([P, 512], F32, tag=f"acc{i}",
                                         name=f"acc{i}")
                               for i in range(2)]

                        def emit_score(kt):
                            st = scp.tile([P, 1024], F32, tag="score")
                            for i in range(2):
                                nc.tensor.matmul(
                                    st[:, i * 512:(i + 1) * 512],
                                    ktT[i * DK:(i + 1) * DK,
                                        hp * skmax + kt * P:
                                        hp * skmax + (kt + 1) * P],
                                    qtT[i * DK:(i + 1) * DK,
                                        hp * SQ + pos * 512:
                                        hp * SQ + (pos + 1) * 512],
                                    start=True, stop=True)
                            return st

                        def do_exp(pos, kt, st, pair=None):
                            """pair: ([128,2048] int8 tile, j) for fp8 pair
                            halves; None -> bf16 p_t (masked path)."""
                            on_dve = USE_SCH and exp_dve(pos, kt)
                            if pair is not None:
                                t8, j = pair
                                dst = t8[:, j * 1024:(j + 1) * 1024]
                                if on_dve:
                                    with nc.allow_low_precision("sch exp"):
                                        nc.vector.tensor_scalar(
                                            dst, st[:], SCH_A_F8, SCH_B_F8,
                                            op0=ALU.mult, op1=ALU.add)
                                else:
                                    nc.scalar.activation(
                                        dst.bitcast(FP8), st[:], AFT.Exp,
                                        bias=ln16[:], scale=1.0 / 8.0)
                                return None
                            p_t = psb.tile([P, 1024], I16, tag="p")
                            if on_dve:
                                with nc.allow_low_precision("sch exp"):
                                    nc.vector.tensor_scalar(
                                        p_t[:], st[:], SCH_A_BF, SCH_B_BF,
                                        op0=ALU.mult, op1=ALU.add)
                            else:
                                nc.scalar.activation(
                                    p_t[:].bitcast(BF16), st[:], AFT.Exp,
                                    bias=ln16[:], scale=1.0 / 8.0)
                            return p_t

                        flat = [kt for u in units for kt in u]
                        sts = {flat[0]: emit_score(flat[0])}

                        def prefetch(kt):
                            fi = flat.index(kt) + 1
                            if fi < len(flat):
                                sts[flat[fi]] = emit_score(flat[fi])

                        vat = _r(va[:], "p (t h c) -> p t h c",
                                 t=S // P, h=H)
                        for u in units:
                            if len(u) == 2:
                                k0, k1 = u
                                p2 = psb.tile([P, 2048], I8, tag="p2")
                                for j, kt in enumerate(u):
                                    st = sts.pop(kt)
                                    prefetch(kt)
                                    do_exp(pos, kt, st, pair=(p2, j))
                                p8 = p2[:].bitcast(FP8)
                                for i in range(2):
                                    h = 2 * hp + i
                                    nc.tensor.matmul(
                                        acc[i][0:DK + 2, :],
                                        vat[:, k0:k0 + 2, h, 0:66],
                                        _r(p8, "p (j x) -> p j x", j=2)[
                                            :, :, i * 512:(i + 1) * 512],
                                        start=(k0 == 0),
                                        stop=(k1 == nkt - 1),
                                        perf_mode=DR)
                            else:
                                kt = u[0]
                                st = sts.pop(kt)
                                prefetch(kt)
                                p_t = do_exp(pos, kt, st)
                                pb = p_t[:].bitcast(BF16)
                                mi = masked(pos, kt)
                                if mi is not None:
                                    mt = masks_sb[:, mi * 512:(mi + 1) * 512]
                                    for i in range(2):
                                        nc.vector.tensor_mul(
                                            pb[:, i * 512:(i + 1) * 512],
                                            pb[:, i * 512:(i + 1) * 512],
                                            mt)
                                for i in range(2):
                                    h = 2 * hp + i
                                    nc.tensor.matmul(
                                        acc[i][0:DK + 2, :],
                                        vat[:, kt, h, 0:66],
                                        pb[:, i * 512:(i + 1) * 512],
                                        start=(kt == 0), stop=(kt == nkt - 1))
                        # epilogue: atT8 = 16 * acc/denom (fp8)
                        rcl = []
                        for i in range(2):
                            rc = lnp.tile([1, 512], BF16, tag="ln_xn",
                                          name="rc")
                            with nc.allow_low_precision("softmax denom"):
                                nc.vector.reciprocal(
                                    rc[:], acc[i][DK:DK + 1, :])
                            rcl.append(rc)
                        rbl = []
                        for i in range(2):
                            rbs = lnp.tile([DK, 512], BF16, tag="ln_xn",
                                           name="rbs")
                            nc.gpsimd.partition_broadcast(rbs[:], rcl[i][:])
                            rbl.append(rbs)
                        for i in range(2):
                            with nc.allow_low_precision("fp8 attn out"):
                                nc.vector.tensor_tensor(
                                    atT8[i * DK:(i + 1) * DK,
                                         hp * SQ + pos * 512:
                                         hp * SQ + (pos + 1) * 512],
                                    acc[i][0:DK, :], rbl[i][:],
                                    op=ALU.mult)
                    # output projection + residual for this pos block
                    for qt in range(pos * 4, (pos + 1) * 4):
                        res = resid(qt)
                        ps = scp.tile([P, 1024], F32, tag="score",
                                      name="ps")[:, 0:512]
                        for ep in range(EP):
                            nc.tensor.matmul(
                                ps[:],
                                pv8(atT8)[:, 2 * ep:2 * ep + 2,
                                          qt * P:(qt + 1) * P],
                                pv8(wo8)[:, 2 * ep:2 * ep + 2, 0:D],
                                start=(ep == 0),
                                stop=(ep == EP - 1 and not has_bo),
                                perf_mode=DR)
                            if ep == EP - 1 and has_bo:
                                nc.tensor.matmul(
                                    ps[:], ones1[:, 0:P], bo[:],
                                    start=False, stop=True)
                        tmp = lnp.tile([P, D], F32, tag="ln_xn", name="wot")
                        nc.scalar.activation(tmp[:], ps[:], AFT.Copy,
                                             scale=DS)
                        nc.vector.tensor_tensor(
                            x_out[:, qt * D:(qt + 1) * D], tmp[:],
                            res, op=ALU.add)
                    if on_pos_done is not None:
                        on_pos_done(pos)

            # ---------------- stage A: LN0 + transposes ----------------
            kvT8 = pp.tile([P, ET * S], FP8, tag="kvT")
            ln_transpose(xkv_d, S, kvT8, do_ln=True)

            # ---------------- weights (all upfront, Pool DGE queue) -------
            wk8 = load_w(w_sa["wk"], "w_a")
            wq8 = load_w(w_sa["wq"], "w_b")
            wv8 = load_w(w_sa["wv"], "w_c")
            wo8 = load_w(w_sa["wo"], "w_d")
            ck8 = load_w(w_ca["wk"], "w_e")
            cq8 = load_w(w_ca["wq"], "w_f")
            cv8 = load_w(w_ca["wv"], "w_g")
            co8 = load_w(w_ca["wo"], "w_h")
            w18 = [load_w(w1_d[k], f"w1{k}", cols=FF) for k in "ab"]
            w28 = []
            for k in "ab":
                t = pp.tile([P, (FF // P) * D], FP8, tag=f"w2{k}")
                nc.sync.dma_start(
                    _r(t[:], "p (t d) -> p t d", t=FF // P),
                    _r(w2_d[k][:], "(t p) d -> p t d", p=P))
                w28.append(t)
            bk = load_bias(b_sa["k"], "b_a", bias_flags["sa_k"])
            bq = load_bias(b_sa["q"], "b_b", bias_flags["sa_q"])
            bv = load_bias(b_sa["v"], "b_c", bias_flags["sa_v"])
            bo = load_bias(b_sa["o"], "b_d", bias_flags["sa_o"])
            cbk = load_bias(b_ca["k"], "b_e", bias_flags["ca_k"])
            cbq = load_bias(b_ca["q"], "b_f", bias_flags["ca_q"])
            cbv = load_bias(b_ca["v"], "b_g", bias_flags["ca_v"])
            cbo = load_bias(b_ca["o"], "b_h", bias_flags["ca_o"])
            b1_sb = load_bias(b1_d, "b1", bias_flags["ff1"], n=FF)
            b2_sb = load_bias(b2_d, "b2", bias_flags["ff2"])
            nc.sync.dma_start(
                _r(masks_sb[:], "p (t c) -> p t c", t=16),
                _r(masks_d[:], "t p c -> p t c"))

            # ---------------- stage B: SA projections ----------------
            ktT_sa = pp.tile([P, ET * S], BF16, tag="ktT")
            qtT_sa = pp.tile([P, ET * SQ], BF16, tag="qtT")
            va_sa = pp.tile([P, 16 * H * 66], FP8, tag="va")
            nc.vector.memset(
                _r(va_sa[:], "p (t h c) -> p t h c", t=16, h=H)[:, :, :, DK:DK + 1],
                1.0)
            nc.vector.memset(
                _r(va_sa[:], "p (t h c) -> p t h c", t=16, h=H)[:, :, :, DK + 1:],
                0.0)
            if DEBUG_TAPS:
                nc.sync.dma_start(taps["dbg_kvT8"][:], kvT8[:])
            projT(wk8, bk, bias_flags["sa_k"], kvT8, S, ktT_sa)
            projT(wq8, bq, bias_flags["sa_q"], kvT8, S, qtT_sa,
                  src_qbs=list(Q_SRC_QBS))
            proj_va(wv8, bv, bias_flags["sa_v"], kvT8, S, va_sa)

            if DEBUG_TAPS:
                nc.sync.dma_start(taps["dbg_ktT"][:], ktT_sa[:])
                nc.sync.dma_start(taps["dbg_qtT"][:], qtT_sa[:])
                nc.sync.dma_start(taps["dbg_va"][:], va_sa[:])
            # ---------------- stage C/D: SA attention + Wo ----------------
            x1 = pp.tile([P, (SQ // P) * D], F32, tag="x1")

            def sa_masked(pos, kt):
                return kt if (pos == 0 or kt >= 8) else None

            def sa_exp_dve(pos, kt):
                if sa_masked(pos, kt) is None:
                    return kt % 4 == 1
                return (kt % 8) == 2

            def q_src_row(qt):
                pos, j = divmod(qt, 4)
                return Q_SRC_QBS[pos] * 512 + j * P

            def sa_resid(qt):
                rt = lnp.tile([P, D], F32, tag="ln_x", name="sa_resid")
                r0 = q_src_row(qt)
                nc.sync.dma_start(rt[:], xkv_d[r0:r0 + P, :])
                return rt[:]

            # hoisted CA prep: encoder transpose overlaps SA attention
            encT8 = pp.tile([P, ET * S], FP8, tag="kvT")  # reuse kvT slot
            ln_transpose(enc_d, S, encT8, do_ln=False)

            attention(ktT_sa, va_sa, qtT_sa, [NKT0, NKT1], sa_masked,
                      sa_exp_dve, wo8, bo, bias_flags["sa_o"], sa_resid, x1)

            if DEBUG_TAPS:
                nc.sync.dma_start(taps["dbg_x1"][:], x1[:])
                nc.sync.dma_start(taps["dbg_atT8"][:],
                                  taps_live["atT8"][:])
            # ---------------- stage E/F: CA ----------------
            ktT_ca = pp.tile([P, ET * S], BF16, tag="ktT")
            qtT_ca = pp.tile([P, ET * SQ], BF16, tag="qtT")
            va_ca = pp.tile([P, 16 * H * 66], FP8, tag="va")
            nc.vector.memset(
                _r(va_ca[:], "p (t h c) -> p t h c", t=16, h=H)[:, :, :, DK:DK + 1],
                1.0)
            nc.vector.memset(
                _r(va_ca[:], "p (t h c) -> p t h c", t=16, h=H)[:, :, :, DK + 1:],
                0.0)
            projT(ck8, cbk, bias_flags["ca_k"], encT8, S, ktT_ca)
            proj_va(cv8, cbv, bias_flags["ca_v"], encT8, S, va_ca)

            h1T8 = pp.tile([P, ET * SQ], FP8, tag="hT")
            ln_transpose_sbuf(x1, h1T8)
            projT(cq8, cbq, bias_flags["ca_q"], h1T8, SQ, qtT_ca)

            x2 = pp.tile([P, (SQ // P) * D], F32, tag="x2")

            attention(ktT_ca, va_ca, qtT_ca, [16, 16], lambda p, k: None,
                      lambda p, k: (k % 2 == 1) and (k % 16 != 15), co8,
                      cbo,
                      bias_flags["ca_o"],
                      lambda qt: x1[:, qt * D:(qt + 1) * D], x2)

            if DEBUG_TAPS:
                nc.sync.dma_start(taps["dbg_x2"][:], x2[:])
            # ---------------- stage G: LN2 + FFN ----------------
            h2T8 = pp.tile([P, ET * SQ], FP8, tag="hT")
            ln_transpose_sbuf(x2, h2T8)

            ffT8 = pp.tile([P, (FF // P) * SQ], FP8, tag="ffT")

            def ffn2_qt(qt):
                ps = scp.tile([P, 1024], F32, tag="score",
                              name="ps")[:, 0:512]
                nfp = FF // P // 2
                for wi, wt in enumerate(w28):
                    for fp in range(nfp):
                        nc.tensor.matmul(
                            ps[:],
                            _r(ffT8[:], "p (t s) -> p t s", t=FF // P)[
                                :, 2 * fp:2 * fp + 2, qt * P:(qt + 1) * P],
                            _r(wt[:], "p (t d) -> p t d", t=FF // P)[
                                :, 2 * fp:2 * fp + 2, :],
                            start=(wi == 0 and fp == 0),
                            stop=(wi == 1 and fp == nfp - 1
                                  and not bias_flags["ff2"]),
                            perf_mode=DR)
                if bias_flags["ff2"]:
                    nc.tensor.matmul(
                        ps[:], ones1[:, 0:P], b2_sb[:],
                        start=False, stop=True)
                tmp = lnp.tile([P, D], F32, tag="ln_xn", name="ff2t")
                copy_scaled(tmp[:], ps[:], DS, on_act=(qt % 2 == 0))
                o_t = lnp.tile([P, D], F32, tag="ln_xn", name="o_t")
                nc.vector.tensor_tensor(
                    o_t[:], tmp[:], x2[:, qt * D:(qt + 1) * D], op=ALU.add)
                nc.sync.dma_start(out_d[qt * P:(qt + 1) * P, :], o_t[:])

            for qb in range(SQ // 512):
                if qb == 1:
                    for qt in range(4):
                        ffn2_qt(qt)
                for ft in range(FF // P):
                    ps = scp.tile([P, 1024], F32, tag="score",
                                  name="ps")[:, 0:512]
                    for wi, wt in enumerate(w18):
                        for ep in range(EP):
                            last = (wi == 1 and ep == EP - 1)
                            nc.tensor.matmul(
                                ps[:],
                                _r(wt[:], "p (e f) -> p e f", e=ET)[
                                    :, 2 * ep:2 * ep + 2,
                                    ft * P:(ft + 1) * P],
                                pv8(h2T8)[:, 2 * ep:2 * ep + 2,
                                          qb * 512:(qb + 1) * 512],
                                start=(wi == 0 and ep == 0),
                                stop=(last and not bias_flags["ff1"]),
                                perf_mode=DR)
                            if last and bias_flags["ff1"]:
                                nc.tensor.matmul(
                                    ps[:], b1_sb[:, ft * P:(ft + 1) * P],
                                    ones1[:, 0:512].bitcast(F32R),
                                    start=False, stop=True)
                    # ffT8 = 16*relu(z); alternate Act/DVE per ft
                    if ft % 2 == 0:
                        nc.scalar.activation(
                            ffT8[:, ft * SQ + qb * 512:
                                 ft * SQ + (qb + 1) * 512],
                            ps[:], AFT.Relu, scale=XS * DS)
                    else:
                        with nc.allow_low_precision("fp8 relu"):
                            nc.vector.tensor_scalar(
                                ffT8[:, ft * SQ + qb * 512:
                                     ft * SQ + (qb + 1) * 512],
                                ps[:], 0.0, XS * DS,
                                op0=ALU.max, op1=ALU.mult)
            for qt in range(4, SQ // P):
                ffn2_qt(qt)

    nc.finalize()
    return nc


_CACHE = {}
LAST_EXEC_NS = None


def kernel(**inputs):
    x = np.asarray(inputs["x"], np.float32)
    enc = np.asarray(inputs["encoder_output"], np.float32)
    src_mask = np.asarray(inputs["src_mask"]).reshape(S)
    tgt_mask = np.asarray(inputs["tgt_mask"]).reshape(S, S)

    def fold(w, g, b, extra_b):
        w = np.asarray(w, np.float32)
        wf = np.asarray(g, np.float32)[:, None] * w
        bf = np.asarray(b, np.float32) @ w + np.asarray(extra_b, np.float32)
        return wf, bf

    def q8(w):
        return np.asarray(w * WS, np.float32).astype(ml_dtypes.float8_e4m3)

    def q8r(w):
        ws = np.asarray(w * WS, np.float32)
        return (ws - ws.astype(ml_dtypes.float8_e4m3)
                .astype(np.float32)).astype(ml_dtypes.float8_e4m3)

    z = np.zeros(D, np.float32)
    sa_wq, bsa_q = fold(inputs["sa_wq"], inputs["ln0_g"], inputs["ln0_b"], z)
    sa_wk, bsa_k = fold(inputs["sa_wk"], inputs["ln0_g"], inputs["ln0_b"], z)
    sa_wv, bsa_v = fold(inputs["sa_wv"], inputs["ln0_g"], inputs["ln0_b"], z)
    sa_wo = np.asarray(inputs["sa_wo"], np.float32)
    bsa_o = np.asarray(inputs["sa_bo"], np.float32)
    ca_wq, bca_q = fold(inputs["ca_wq"], inputs["ln1_g"], inputs["ln1_b"], z)
    ca_wk = np.asarray(inputs["ca_wk"], np.float32)
    bca_k = np.zeros(D, np.float32)
    ca_wv = np.asarray(inputs["ca_wv"], np.float32)
    bca_v = np.zeros(D, np.float32)
    ca_wo = np.asarray(inputs["ca_wo"], np.float32)
    bca_o = np.asarray(inputs["ca_bo"], np.float32)
    w1, b1 = fold(inputs["ff_w1"], inputs["ln2_g"], inputs["ln2_b"],
                  np.asarray(inputs["ff_b1"], np.float32))
    w2 = np.asarray(inputs["ff_w2"], np.float32)
    b2 = np.asarray(inputs["ff_b2"], np.float32)

    bias_flags = {
        "sa_q": bool(np.any(bsa_q)), "sa_k": bool(np.any(bsa_k)),
        "sa_v": bool(np.any(bsa_v)), "sa_o": bool(np.any(bsa_o)),
        "ca_q": bool(np.any(bca_q)), "ca_k": bool(np.any(bca_k)),
        "ca_v": bool(np.any(bca_v)), "ca_o": bool(np.any(bca_o)),
        "ff1": bool(np.any(b1)), "ff2": bool(np.any(b2)),
        "msrc1": bool(np.all(src_mask == 1)),
    }

    key = tuple(sorted(bias_flags.items()))
    if key not in _CACHE:
        _CACHE[key] = build_program(bias_flags)
    nc = _CACHE[key]

    ident = np.eye(P, dtype=np.float32)
    msrc = src_mask.astype(np.float32).reshape(S // P, P).T.copy()
    BS = WS * XS  # bias pre-scale (descaled by DS in the psum copy)

    shared = {
        "ident": ident, "msrc": msrc,
        "sa_wq": q8(sa_wq), "sa_wk": q8(sa_wk), "sa_wv": q8(sa_wv),
        "sa_wo": q8(sa_wo),
        "ca_wq": q8(ca_wq), "ca_wk": q8(ca_wk), "ca_wv": q8(ca_wv),
        "ca_wo": q8(ca_wo),
        "w1a": q8(w1), "w1b": q8r(w1), "w2a": q8(w2), "w2b": q8r(w2),
        "bsa_q": bsa_q[None] * BS, "bsa_k": bsa_k[None] * BS,
        "bsa_v": bsa_v[None] * BS, "bsa_o": bsa_o[None] * BS,
        "bca_q": bca_q[None] * BS, "bca_k": bca_k[None] * BS,
        "bca_v": bca_v[None] * BS, "bca_o": bca_o[None] * BS,
        "b1": b1[None] * BS, "b2": b2[None] * BS,
    }

    in_maps = []
    for c in range(8):
        b, r = divmod(c, 2)
        perm = PERM_BLOCKS[r]
        rows = np.concatenate(
            [np.arange(gb * 512, (gb + 1) * 512) for gb in perm])
        gb0, gb1 = OWN_BLOCKS[r]
        assert perm[0] == gb0 and perm[2] == gb1
        mk = np.zeros((16, P, 512), np.float32)
        for pos, gb in enumerate((gb0, gb1)):
            qs = slice(gb * 512, (gb + 1) * 512)
            mrow = tgt_mask[qs][:, rows]
            for j in range(8):
                kt = j if pos == 0 else 8 + j
                ks = slice(kt * P, (kt + 1) * P)
                mk[pos * 8 + j] = mrow[:, ks].T
            ext = (NKT0 if pos == 0 else NKT1) * P
            assert not np.any(mrow[:, ext:]), "tgt_mask beyond extent"
        im = dict(shared)
        im["xkv"] = np.ascontiguousarray(x[b][rows])
        im["enc"] = np.ascontiguousarray(enc[b])
        im["masks"] = mk.astype(ml_dtypes.bfloat16)
        in_maps.append(im)

    res = run_bass_kernel_spmd(nc, in_maps, core_ids=list(range(8)))
    global LAST_EXEC_NS
    LAST_EXEC_NS = res.exec_time_ns

    out = np.empty((B, S, D), np.float32)
    for c in range(8):
        b, r = divmod(c, 2)
        gb0, gb1 = OWN_BLOCKS[r]
        o = res.results[c]["out"]
        out[b, gb0 * 512:(gb0 + 1) * 512] = o[0:512]
        out[b, gb1 * 512:(gb1 + 1) * 512] = o[512:1024]
    return out


# revision 75
# speedup vs baseline: 1.0450x; 1.0012x over previous
"""Trainium2 Bass kernel for a pre-LN transformer decoder block.

Shapes (hardcoded): B=4, S_TGT=S_SRC=2048, D=512, H=8, DK=64, FF=2048, fp32.

Sharding: 8 cores; core c handles batch c//2. The two cores of a batch split
the 2048 query rows into two causal-balanced groups of 2x512 rows:
  r0: global q-blocks [0:512) and [1536:2048)
  r1: global q-blocks [512:1024) and [1024:1536)
All cores run one identical SPMD program. Keys (and the x rows feeding K/V)
are PERMUTED per core so that the own q-blocks land at canonical positions:
  pi = [own0 | filler0 | own1 | filler1]   (4 blocks of 512 rows)
With this order both ranks see SA extents of 8 k-tiles (pos0) and 16 (pos1),
diagonal mask tiles align, and Q^T is just columns {block0, block2} of the
transposed/normalized x. Per-core visibility is carried by mask DATA built
on the host. Cross-attention is unmasked full-extent.

Precision/layout strategy:
 - Projections / FFN / Wo run as fp8e4m3 DoubleRow matmuls (2 contraction
   rows per pass over e-tile pairs, 4x PE rate vs f32r). Weights are scaled
   x64 and activations x16 into fp8; every PSUM result is descaled by
   2^-10 in its PSUM->SBUF copy (engine-alternated between DVE and Act).
 - K^T/Q^T are bf16 (scores at full PE rate); P is 16*exp(score/8), stored
   fp8 on unmasked k-tile pairs (DoubleRow PV) and bf16 on masked tiles.
   The ones-column of V (=16) carries the softmax denominator; the x16
   cancels in the division.
 - exp alternates between Act (native Exp) and DVE (Schraudolph bit-trick:
   bits = int(A*score + B) reinterpreted as bf16/e4m3), balancing the
   otherwise Act-bound attention spans.
"""

import numpy as np
import ml_dtypes

import concourse.bass as bass
import concourse.bacc as bacc
import concourse.mybir as mybir
import concourse.tile as tile
from concourse.bass_utils import run_bass_kernel_spmd

F32 = mybir.dt.float32
F32R = mybir.dt.float32r
BF16 = mybir.dt.bfloat16
FP8 = mybir.dt.float8e4
I8 = mybir.dt.int8
I16 = mybir.dt.int16
AFT = mybir.ActivationFunctionType
ALU = mybir.AluOpType
AXL = mybir.AxisListType
DR = mybir.MatmulPerfMode.DoubleRow

B, S, D, H, DK, FF = 4, 2048, 512, 8, 64, 2048
P = 128            # partitions
ET = D // P        # 4 e-tiles of 128 over the model dim
EP = ET // 2       # e-tile pairs for DoubleRow
SQ = 1024          # own query rows per core
NKT0, NKT1 = 8, 16  # uniform k-tile extents for SA pos0 / pos1
EPS = 1e-6

WS = 64.0          # fp8 weight scale
XS = 16.0          # fp8 activation scale
DS = 1.0 / (WS * XS)   # descale after a DoubleRow matmul
LN16 = float(np.log(16.0))
LOG2E = 1.4426950408889634
# Schraudolph exp: bits = trunc(score*A + B); B includes the x16 bias
SCH_A_BF = 128.0 * LOG2E / 8.0
SCH_B_BF = (127.0 + 4.0) * 128.0 - 8.0
SCH_A_F8 = 8.0 * LOG2E / 8.0
SCH_B_F8 = (7.0 + 4.0) * 8.0

OWN_BLOCKS = {0: (0, 3), 1: (1, 2)}
PERM_BLOCKS = {0: (0, 1, 3, 2), 1: (1, 0, 2, 3)}
Q_SRC_QBS = (0, 2)
# combined 4-in-1 transpose PSUM (HW-proven); CoreSim's checker rejects it,
# so debugging scripts can flip this off before build.
COMBINED_TP = True
USE_SCH = True     # DVE Schraudolph exp offload
USE_PAIRS = True   # fp8 DoubleRow PV on unmasked k-tile pairs
DEBUG_TAPS = False  # dump intermediates to DRAM for debugging


def _r(ap, pattern, **kw):
    return ap.rearrange(pattern, **kw)


def build_program(bias_flags):
    """Build the SPMD Bass program. bias_flags: dict of bools saying which
    folded biases are nonzero (uniform across cores)."""
    nc = bacc.Bacc("TRN2", target_bir_lowering=False, debug=False, num_devices=8)

    def din(name, shape, dt=F32):
        return nc.dram_tensor(name, shape, dt, kind="ExternalInput").ap()

    xkv_d = din("xkv", [S, D])
    enc_d = din("enc", [S, D])
    masks_d = din("masks", [16, P, 512], BF16)
    msrc_d = din("msrc", [P, S // P])
    ident_d = din("ident", [P, P])
    # fp8 weights, pre-scaled x64
    w_sa = {k: din(f"sa_{k}", [D, D], FP8) for k in ("wq", "wk", "wv", "wo")}
    w_ca = {k: din(f"ca_{k}", [D, D], FP8) for k in ("wq", "wk", "wv", "wo")}
    w1_d = {k: din(f"w1{k}", [D, FF], FP8) for k in "ab"}
    w2_d = {k: din(f"w2{k}", [FF, D], FP8) for k in "ab"}
    # folded biases (pre-scaled x1024), [1, D] / [1, FF]
    b_sa = {k: din(f"bsa_{k}", [1, D]) for k in ("q", "k", "v", "o")}
    b_ca = {k: din(f"bca_{k}", [1, D]) for k in ("q", "k", "v", "o")}
    b1_d = din("b1", [1, FF])
    b2_d = din("b2", [1, D])
    out_d = nc.dram_tensor("out", [SQ, D], F32, kind="ExternalOutput").ap()
    taps = {}
    taps_live = {}
    if DEBUG_TAPS:
        for nm, shape, dt in [
                ("dbg_kvT8", [P, ET * S], FP8),
                ("dbg_ktT", [P, ET * S], BF16),
                ("dbg_qtT", [P, ET * SQ], BF16),
                ("dbg_va", [P, 16 * H * 66], FP8),
                ("dbg_atT8", [P, ET * SQ], FP8),
                ("dbg_x1", [P, (SQ // P) * D], F32),
                ("dbg_x2", [P, (SQ // P) * D], F32)]:
            taps[nm] = nc.dram_tensor(nm, shape, dt,
                                      kind="ExternalOutput").ap()

    with tile.TileContext(nc) as tc:
        with (
            tc.tile_pool(name="persist", bufs=1) as pp,
            tc.tile_pool(name="ln_sb", bufs=6) as lnp,
            tc.tile_pool(name="p_sb", bufs=5) as psb,
            tc.tile_pool(name="ln_st", bufs=4) as stp,
            tc.tile_pool(name="sc_ps", bufs=3, space="PSUM") as scp,
            tc.tile_pool(name="acc_ps", bufs=1, space="PSUM") as accp,
        ):
            ident = pp.tile([P, P], F32R, tag="ident")
            nc.sync.dma_start(ident[:], ident_d[:].bitcast(F32R))
            ones1f = pp.tile([1, P], F32, tag="ones1f")
            nc.vector.memset(ones1f[:], 1.0)
            ones1 = pp.tile([1, P], F32R, tag="ones1")
            nc.vector.tensor_copy(ones1[:], ones1f[:])
            c16_bf = pp.tile([1, P], BF16, tag="c16_bf")
            nc.vector.memset(c16_bf[:], 16.0)
            ln16 = pp.tile([P, 1], F32, tag="ln16")
            nc.vector.memset(ln16[:], LN16)
            msrc = pp.tile([P, S // P], F32, tag="msrc")
            nc.sync.dma_start(msrc[:], msrc_d[:])
            masks_sb = pp.tile([P, 16 * 512], BF16, tag="masks")

            def load_w(dram, name, cols=D):
                # [cin, cols] -> sbuf [128, ET, cols] fp8
                t = pp.tile([P, ET * cols], FP8, tag=name)
                nc.sync.dma_start(
                    _r(t[:], "p (e d) -> p e d", e=ET),
                    _r(dram[:], "(e p) d -> p e d", p=P))
                return t

            def load_bias(dram, name, flag, n=D):
                if not flag:
                    return None
                t = pp.tile([1, n], F32R, tag=name)
                nc.sync.dma_start(t[:], dram[:].bitcast(F32R))
                return t

            def pv8(t8):
                return _r(t8[:], "p (e s) -> p e s", e=ET)

            def copy_scaled(dst, src, c, on_act):
                """PSUM->SBUF copy with scale, engine-balanced."""
                with nc.allow_low_precision("fp8/bf16 staging"):
                    if on_act:
                        nc.scalar.activation(dst, src, AFT.Copy, scale=c)
                    else:
                        nc.vector.tensor_scalar_mul(dst, src, c)

            def ln_stats(x_t):
                """x_t: [128, 512] f32 sbuf -> (scale, bias) per-row [128,1]."""
                sx = stp.tile([P, 1], F32, tag="sx")
                dump = lnp.tile([P, D], F32, tag="ln_xn")
                sq = stp.tile([P, 1], F32, tag="sq")
                nc.scalar.activation(dump[:], x_t[:], AFT.Square,
                                     accum_out=sq[:])
                nc.vector.reduce_sum(sx[:], x_t[:], axis=AXL.X)
                mu = stp.tile([P, 1], F32, tag="mu")
                nc.vector.tensor_scalar_mul(mu[:], sx[:], 1.0 / D)
                m2 = stp.tile([P, 1], F32, tag="m2")
                nc.vector.tensor_mul(m2[:], mu[:], mu[:])
                v1 = stp.tile([P, 1], F32, tag="v1")
                nc.vector.tensor_scalar(v1[:], m2[:], -float(D), None,
                                        op0=ALU.mult)
                nc.vector.tensor_add(v1[:], v1[:], sq[:])
                std = stp.tile([P, 1], F32, tag="std")
                nc.scalar.activation(std[:], v1[:], AFT.Sqrt,
                                     scale=1.0 / (D - 1))
                nc.vector.tensor_scalar_add(std[:], std[:], EPS)
                s = stp.tile([P, 1], F32, tag="s")
                nc.vector.reciprocal(s[:], std[:])
                nb = stp.tile([P, 1], F32, tag="nb")
                nc.vector.tensor_mul(nb[:], mu[:], s[:])
                nc.vector.tensor_scalar_mul(nb[:], nb[:], -1.0)
                return s, nb

            def transpose4(xn, dstT8, rows, t, on_act):
                """Transpose [128, 512] f32r sbuf tile t into dstT8
                [128, ET*rows] fp8 (x16): 4 e-transposes, 1 scaled copy."""
                if COMBINED_TP:
                    ps = scp.tile([P, 1024], F32R, tag="score",
                                  name="tp")[:, 0:512]
                    for e in range(ET):
                        nc.tensor.matmul(
                            ps[:, e * P:(e + 1) * P],
                            xn[:, e * P:(e + 1) * P], ident[:],
                            start=(e == 0), stop=(e == ET - 1),
                            is_transpose=True, skip_group_check=(e != 0))
                    copy_scaled(
                        _r(dstT8[:], "p (e s) -> p e s", e=ET)[
                            :, :, t * P:(t + 1) * P],
                        _r(ps[:], "p (e c) -> p e c", e=ET), XS, on_act)
                else:
                    for e in range(ET):
                        ps = scp.tile([P, 1024], F32R, tag="score",
                                      name="tp")[:, 0:P]
                        nc.tensor.matmul(
                            ps[:], xn[:, e * P:(e + 1) * P], ident[:],
                            start=True, stop=True, is_transpose=True)
                        copy_scaled(
                            _r(dstT8[:], "p (e s) -> p e s", e=ET)[
                                :, e, t * P:(t + 1) * P],
                            ps[:], XS, on_act)

            def ln_transpose(src_d, rows, dstT8, do_ln=True):
                nt = rows // P
                for t in range(nt):
                    if do_ln:
                        x_t = lnp.tile([P, D], F32, tag="ln_x")
                        nc.sync.dma_start(x_t[:], src_d[t * P:(t + 1) * P, :])
                        s, nb = ln_stats(x_t)
                        xn = lnp.tile([P, D], F32R, tag="ln_xn")
                        nc.vector.tensor_scalar(xn[:], x_t[:], s[:], nb[:],
                                                op0=ALU.mult, op1=ALU.add)
                        transpose4(xn[:], dstT8, rows, t,
                                   on_act=(t % 2 == 0))
                    else:
                        xn = lnp.tile([P, D], F32R, tag="ln_x")
                        nc.sync.dma_start(
                            xn[:], src_d[t * P:(t + 1) * P, :].bitcast(F32R))
                        transpose4(xn[:], dstT8, rows, t, on_act=(t % 2 == 0))

            def ln_transpose_sbuf(xsb, dstT8, tiles=None):
                for t in (tiles if tiles is not None else range(SQ // P)):
                    x_t = xsb[:, t * D:(t + 1) * D]
                    s, nb = ln_stats(x_t)
                    xn = lnp.tile([P, D], F32R, tag="ln_xn")
                    nc.vector.tensor_scalar(xn[:], x_t, s[:], nb[:],
                                            op0=ALU.mult, op1=ALU.add)
                    transpose4(xn[:], dstT8, SQ, t, on_act=(t % 2 == 0))

            def projT(wt8, bt, has_b, srcT8, src_rows, dstT, src_qbs=None):
                """dstT[:, dt, :] = bf16 (W.T @ xn.T)-slice via DoubleRow."""
                if src_qbs is None:
                    src_qbs = list(range(src_rows // 512))
                nqb = len(src_qbs)
                for dt in range(ET):
                    for dqb, qb in enumerate(src_qbs):
                        ps = scp.tile([P, 1024], F32, tag="score",
                                      name="ps")[:, 0:512]
                        for ep in range(EP):
                            nc.tensor.matmul(
                                ps[:],
                                pv8(wt8)[:, 2 * ep:2 * ep + 2,
                                         dt * P:(dt + 1) * P],
                                pv8(srcT8)[:, 2 * ep:2 * ep + 2,
                                           qb * 512:(qb + 1) * 512],
                                start=(ep == 0),
                                stop=(ep == EP - 1 and not has_b),
                                perf_mode=DR)
                            if ep == EP - 1 and has_b:
                                nc.tensor.matmul(
                                    ps[:], bt[:, dt * P:(dt + 1) * P],
                                    ones1[:, 0:512].bitcast(F32R),
                                    start=False, stop=True)
                        copy_scaled(
                            dstT[:, dt * nqb * 512 + dqb * 512:
                                 dt * nqb * 512 + (dqb + 1) * 512],
                            ps[:], DS, on_act=((dt + dqb) % 2 == 0))

            def proj_va(wt8, bt, has_b, srcT8, src_rows, va):
                """V projection, token-major: va [128, nkt*8*66] fp8 = 16*V
                (+ src-mask row scaling), ones column = 16."""
                nkt = src_rows // P
                for kt in range(nkt):
                    ps = scp.tile([P, 1024], F32, tag="score",
                                  name="ps")[:, 0:512]
                    for ep in range(EP):
                        nc.tensor.matmul(
                            ps[:],
                            pv8(srcT8)[:, 2 * ep:2 * ep + 2,
                                       kt * P:(kt + 1) * P],
                            pv8(wt8)[:, 2 * ep:2 * ep + 2, 0:D],
                            start=(ep == 0),
                            stop=(ep == EP - 1 and not has_b),
                            perf_mode=DR)
                        if ep == EP - 1 and has_b:
                            nc.tensor.matmul(
                                ps[:], ones1[:, 0:P], bt[:],
                                start=False, stop=True)
                    dst = _r(va[:], "p (t h c) -> p t h c", t=nkt, h=H)
                    if bias_flags.get("msrc1"):
                        copy_scaled(dst[:, kt, :, 0:DK],
                                    _r(ps[:], "p (h c) -> p h c", h=H),
                                    XS * DS, on_act=(kt % 2 == 0))
                    else:
                        with nc.allow_low_precision("fp8 va"):
                            nc.vector.tensor_scalar(
                                dst[:, kt, :, 0:DK],
                                _r(ps[:], "p (h c) -> p h c", h=H),
                                msrc[:, kt:kt + 1], XS * DS,
                                op0=ALU.mult, op1=ALU.mult)

            def attention(ktT, va, qtT, nkts, masked, exp_dve, wo8, bo,
                          has_bo, resid, x_out, on_pos_done=None):
                """ktT [128, 4*S_k] bf16; va [128, nkt*8*66] fp8 (16*V);
                qtT [128, 4*1024] bf16; masked: fn(pos,kt)-> mask idx|None;
                exp_dve: fn(pos,kt)->bool; x_out [128,8*512] f32 resid+attn."""
                skmax = max(nkts) * P
                atT8 = pp.tile([P, ET * SQ], FP8, tag="attnT")
                taps_live["atT8"] = atT8
                for pos, nkt in enumerate(nkts):
                    units = []
                    kt = 0
                    while kt < nkt:
                        if (USE_PAIRS and masked(pos, kt) is None
                                and kt + 1 < nkt
                                and masked(pos, kt + 1) is None):
                            units.append((kt, kt + 1))
                            kt += 2
                        else:
                            units.append((kt,))
                            kt += 1
                    for hp in range(H // 2):
                        acc = [accp.tile([P, 512], F32, tag=f"acc{i}",
                                         name=f"acc{i}")
                               for i in range(2)]

                        def emit_score(kt):
                            st = scp.tile([P, 1024], F32, tag="score")
                            for i in range(2):
                                nc.tensor.matmul(
                                    st[:, i * 512:(i + 1) * 512],
                                    ktT[i * DK:(i + 1) * DK,
                                        hp * skmax + kt * P:
                                        hp * skmax + (kt + 1) * P],
                                    qtT[i * DK:(i + 1) * DK,
                                        hp * SQ + pos * 512:
                                        hp * SQ + (pos + 1) * 512],
                                    start=True, stop=True)
                            return st

                        def do_exp(pos, kt, st, pair=None):
                            """pair: ([128,2048] int8 tile, j) for fp8 pair
                            halves; None -> bf16 p_t (masked path)."""
                            on_dve = USE_SCH and exp_dve(pos, kt)
                            if pair is not None:
                                t8, j = pair
                                dst = t8[:, j * 1024:(j + 1) * 1024]
                                if on_dve:
                                    with nc.allow_low_precision("sch exp"):
                                        nc.vector.tensor_scalar(
                                            dst, st[:], SCH_A_F8, SCH_B_F8,
                                            op0=ALU.mult, op1=ALU.add)
                                else:
                                    nc.scalar.activation(
                                        dst.bitcast(FP8), st[:], AFT.Exp,
                                        bias=ln16[:], scale=1.0 / 8.0)
                                return None
                            p_t = psb.tile([P, 1024], I16, tag="p")
                            if on_dve:
                                with nc.allow_low_precision("sch exp"):
                                    nc.vector.tensor_scalar(
                                        p_t[:], st[:], SCH_A_BF, SCH_B_BF,
                                        op0=ALU.mult, op1=ALU.add)
                            else:
                                nc.scalar.activation(
                                    p_t[:].bitcast(BF16), st[:], AFT.Exp,
                                    bias=ln16[:], scale=1.0 / 8.0)
                            return p_t

                        flat = [kt for u in units for kt in u]
                        sts = {flat[0]: emit_score(flat[0])}

                        def prefetch(kt):
                            fi = flat.index(kt) + 1
                            if fi < len(flat):
                                sts[flat[fi]] = emit_score(flat[fi])

                        vat = _r(va[:], "p (t h c) -> p t h c",
                                 t=S // P, h=H)
                        for u in units:
                            if len(u) == 2:
                                k0, k1 = u
                                p2 = psb.tile([P, 2048], I8, tag="p2")
                                for j, kt in enumerate(u):
                                    st = sts.pop(kt)
                                    prefetch(kt)
                                    do_exp(pos, kt, st, pair=(p2, j))
                                p8 = p2[:].bitcast(FP8)
                                for i in range(2):
                                    h = 2 * hp + i
                                    nc.tensor.matmul(
                                        acc[i][0:DK + 2, :],
                                        vat[:, k0:k0 + 2, h, 0:66],
                                        _r(p8, "p (j x) -> p j x", j=2)[
                                            :, :, i * 512:(i + 1) * 512],
                                        start=(k0 == 0),
                                        stop=(k1 == nkt - 1),
                                        perf_mode=DR)
                            else:
                                kt = u[0]
                                st = sts.pop(kt)
                                prefetch(kt)
                                p_t = do_exp(pos, kt, st)
                                pb = p_t[:].bitcast(BF16)
                                mi = masked(pos, kt)
                                if mi is not None:
                                    mt = masks_sb[:, mi * 512:(mi + 1) * 512]
                                    for i in range(2):
                                        nc.vector.tensor_mul(
                                            pb[:, i * 512:(i + 1) * 512],
                                            pb[:, i * 512:(i + 1) * 512],
                                            mt)
                                for i in range(2):
                                    h = 2 * hp + i
                                    nc.tensor.matmul(
                                        acc[i][0:DK + 2, :],
                                        vat[:, kt, h, 0:66],
                                        pb[:, i * 512:(i + 1) * 512],
                                        start=(kt == 0), stop=(kt == nkt - 1))
                        # epilogue: atT8 = 16 * acc/denom (fp8)
                        rcl = []
                        for i in range(2):
                            rc = lnp.tile([1, 512], BF16, tag="ln_xn",
                                          name="rc")
                            with nc.allow_low_precision("softmax denom"):
                                nc.vector.reciprocal(
                                    rc[:], acc[i][DK:DK + 1, :])
                            rcl.append(rc)
                        rbl = []
                        for i in range(2):
                            rbs = lnp.tile([DK, 512], BF16, tag="ln_xn",
                                           name="rbs")
                            nc.gpsimd.partition_broadcast(rbs[:], rcl[i][:])
                            rbl.append(rbs)
                        for i in range(2):
                            with nc.allow_low_precision("fp8 attn out"):
                                nc.vector.tensor_tensor(
                                    atT8[i * DK:(i + 1) * DK,
                                         hp * SQ + pos * 512:
                                         hp * SQ + (pos + 1) * 512],
                                    acc[i][0:DK, :], rbl[i][:],
                                    op=ALU.mult)
                    # output projection + residual for this pos block
                    for qt in range(pos * 4, (pos + 1) * 4):
                        res = resid(qt)
                        ps = scp.tile([P, 1024], F32, tag="score",
                                      name="ps")[:, 0:512]
                        for ep in range(EP):
                            nc.tensor.matmul(
                                ps[:],
                                pv8(atT8)[:, 2 * ep:2 * ep + 2,
                                          qt * P:(qt + 1) * P],
                                pv8(wo8)[:, 2 * ep:2 * ep + 2, 0:D],
                                start=(ep == 0),
                                stop=(ep == EP - 1 and not has_bo),
                                perf_mode=DR)
                            if ep == EP - 1 and has_bo:
                                nc.tensor.matmul(
                                    ps[:], ones1[:, 0:P], bo[:],
                                    start=False, stop=True)
                        tmp = lnp.tile([P, D], F32, tag="ln_xn", name="wot")
                        nc.scalar.activation(tmp[:], ps[:], AFT.Copy,
                                             scale=DS)
                        nc.vector.tensor_tensor(
                            x_out[:, qt * D:(qt + 1) * D], tmp[:],
                            res, op=ALU.add)
                    if on_pos_done is not None:
                        on_pos_done(pos)

            # ---------------- stage A: LN0 + transposes ----------------
            kvT8 = pp.tile([P, ET * S], FP8, tag="kvT")
            ln_transpose(xkv_d, S, kvT8, do_ln=True)

            # ---------------- weights (all upfront, Pool DGE queue) -------
            wk8 = load_w(w_sa["wk"], "w_a")
            wq8 = load_w(w_sa["wq"], "w_b")
            wv8 = load_w(w_sa["wv"], "w_c")
            wo8 = load_w(w_sa["wo"], "w_d")
            ck8 = load_w(w_ca["wk"], "w_e")
            cq8 = load_w(w_ca["wq"], "w_f")
            cv8 = load_w(w_ca["wv"], "w_g")
            co8 = load_w(w_ca["wo"], "w_h")
            w18 = [load_w(w1_d[k], f"w1{k}", cols=FF) for k in "ab"]
            w28 = []
            for k in "ab":
                t = pp.tile([P, (FF // P) * D], FP8, tag=f"w2{k}")
                nc.sync.dma_start(
                    _r(t[:], "p (t d) -> p t d", t=FF // P),
                    _r(w2_d[k][:], "(t p) d -> p t d", p=P))
                w28.append(t)
            bk = load_bias(b_sa["k"], "b_a", bias_flags["sa_k"])
            bq = load_bias(b_sa["q"], "b_b", bias_flags["sa_q"])
            bv = load_bias(b_sa["v"], "b_c", bias_flags["sa_v"])
            bo = load_bias(b_sa["o"], "b_d", bias_flags["sa_o"])
            cbk = load_bias(b_ca["k"], "b_e", bias_flags["ca_k"])
            cbq = load_bias(b_ca["q"], "b_f", bias_flags["ca_q"])
            cbv = load_bias(b_ca["v"], "b_g", bias_flags["ca_v"])
            cbo = load_bias(b_ca["o"], "b_h", bias_flags["ca_o"])
            b1_sb = load_bias(b1_d, "b1", bias_flags["ff1"], n=FF)
            b2_sb = load_bias(b2_d, "b2", bias_flags["ff2"])
            nc.sync.dma_start(
                _r(masks_sb[:], "p (t c) -> p t c", t=16),
                _r(masks_d[:], "t p c -> p t c"))

            # ---------------- stage B: SA projections ----------------
            ktT_sa = pp.tile([P, ET * S], BF16, tag="ktT")
            qtT_sa = pp.tile([P, ET * SQ], BF16, tag="qtT")
            va_sa = pp.tile([P, 16 * H * 66], FP8, tag="va")
            nc.vector.memset(
                _r(va_sa[:], "p (t h c) -> p t h c", t=16, h=H)[:, :, :, DK:DK + 1],
                1.0)
            nc.vector.memset(
                _r(va_sa[:], "p (t h c) -> p t h c", t=16, h=H)[:, :, :, DK + 1:],
                0.0)
            if DEBUG_TAPS:
                nc.sync.dma_start(taps["dbg_kvT8"][:], kvT8[:])
            projT(wk8, bk, bias_flags["sa_k"], kvT8, S, ktT_sa)
            projT(wq8, bq, bias_flags["sa_q"], kvT8, S, qtT_sa,
                  src_qbs=list(Q_SRC_QBS))
            proj_va(wv8, bv, bias_flags["sa_v"], kvT8, S, va_sa)

            if DEBUG_TAPS:
                nc.sync.dma_start(taps["dbg_ktT"][:], ktT_sa[:])
                nc.sync.dma_start(taps["dbg_qtT"][:], qtT_sa[:])
                nc.sync.dma_start(taps["dbg_va"][:], va_sa[:])
            # ---------------- stage C/D: SA attention + Wo ----------------
            x1 = pp.tile([P, (SQ // P) * D], F32, tag="x1")

            def sa_masked(pos, kt):
                return kt if (pos == 0 or kt >= 8) else None

            def sa_exp_dve(pos, kt):
                if sa_masked(pos, kt) is None:
                    return kt % 4 == 1
                return (kt % 8) == 2

            def q_src_row(qt):
                pos, j = divmod(qt, 4)
                return Q_SRC_QBS[pos] * 512 + j * P

            def sa_resid(qt):
                rt = lnp.tile([P, D], F32, tag="ln_x", name="sa_resid")
                r0 = q_src_row(qt)
                nc.sync.dma_start(rt[:], xkv_d[r0:r0 + P, :])
                return rt[:]

            # hoisted CA prep: encoder transpose overlaps SA attention
            encT8 = pp.tile([P, ET * S], FP8, tag="kvT")  # reuse kvT slot
            ln_transpose(enc_d, S, encT8, do_ln=False)

            attention(ktT_sa, va_sa, qtT_sa, [NKT0, NKT1], sa_masked,
                      sa_exp_dve, wo8, bo, bias_flags["sa_o"], sa_resid, x1)

            if DEBUG_TAPS:
                nc.sync.dma_start(taps["dbg_x1"][:], x1[:])
                nc.sync.dma_start(taps["dbg_atT8"][:],
                                  taps_live["atT8"][:])
            # ---------------- stage E/F: CA ----------------
            ktT_ca = pp.tile([P, ET * S], BF16, tag="ktT")
            qtT_ca = pp.tile([P, ET * SQ], BF16, tag="qtT")
            va_ca = pp.tile([P, 16 * H * 66], FP8, tag="va")
            nc.vector.memset(
                _r(va_ca[:], "p (t h c) -> p t h c", t=16, h=H)[:, :, :, DK:DK + 1],
                1.0)
            nc.vector.memset(
                _r(va_ca[:], "p (t h c) -> p t h c", t=16, h=H)[:, :, :, DK + 1:],
                0.0)
            projT(ck8, cbk, bias_flags["ca_k"], encT8, S, ktT_ca)
            proj_va(cv8, cbv, bias_flags["ca_v"], encT8, S, va_ca)

            h1T8 = pp.tile([P, ET * SQ], FP8, tag="hT")
            ln_transpose_sbuf(x1, h1T8)
            projT(cq8, cbq, bias_flags["ca_q"], h1T8, SQ, qtT_ca)

            x2 = pp.tile([P, (SQ // P) * D], F32, tag="x2")

            attention(ktT_ca, va_ca, qtT_ca, [16, 16], lambda p, k: None,
                      lambda p, k: (k % 2 == 1) and (k % 16 != 15), co8,
                      cbo,
                      bias_flags["ca_o"],
                      lambda qt: x1[:, qt * D:(qt + 1) * D], x2)

            if DEBUG_TAPS:
                nc.sync.dma_start(taps["dbg_x2"][:], x2[:])
            # ---------------- stage G: LN2 + FFN ----------------
            h2T8 = pp.tile([P, ET * SQ], FP8, tag="hT")
            ln_transpose_sbuf(x2, h2T8)

            ffT8 = pp.tile([P, (FF // P) * SQ], FP8, tag="ffT")

            def ffn2_qt(qt):
                ps = scp.tile([P, 1024], F32, tag="score",
                              name="ps")[:, 0:512]
                nfp = FF // P // 2
                for wi, wt in enumerate(w28):
                    for fp in range(nfp):
                        nc.tensor.matmul(
                            ps[:],
                            _r(ffT8[:], "p (t s) -> p t s", t=FF // P)[
                                :, 2 * fp:2 * fp + 2, qt * P:(qt + 1) * P],
                            _r(wt[:], "p (t d) -> p t d", t=FF // P)[
                                :, 2 * fp:2 * fp + 2, :],
                            start=(wi == 0 and fp == 0),
                            stop=(wi == 1 and fp == nfp - 1
                                  and not bias_flags["ff2"]),
                            perf_mode=DR)
                if bias_flags["ff2"]:
                    nc.tensor.matmul(
                        ps[:], ones1[:, 0:P], b2_sb[:],
                        start=False, stop=True)
                tmp = lnp.tile([P, D], F32, tag="ln_xn", name="ff2t")
                copy_scaled(tmp[:], ps[:], DS, on_act=(qt % 2 == 0))
                o_t = lnp.tile([P, D], F32, tag="ln_xn", name="o_t")
                nc.vector.tensor_tensor(
                    o_t[:], tmp[:], x2[:, qt * D:(qt + 1) * D], op=ALU.add)
                nc.sync.dma_start(out_d[qt * P:(qt + 1) * P, :], o_t[:])

            for qb in range(SQ // 512):
                if qb == 1:
                    for qt in range(4):
                        ffn2_qt(qt)
                for ft in range(FF // P):
                    ps = scp.tile([P, 1024], F32, tag="score",
                                  name="ps")[:, 0:512]
                    for wi, wt in enumerate(w18):
                        for ep in range(EP):
                            last = (wi == 1 and ep == EP - 1)
                            nc.tensor.matmul(
                                ps[:],
                                _r(wt[:], "p (e f) -> p e f", e=ET)[
                                    :, 2 * ep:2 * ep + 2,
                                    ft * P:(ft + 1) * P],
                                pv8(h2T8)[:, 2 * ep:2 * ep + 2,
                                          qb * 512:(qb + 1) * 512],
                                start=(wi == 0 and ep == 0),
                                stop=(last and not bias_flags["ff1"]),
                                perf_mode=DR)
                            if last and bias_flags["ff1"]:
                                nc.tensor.matmul(
                                    ps[:], b1_sb[:, ft * P:(ft + 1) * P],
                                    ones1[:, 0:512].bitcast(F32R),
                                    start=False, stop=True)
                    # ffT8 = 16*relu(z); alternate Act/DVE per ft
                    if ft % 2 == 0:
                        nc.scalar.activation(
                            ffT8[:, ft * SQ + qb * 512:
                                 ft * SQ + (qb + 1) * 512],
                            ps[:], AFT.Relu, scale=XS * DS)
                    else:
                        with nc.allow_low_precision("fp8 relu"):
                            nc.vector.tensor_scalar(
                                ffT8[:, ft * SQ + qb * 512:
                                     ft * SQ + (qb + 1) * 512],
                                ps[:], 0.0, XS * DS,
                                op0=ALU.max, op1=ALU.mult)
            for qt in range(4, SQ // P):
                ffn2_qt(qt)

    nc.finalize()
    return nc


_CACHE = {}
LAST_EXEC_NS = None


def kernel(**inputs):
    x = np.asarray(inputs["x"], np.float32)
    enc = np.asarray(inputs["encoder_output"], np.float32)
    src_mask = np.asarray(inputs["src_mask"]).reshape(S)
    tgt_mask = np.asarray(inputs["tgt_mask"]).reshape(S, S)

    def fold(w, g, b, extra_b):
        w = np.asarray(w, np.float32)
        wf = np.asarray(g, np.float32)[:, None] * w
        bf = np.asarray(b, np.float32) @ w + np.asarray(extra_b, np.float32)
        return wf, bf

    def q8(w):
        return np.asarray(w * WS, np.float32).astype(ml_dtypes.float8_e4m3)

    def q8r(w):
        ws = np.asarray(w * WS, np.float32)
        return (ws - ws.astype(ml_dtypes.float8_e4m3)
                .astype(np.float32)).astype(ml_dtypes.float8_e4m3)

    z = np.zeros(D, np.float32)
    sa_wq, bsa_q = fold(inputs["sa_wq"], inputs["ln0_g"], inputs["ln0_b"], z)
    sa_wk, bsa_k = fold(inputs["sa_wk"], inputs["ln0_g"], inputs["ln0_b"], z)
    sa_wv, bsa_v = fold(inputs["sa_wv"], inputs["ln0_g"], inputs["ln0_b"], z)
    sa_wo = np.asarray(inputs["sa_wo"], np.float32)
    bsa_o = np.asarray(inputs["sa_bo"], np.float32)
    ca_wq, bca_q = fold(inputs["ca_wq"], inputs["ln1_g"], inputs["ln1_b"], z)
    ca_wk = np.asarray(inputs["ca_wk"], np.float32)
    bca_k = np.zeros(D, np.float32)
    ca_wv = np.asarray(inputs["ca_wv"], np.float32)
    bca_v = np.zeros(D, np.float32)
    ca_wo = np.asarray(inputs["ca_wo"], np.float32)
    bca_o = np.asarray(inputs["ca_bo"], np.float32)
    w1, b1 = fold(inputs["ff_w1"], inputs["ln2_g"], inputs["ln2_b"],
                  np.asarray(inputs["ff_b1"], np.float32))
    w2 = np.asarray(inputs["ff_w2"], np.float32)
    b2 = np.asarray(inputs["ff_b2"], np.float32)

    bias_flags = {
        "sa_q": bool(np.any(bsa_q)), "sa_k": bool(np.any(bsa_k)),
        "sa_v": bool(np.any(bsa_v)), "sa_o": bool(np.any(bsa_o)),
        "ca_q": bool(np.any(bca_q)), "ca_k": bool(np.any(bca_k)),
        "ca_v": bool(np.any(bca_v)), "ca_o": bool(np.any(bca_o)),
        "ff1": bool(np.any(b1)), "ff2": bool(np.any(b2)),
        "msrc1": bool(np.all(src_mask == 1)),
    }

    key = tuple(sorted(bias_flags.items()))
    if key not in _CACHE:
        _CACHE[key] = build_program(bias_flags)
    nc = _CACHE[key]

    ident = np.eye(P, dtype=np.float32)
    msrc = src_mask.astype(np.float32).reshape(S // P, P).T.copy()
    BS = WS * XS  # bias pre-scale (descaled by DS in the psum copy)

    shared = {
        "ident": ident, "msrc": msrc,
        "sa_wq": q8(sa_wq), "sa_wk": q8(sa_wk), "sa_wv": q8(sa_wv),
        "sa_wo": q8(sa_wo),
        "ca_wq": q8(ca_wq), "ca_wk": q8(ca_wk), "ca_wv": q8(ca_wv),
        "ca_wo": q8(ca_wo),
        "w1a": q8(w1), "w1b": q8r(w1), "w2a": q8(w2), "w2b": q8r(w2),
        "bsa_q": bsa_q[None] * BS, "bsa_k": bsa_k[None] * BS,
        "bsa_v": bsa_v[None] * BS, "bsa_o": bsa_o[None] * BS,
        "bca_q": bca_q[None] * BS, "bca_k": bca_k[None] * BS,
        "bca_v": bca_v[None] * BS, "bca_o": bca_o[None] * BS,
        "b1": b1[None] * BS, "b2": b2[None] * BS,
    }

    in_maps = []
    for c in range(8):
        b, r = divmod(c, 2)
        perm = PERM_BLOCKS[r]
        rows = np.concatenate(
            [np.arange(gb * 512, (gb + 1) * 512) for gb in perm])
        gb0, gb1 = OWN_BLOCKS[r]
        assert perm[0] == gb0 and perm[2] == gb1
        mk = np.zeros((16, P, 512), np.float32)
        for pos, gb in enumerate((gb0, gb1)):
            qs = slice(gb * 512, (gb + 1) * 512)
            mrow = tgt_mask[qs][:, rows]
            for j in range(8):
                kt = j if pos == 0 else 8 + j
                ks = slice(kt * P, (kt + 1) * P)
                mk[pos * 8 + j] = mrow[:, ks].T
            ext = (NKT0 if pos == 0 else NKT1) * P
            assert not np.any(mrow[:, ext:]), "tgt_mask beyond extent"
        im = dict(shared)
        im["xkv"] = np.ascontiguousarray(x[b][rows])
        im["enc"] = np.ascontiguousarray(enc[b])
        im["masks"] = mk.astype(ml_dtypes.bfloat16)
        in_maps.append(im)

    res = run_bass_kernel_spmd(nc, in_maps, core_ids=list(range(8)))
    global LAST_EXEC_NS
    LAST_EXEC_NS = res.exec_time_ns

    out = np.empty((B, S, D), np.float32)
    for c in range(8):
        b, r = divmod(c, 2)
        gb0, gb1 = OWN_BLOCKS[r]
        o = res.results[c]["out"]
        out[b, gb0 * 512:(gb0 + 1) * 512] = o[0:512]
        out[b, gb1 * 512:(gb1 + 1) * 512] = o[512:1024]
    return out


# revision 81
# speedup vs baseline: 1.0498x; 1.0045x over previous
"""Trainium2 Bass kernel for a pre-LN transformer decoder block.

Shapes (hardcoded): B=4, S_TGT=S_SRC=2048, D=512, H=8, DK=64, FF=2048, fp32.

Sharding: 8 cores; core c handles batch c//2. The two cores of a batch split
the 2048 query rows into two causal-balanced groups of 2x512 rows:
  r0: global q-blocks [0:512) and [1536:2048)
  r1: global q-blocks [512:1024) and [1024:1536)
All cores run one identical SPMD program. Keys (and the x rows feeding K/V)
are PERMUTED per core so that the own q-blocks land at canonical positions:
  pi = [own0 | filler0 | own1 | filler1]   (4 blocks of 512 rows)
With this order both ranks see SA extents of 8 k-tiles (pos0) and 16 (pos1),
diagonal mask tiles align, and Q^T is just columns {block0, block2} of the
transposed/normalized x. Per-core visibility is carried by mask DATA built
on the host. Cross-attention is unmasked full-extent.

Precision/layout strategy:
 - Projections / FFN / Wo run as fp8e4m3 DoubleRow matmuls (2 contraction
   rows per pass over e-tile pairs, 4x PE rate vs f32r). Weights are scaled
   x64 and activations x16 into fp8; every PSUM result is descaled by
   2^-10 in its PSUM->SBUF copy (engine-alternated between DVE and Act).
 - K^T/Q^T are bf16 (scores at full PE rate); P is 16*exp(score/8), stored
   fp8 on unmasked k-tile pairs (DoubleRow PV) and bf16 on masked tiles.
   The ones-column of V (=16) carries the softmax denominator; the x16
   cancels in the division.
 - exp alternates between Act (native Exp) and DVE (Schraudolph bit-trick:
   bits = int(A*score + B) reinterpreted as bf16/e4m3), balancing the
   otherwise Act-bound attention spans.
"""

import numpy as np
import ml_dtypes

import concourse.bass as bass
import concourse.bacc as bacc
import concourse.mybir as mybir
import concourse.tile as tile
from concourse.bass_utils import run_bass_kernel_spmd

F32 = mybir.dt.float32
F32R = mybir.dt.float32r
BF16 = mybir.dt.bfloat16
FP8 = mybir.dt.float8e4
I8 = mybir.dt.int8
I16 = mybir.dt.int16
AFT = mybir.ActivationFunctionType
ALU = mybir.AluOpType
AXL = mybir.AxisListType
DR = mybir.MatmulPerfMode.DoubleRow

B, S, D, H, DK, FF = 4, 2048, 512, 8, 64, 2048
P = 128            # partitions
ET = D // P        # 4 e-tiles of 128 over the model dim
EP = ET // 2       # e-tile pairs for DoubleRow
SQ = 1024          # own query rows per core
NKT0, NKT1 = 8, 16  # uniform k-tile extents for SA pos0 / pos1
EPS = 1e-6

WS = 64.0          # fp8 weight scale
XS = 16.0          # fp8 activation scale
DS = 1.0 / (WS * XS)   # descale after a DoubleRow matmul
LN16 = float(np.log(16.0))
LOG2E = 1.4426950408889634
# Schraudolph exp: bits = trunc(score*A + B); B includes the x16 bias
SCH_A_BF = 128.0 * LOG2E / 8.0
SCH_B_BF = (127.0 + 4.0) * 128.0 - 8.0
SCH_A_F8 = 8.0 * LOG2E / 8.0
SCH_B_F8 = (7.0 + 4.0) * 8.0

OWN_BLOCKS = {0: (0, 3), 1: (1, 2)}
PERM_BLOCKS = {0: (0, 1, 3, 2), 1: (1, 0, 2, 3)}
Q_SRC_QBS = (0, 2)
# combined 4-in-1 transpose PSUM (HW-proven); CoreSim's checker rejects it,
# so debugging scripts can flip this off before build.
COMBINED_TP = True
USE_SCH = True     # DVE Schraudolph exp offload
USE_PAIRS = True   # fp8 DoubleRow PV on unmasked k-tile pairs
DEBUG_TAPS = False  # dump intermediates to DRAM for debugging


def _r(ap, pattern, **kw):
    return ap.rearrange(pattern, **kw)


def build_program(bias_flags):
    """Build the SPMD Bass program. bias_flags: dict of bools saying which
    folded biases are nonzero (uniform across cores)."""
    nc = bacc.Bacc("TRN2", target_bir_lowering=False, debug=False, num_devices=8)

    def din(name, shape, dt=F32):
        return nc.dram_tensor(name, shape, dt, kind="ExternalInput").ap()

    xkv_d = din("xkv", [S, D])
    enc_d = din("enc", [S, D])
    masks_d = din("masks", [16, P, 512], BF16)
    msrc_d = din("msrc", [P, S // P])
    ident_d = din("ident", [P, P])
    # fp8 weights, pre-scaled x64
    w_sa = {k: din(f"sa_{k}", [D, D], FP8) for k in ("wq", "wk", "wv", "wo")}
    w_ca = {k: din(f"ca_{k}", [D, D], FP8) for k in ("wq", "wk", "wv", "wo")}
    w1_d = {k: din(f"w1{k}", [D, FF], FP8) for k in "ab"}
    w2_d = {k: din(f"w2{k}", [FF, D], FP8) for k in "ab"}
    # folded biases (pre-scaled x1024), [1, D] / [1, FF]
    b_sa = {k: din(f"bsa_{k}", [1, D]) for k in ("q", "k", "v", "o")}
    b_ca = {k: din(f"bca_{k}", [1, D]) for k in ("q", "k", "v", "o")}
    b1_d = din("b1", [1, FF])
    b2_d = din("b2", [1, D])
    out_d = nc.dram_tensor("out", [SQ, D], F32, kind="ExternalOutput").ap()
    taps = {}
    taps_live = {}
    if DEBUG_TAPS:
        for nm, shape, dt in [
                ("dbg_kvT8", [P, ET * S], FP8),
                ("dbg_ktT", [P, ET * S], BF16),
                ("dbg_qtT", [P, ET * SQ], BF16),
                ("dbg_va", [P, 16 * H * 66], FP8),
                ("dbg_atT8", [P, ET * SQ], FP8),
                ("dbg_x1", [P, (SQ // P) * D], F32),
                ("dbg_x2", [P, (SQ // P) * D], F32)]:
            taps[nm] = nc.dram_tensor(nm, shape, dt,
                                      kind="ExternalOutput").ap()

    with tile.TileContext(nc) as tc:
        with (
            tc.tile_pool(name="persist", bufs=1) as pp,
            tc.tile_pool(name="ln_sb", bufs=6) as lnp,
            tc.tile_pool(name="p_sb", bufs=5) as psb,
            tc.tile_pool(name="ln_st", bufs=4) as stp,
            tc.tile_pool(name="sc_ps", bufs=3, space="PSUM") as scp,
            tc.tile_pool(name="acc_ps", bufs=1, space="PSUM") as accp,
        ):
            ident = pp.tile([P, P], F32R, tag="ident")
            nc.sync.dma_start(ident[:], ident_d[:].bitcast(F32R))
            ones1f = pp.tile([1, P], F32, tag="ones1f")
            nc.vector.memset(ones1f[:], 1.0)
            ones1 = pp.tile([1, P], F32R, tag="ones1")
            nc.vector.tensor_copy(ones1[:], ones1f[:])
            c16_bf = pp.tile([1, P], BF16, tag="c16_bf")
            nc.vector.memset(c16_bf[:], 16.0)
            ln16 = pp.tile([P, 1], F32, tag="ln16")
            nc.vector.memset(ln16[:], LN16)
            msrc = pp.tile([P, S // P], F32, tag="msrc")
            nc.sync.dma_start(msrc[:], msrc_d[:])
            masks_sb = pp.tile([P, 16 * 512], BF16, tag="masks")

            def load_w(dram, name, cols=D):
                # [cin, cols] -> sbuf [128, ET, cols] fp8
                t = pp.tile([P, ET * cols], FP8, tag=name)
                nc.sync.dma_start(
                    _r(t[:], "p (e d) -> p e d", e=ET),
                    _r(dram[:], "(e p) d -> p e d", p=P))
                return t

            def load_bias(dram, name, flag, n=D):
                if not flag:
                    return None
                t = pp.tile([1, n], F32R, tag=name)
                nc.sync.dma_start(t[:], dram[:].bitcast(F32R))
                return t

            def pv8(t8):
                return _r(t8[:], "p (e s) -> p e s", e=ET)

            def copy_scaled(dst, src, c, on_act):
                """PSUM->SBUF copy with scale, engine-balanced."""
                with nc.allow_low_precision("fp8/bf16 staging"):
                    if on_act:
                        nc.scalar.activation(dst, src, AFT.Copy, scale=c)
                    else:
                        nc.vector.tensor_scalar_mul(dst, src, c)

            def ln_stats(x_t, sx_act=False):
                """x_t: [128, 512] f32 sbuf -> (scale, bias) per-row [128,1]."""
                sx = stp.tile([P, 1], F32, tag="sx")
                dump = lnp.tile([P, D], F32, tag="ln_xn")
                sq = stp.tile([P, 1], F32, tag="sq")
                nc.scalar.activation(dump[:], x_t[:], AFT.Square,
                                     accum_out=sq[:])
                if sx_act:
                    dump2 = lnp.tile([P, D], F32, tag="ln_xn", name="dump2")
                    nc.scalar.activation(dump2[:], x_t[:], AFT.Identity,
                                         accum_out=sx[:])
                else:
                    nc.vector.reduce_sum(sx[:], x_t[:], axis=AXL.X)
                mu = stp.tile([P, 1], F32, tag="mu")
                nc.vector.tensor_scalar_mul(mu[:], sx[:], 1.0 / D)
                m2 = stp.tile([P, 1], F32, tag="m2")
                nc.vector.tensor_mul(m2[:], mu[:], mu[:])
                v1 = stp.tile([P, 1], F32, tag="v1")
                nc.vector.tensor_scalar(v1[:], m2[:], -float(D), None,
                                        op0=ALU.mult)
                nc.vector.tensor_add(v1[:], v1[:], sq[:])
                std = stp.tile([P, 1], F32, tag="std")
                nc.scalar.activation(std[:], v1[:], AFT.Sqrt,
                                     scale=1.0 / (D - 1))
                nc.vector.tensor_scalar_add(std[:], std[:], EPS)
                s = stp.tile([P, 1], F32, tag="s")
                nc.vector.reciprocal(s[:], std[:])
                nb = stp.tile([P, 1], F32, tag="nb")
                nc.vector.tensor_mul(nb[:], mu[:], s[:])
                nc.vector.tensor_scalar_mul(nb[:], nb[:], -1.0)
                return s, nb

            def transpose4(xn, dstT8, rows, t, on_act):
                """Transpose [128, 512] f32r sbuf tile t into dstT8
                [128, ET*rows] fp8 (x16): 4 e-transposes, 1 scaled copy."""
                if COMBINED_TP:
                    ps = scp.tile([P, 1024], F32R, tag="score",
                                  name="tp")[:, 0:512]
                    for e in range(ET):
                        nc.tensor.matmul(
                            ps[:, e * P:(e + 1) * P],
                            xn[:, e * P:(e + 1) * P], ident[:],
                            start=(e == 0), stop=(e == ET - 1),
                            is_transpose=True, skip_group_check=(e != 0))
                    copy_scaled(
                        _r(dstT8[:], "p (e s) -> p e s", e=ET)[
                            :, :, t * P:(t + 1) * P],
                        _r(ps[:], "p (e c) -> p e c", e=ET), XS, on_act)
                else:
                    for e in range(ET):
                        ps = scp.tile([P, 1024], F32R, tag="score",
                                      name="tp")[:, 0:P]
                        nc.tensor.matmul(
                            ps[:], xn[:, e * P:(e + 1) * P], ident[:],
                            start=True, stop=True, is_transpose=True)
                        copy_scaled(
                            _r(dstT8[:], "p (e s) -> p e s", e=ET)[
                                :, e, t * P:(t + 1) * P],
                            ps[:], XS, on_act)

            def ln_transpose(src_d, rows, dstT8, do_ln=True):
                nt = rows // P
                for t in range(nt):
                    if do_ln:
                        x_t = lnp.tile([P, D], F32, tag="ln_x")
                        nc.sync.dma_start(x_t[:], src_d[t * P:(t + 1) * P, :])
                        s, nb = ln_stats(x_t, sx_act=(t % 3 != 0))
                        xn = lnp.tile([P, D], F32R, tag="ln_xn")
                        nc.vector.tensor_scalar(xn[:], x_t[:], s[:], nb[:],
                                                op0=ALU.mult, op1=ALU.add)
                        transpose4(xn[:], dstT8, rows, t,
                                   on_act=(t % 2 == 0))
                    else:
                        xn = lnp.tile([P, D], F32R, tag="ln_x")
                        nc.sync.dma_start(
                            xn[:], src_d[t * P:(t + 1) * P, :].bitcast(F32R))
                        transpose4(xn[:], dstT8, rows, t, on_act=(t % 2 == 0))

            def ln_transpose_sbuf(xsb, dstT8, tiles=None):
                for t in (tiles if tiles is not None else range(SQ // P)):
                    x_t = xsb[:, t * D:(t + 1) * D]
                    s, nb = ln_stats(x_t)
                    xn = lnp.tile([P, D], F32R, tag="ln_xn")
                    nc.vector.tensor_scalar(xn[:], x_t, s[:], nb[:],
                                            op0=ALU.mult, op1=ALU.add)
                    transpose4(xn[:], dstT8, SQ, t, on_act=(t % 2 == 0))

            def projT(wt8, bt, has_b, srcT8, src_rows, dstT, src_qbs=None):
                """dstT[:, dt, :] = bf16 (W.T @ xn.T)-slice via DoubleRow."""
                if src_qbs is None:
                    src_qbs = list(range(src_rows // 512))
                nqb = len(src_qbs)
                for dt in range(ET):
                    for dqb, qb in enumerate(src_qbs):
                        ps = scp.tile([P, 1024], F32, tag="score",
                                      name="ps")[:, 0:512]
                        for ep in range(EP):
                            nc.tensor.matmul(
                                ps[:],
                                pv8(wt8)[:, 2 * ep:2 * ep + 2,
                                         dt * P:(dt + 1) * P],
                                pv8(srcT8)[:, 2 * ep:2 * ep + 2,
                                           qb * 512:(qb + 1) * 512],
                                start=(ep == 0),
                                stop=(ep == EP - 1 and not has_b),
                                perf_mode=DR)
                            if ep == EP - 1 and has_b:
                                nc.tensor.matmul(
                                    ps[:], bt[:, dt * P:(dt + 1) * P],
                                    ones1[:, 0:512].bitcast(F32R),
                                    start=False, stop=True)
                        copy_scaled(
                            dstT[:, dt * nqb * 512 + dqb * 512:
                                 dt * nqb * 512 + (dqb + 1) * 512],
                            ps[:], DS, on_act=((dt + dqb) % 2 == 0))

            def proj_va(wt8, bt, has_b, srcT8, src_rows, va):
                """V projection, token-major: va [128, nkt*8*66] fp8 = 16*V
                (+ src-mask row scaling), ones column = 16."""
                nkt = src_rows // P
                for kt in range(nkt):
                    ps = scp.tile([P, 1024], F32, tag="score",
                                  name="ps")[:, 0:512]
                    for ep in range(EP):
                        nc.tensor.matmul(
                            ps[:],
                            pv8(srcT8)[:, 2 * ep:2 * ep + 2,
                                       kt * P:(kt + 1) * P],
                            pv8(wt8)[:, 2 * ep:2 * ep + 2, 0:D],
                            start=(ep == 0),
                            stop=(ep == EP - 1 and not has_b),
                            perf_mode=DR)
                        if ep == EP - 1 and has_b:
                            nc.tensor.matmul(
                                ps[:], ones1[:, 0:P], bt[:],
                                start=False, stop=True)
                    dst = _r(va[:], "p (t h c) -> p t h c", t=nkt, h=H)
                    if bias_flags.get("msrc1"):
                        copy_scaled(dst[:, kt, :, 0:DK],
                                    _r(ps[:], "p (h c) -> p h c", h=H),
                                    XS * DS, on_act=(kt % 2 == 0))
                    else:
                        with nc.allow_low_precision("fp8 va"):
                            nc.vector.tensor_scalar(
                                dst[:, kt, :, 0:DK],
                                _r(ps[:], "p (h c) -> p h c", h=H),
                                msrc[:, kt:kt + 1], XS * DS,
                                op0=ALU.mult, op1=ALU.mult)

            def attention(ktT, va, qtT, nkts, masked, exp_dve, wo8, bo,
                          has_bo, resid, x_out, on_pos_done=None):
                """ktT [128, 4*S_k] bf16; va [128, nkt*8*66] fp8 (16*V);
                qtT [128, 4*1024] bf16; masked: fn(pos,kt)-> mask idx|None;
                exp_dve: fn(pos,kt)->bool; x_out [128,8*512] f32 resid+attn."""
                skmax = max(nkts) * P
                atT8 = pp.tile([P, ET * SQ], FP8, tag="attnT")
                taps_live["atT8"] = atT8
                for pos, nkt in enumerate(nkts):
                    units = []
                    kt = 0
                    while kt < nkt:
                        if (USE_PAIRS and masked(pos, kt) is None
                                and kt + 1 < nkt
                                and masked(pos, kt + 1) is None):
                            units.append((kt, kt + 1))
                            kt += 2
                        else:
                            units.append((kt,))
                            kt += 1
                    for hp in range(H // 2):
                        acc = [accp.tile([P, 512], F32, tag=f"acc{i}",
                                         name=f"acc{i}")
                               for i in range(2)]

                        def emit_score(kt):
                            st = scp.tile([P, 1024], F32, tag="score")
                            for i in range(2):
                                nc.tensor.matmul(
                                    st[:, i * 512:(i + 1) * 512],
                                    ktT[i * DK:(i + 1) * DK,
                                        hp * skmax + kt * P:
                                        hp * skmax + (kt + 1) * P],
                                    qtT[i * DK:(i + 1) * DK,
                                        hp * SQ + pos * 512:
                                        hp * SQ + (pos + 1) * 512],
                                    start=True, stop=True)
                            return st

                        def do_exp(pos, kt, st, pair=None):
                            """pair: ([128,2048] int8 tile, j) for fp8 pair
                            halves; None -> bf16 p_t (masked path)."""
                            on_dve = USE_SCH and exp_dve(pos, kt)
                            if pair is not None:
                                t8, j = pair
                                dst = t8[:, j * 1024:(j + 1) * 1024]
                                if on_dve:
                                    with nc.allow_low_precision("sch exp"):
                                        nc.vector.tensor_scalar(
                                            dst, st[:], SCH_A_F8, SCH_B_F8,
                                            op0=ALU.mult, op1=ALU.add)
                                else:
                                    nc.scalar.activation(
                                        dst.bitcast(FP8), st[:], AFT.Exp,
                                        bias=ln16[:], scale=1.0 / 8.0)
                                return None
                            p_t = psb.tile([P, 1024], I16, tag="p")
                            if on_dve:
                                with nc.allow_low_precision("sch exp"):
                                    nc.vector.tensor_scalar(
                                        p_t[:], st[:], SCH_A_BF, SCH_B_BF,
                                        op0=ALU.mult, op1=ALU.add)
                            else:
                                nc.scalar.activation(
                                    p_t[:].bitcast(BF16), st[:], AFT.Exp,
                                    bias=ln16[:], scale=1.0 / 8.0)
                            return p_t

                        flat = [kt for u in units for kt in u]
                        sts = {flat[0]: emit_score(flat[0])}

                        def prefetch(kt):
                            fi = flat.index(kt) + 1
                            if fi < len(flat):
                                sts[flat[fi]] = emit_score(flat[fi])

                        vat = _r(va[:], "p (t h c) -> p t h c",
                                 t=S // P, h=H)
                        for u in units:
                            if len(u) == 2:
                                k0, k1 = u
                                p2 = psb.tile([P, 2048], I8, tag="p2")
                                for j, kt in enumerate(u):
                                    st = sts.pop(kt)
                                    prefetch(kt)
                                    do_exp(pos, kt, st, pair=(p2, j))
                                p8 = p2[:].bitcast(FP8)
                                for i in range(2):
                                    h = 2 * hp + i
                                    nc.tensor.matmul(
                                        acc[i][0:DK + 2, :],
                                        vat[:, k0:k0 + 2, h, 0:66],
                                        _r(p8, "p (j x) -> p j x", j=2)[
                                            :, :, i * 512:(i + 1) * 512],
                                        start=(k0 == 0),
                                        stop=(k1 == nkt - 1),
                                        perf_mode=DR)
                            else:
                                kt = u[0]
                                st = sts.pop(kt)
                                prefetch(kt)
                                p_t = do_exp(pos, kt, st)
                                pb = p_t[:].bitcast(BF16)
                                mi = masked(pos, kt)
                                if mi is not None:
                                    mt = masks_sb[:, mi * 512:(mi + 1) * 512]
                                    for i in range(2):
                                        nc.vector.tensor_mul(
                                            pb[:, i * 512:(i + 1) * 512],
                                            pb[:, i * 512:(i + 1) * 512],
                                            mt)
                                for i in range(2):
                                    h = 2 * hp + i
                                    nc.tensor.matmul(
                                        acc[i][0:DK + 2, :],
                                        vat[:, kt, h, 0:66],
                                        pb[:, i * 512:(i + 1) * 512],
                                        start=(kt == 0), stop=(kt == nkt - 1))
                        # epilogue: atT8 = 16 * acc/denom (fp8)
                        rcl = []
                        for i in range(2):
                            rc = lnp.tile([1, 512], BF16, tag="ln_xn",
                                          name="rc")
                            with nc.allow_low_precision("softmax denom"):
                                nc.vector.reciprocal(
                                    rc[:], acc[i][DK:DK + 1, :])
                            rcl.append(rc)
                        rbl = []
                        for i in range(2):
                            rbs = lnp.tile([DK, 512], BF16, tag="ln_xn",
                                           name="rbs")
                            nc.gpsimd.partition_broadcast(rbs[:], rcl[i][:])
                            rbl.append(rbs)
                        for i in range(2):
                            with nc.allow_low_precision("fp8 attn out"):
                                nc.vector.tensor_tensor(
                                    atT8[i * DK:(i + 1) * DK,
                                         hp * SQ + pos * 512:
                                         hp * SQ + (pos + 1) * 512],
                                    acc[i][0:DK, :], rbl[i][:],
                                    op=ALU.mult)
                    # output projection + residual for this pos block
                    for qt in range(pos * 4, (pos + 1) * 4):
                        res = resid(qt)
                        ps = scp.tile([P, 1024], F32, tag="score",
                                      name="ps")[:, 0:512]
                        for ep in range(EP):
                            nc.tensor.matmul(
                                ps[:],
                                pv8(atT8)[:, 2 * ep:2 * ep + 2,
                                          qt * P:(qt + 1) * P],
                                pv8(wo8)[:, 2 * ep:2 * ep + 2, 0:D],
                                start=(ep == 0),
                                stop=(ep == EP - 1 and not has_bo),
                                perf_mode=DR)
                            if ep == EP - 1 and has_bo:
                                nc.tensor.matmul(
                                    ps[:], ones1[:, 0:P], bo[:],
                                    start=False, stop=True)
                        tmp = lnp.tile([P, D], F32, tag="ln_xn", name="wot")
                        nc.scalar.activation(tmp[:], ps[:], AFT.Copy,
                                             scale=DS)
                        nc.vector.tensor_tensor(
                            x_out[:, qt * D:(qt + 1) * D], tmp[:],
                            res, op=ALU.add)
                    if on_pos_done is not None:
                        on_pos_done(pos)

            # ---------------- stage A: LN0 + transposes ----------------
            kvT8 = pp.tile([P, ET * S], FP8, tag="kvT")
            ln_transpose(xkv_d, S, kvT8, do_ln=True)

            # ---------------- weights (all upfront, Pool DGE queue) -------
            wk8 = load_w(w_sa["wk"], "w_a")
            wq8 = load_w(w_sa["wq"], "w_b")
            wv8 = load_w(w_sa["wv"], "w_c")
            wo8 = load_w(w_sa["wo"], "w_d")
            ck8 = load_w(w_ca["wk"], "w_e")
            cq8 = load_w(w_ca["wq"], "w_f")
            cv8 = load_w(w_ca["wv"], "w_g")
            co8 = load_w(w_ca["wo"], "w_h")
            w18 = [load_w(w1_d[k], f"w1{k}", cols=FF) for k in "ab"]
            w28 = []
            for k in "ab":
                t = pp.tile([P, (FF // P) * D], FP8, tag=f"w2{k}")
                nc.sync.dma_start(
                    _r(t[:], "p (t d) -> p t d", t=FF // P),
                    _r(w2_d[k][:], "(t p) d -> p t d", p=P))
                w28.append(t)
            bk = load_bias(b_sa["k"], "b_a", bias_flags["sa_k"])
            bq = load_bias(b_sa["q"], "b_b", bias_flags["sa_q"])
            bv = load_bias(b_sa["v"], "b_c", bias_flags["sa_v"])
            bo = load_bias(b_sa["o"], "b_d", bias_flags["sa_o"])
            cbk = load_bias(b_ca["k"], "b_e", bias_flags["ca_k"])
            cbq = load_bias(b_ca["q"], "b_f", bias_flags["ca_q"])
            cbv = load_bias(b_ca["v"], "b_g", bias_flags["ca_v"])
            cbo = load_bias(b_ca["o"], "b_h", bias_flags["ca_o"])
            b1_sb = load_bias(b1_d, "b1", bias_flags["ff1"], n=FF)
            b2_sb = load_bias(b2_d, "b2", bias_flags["ff2"])
            nc.sync.dma_start(
                _r(masks_sb[:], "p (t c) -> p t c", t=16),
                _r(masks_d[:], "t p c -> p t c"))

            # ---------------- stage B: SA projections ----------------
            ktT_sa = pp.tile([P, ET * S], BF16, tag="ktT")
            qtT_sa = pp.tile([P, ET * SQ], BF16, tag="qtT")
            va_sa = pp.tile([P, 16 * H * 66], FP8, tag="va")
            nc.vector.memset(
                _r(va_sa[:], "p (t h c) -> p t h c", t=16, h=H)[:, :, :, DK:DK + 1],
                1.0)
            nc.vector.memset(
                _r(va_sa[:], "p (t h c) -> p t h c", t=16, h=H)[:, :, :, DK + 1:],
                0.0)
            if DEBUG_TAPS:
                nc.sync.dma_start(taps["dbg_kvT8"][:], kvT8[:])
            projT(wk8, bk, bias_flags["sa_k"], kvT8, S, ktT_sa)
            projT(wq8, bq, bias_flags["sa_q"], kvT8, S, qtT_sa,
                  src_qbs=list(Q_SRC_QBS))
            proj_va(wv8, bv, bias_flags["sa_v"], kvT8, S, va_sa)

            if DEBUG_TAPS:
                nc.sync.dma_start(taps["dbg_ktT"][:], ktT_sa[:])
                nc.sync.dma_start(taps["dbg_qtT"][:], qtT_sa[:])
                nc.sync.dma_start(taps["dbg_va"][:], va_sa[:])
            # ---------------- stage C/D: SA attention + Wo ----------------
            x1 = pp.tile([P, (SQ // P) * D], F32, tag="x1")

            def sa_masked(pos, kt):
                return kt if (pos == 0 or kt >= 8) else None

            def sa_exp_dve(pos, kt):
                if sa_masked(pos, kt) is None:
                    return kt % 4 == 1
                return (kt % 8) == 2

            def q_src_row(qt):
                pos, j = divmod(qt, 4)
                return Q_SRC_QBS[pos] * 512 + j * P

            def sa_resid(qt):
                rt = lnp.tile([P, D], F32, tag="ln_x", name="sa_resid")
                r0 = q_src_row(qt)
                nc.sync.dma_start(rt[:], xkv_d[r0:r0 + P, :])
                return rt[:]

            # hoisted CA prep: encoder transpose overlaps SA attention
            encT8 = pp.tile([P, ET * S], FP8, tag="kvT")  # reuse kvT slot
            ln_transpose(enc_d, S, encT8, do_ln=False)

            attention(ktT_sa, va_sa, qtT_sa, [NKT0, NKT1], sa_masked,
                      sa_exp_dve, wo8, bo, bias_flags["sa_o"], sa_resid, x1)

            if DEBUG_TAPS:
                nc.sync.dma_start(taps["dbg_x1"][:], x1[:])
                nc.sync.dma_start(taps["dbg_atT8"][:],
                                  taps_live["atT8"][:])
            # ---------------- stage E/F: CA ----------------
            ktT_ca = pp.tile([P, ET * S], BF16, tag="ktT")
            qtT_ca = pp.tile([P, ET * SQ], BF16, tag="qtT")
            va_ca = pp.tile([P, 16 * H * 66], FP8, tag="va")
            nc.vector.memset(
                _r(va_ca[:], "p (t h c) -> p t h c", t=16, h=H)[:, :, :, DK:DK + 1],
                1.0)
            nc.vector.memset(
                _r(va_ca[:], "p (t h c) -> p t h c", t=16, h=H)[:, :, :, DK + 1:],
                0.0)
            projT(ck8, cbk, bias_flags["ca_k"], encT8, S, ktT_ca)
            proj_va(cv8, cbv, bias_flags["ca_v"], encT8, S, va_ca)

            h1T8 = pp.tile([P, ET * SQ], FP8, tag="hT")
            ln_transpose_sbuf(x1, h1T8)
            projT(cq8, cbq, bias_flags["ca_q"], h1T8, SQ, qtT_ca)

            x2 = pp.tile([P, (SQ // P) * D], F32, tag="x2")

            attention(ktT_ca, va_ca, qtT_ca, [16, 16], lambda p, k: None,
                      lambda p, k: (k % 2 == 1) and (k % 16 != 15), co8,
                      cbo,
                      bias_flags["ca_o"],
                      lambda qt: x1[:, qt * D:(qt + 1) * D], x2)

            if DEBUG_TAPS:
                nc.sync.dma_start(taps["dbg_x2"][:], x2[:])
            # ---------------- stage G: LN2 + FFN ----------------
            h2T8 = pp.tile([P, ET * SQ], FP8, tag="hT")
            ln_transpose_sbuf(x2, h2T8)

            ffT8 = pp.tile([P, (FF // P) * SQ], FP8, tag="ffT")

            def ffn2_qt(qt):
                ps = scp.tile([P, 1024], F32, tag="score",
                              name="ps")[:, 0:512]
                nfp = FF // P // 2
                for wi, wt in enumerate(w28):
                    for fp in range(nfp):
                        nc.tensor.matmul(
                            ps[:],
                            _r(ffT8[:], "p (t s) -> p t s", t=FF // P)[
                                :, 2 * fp:2 * fp + 2, qt * P:(qt + 1) * P],
                            _r(wt[:], "p (t d) -> p t d", t=FF // P)[
                                :, 2 * fp:2 * fp + 2, :],
                            start=(wi == 0 and fp == 0),
                            stop=(wi == 1 and fp == nfp - 1
                                  and not bias_flags["ff2"]),
                            perf_mode=DR)
                if bias_flags["ff2"]:
                    nc.tensor.matmul(
                        ps[:], ones1[:, 0:P], b2_sb[:],
                        start=False, stop=True)
                tmp = lnp.tile([P, D], F32, tag="ln_xn", name="ff2t")
                copy_scaled(tmp[:], ps[:], DS, on_act=(qt % 2 == 0))
                o_t = lnp.tile([P, D], F32, tag="ln_xn", name="o_t")
                nc.vector.tensor_tensor(
                    o_t[:], tmp[:], x2[:, qt * D:(qt + 1) * D], op=ALU.add)
                nc.sync.dma_start(out_d[qt * P:(qt + 1) * P, :], o_t[:])

            for qb in range(SQ // 512):
                if qb == 1:
                    for qt in range(4):
                        ffn2_qt(qt)
                for ft in range(FF // P):
                    ps = scp.tile([P, 1024], F32, tag="score",
                                  name="ps")[:, 0:512]
                    for wi, wt in enumerate(w18):
                        for ep in range(EP):
                            last = (wi == 1 and ep == EP - 1)
                            nc.tensor.matmul(
                                ps[:],
                                _r(wt[:], "p (e f) -> p e f", e=ET)[
                                    :, 2 * ep:2 * ep + 2,
                                    ft * P:(ft + 1) * P],
                                pv8(h2T8)[:, 2 * ep:2 * ep + 2,
                                          qb * 512:(qb + 1) * 512],
                                start=(wi == 0 and ep == 0),
                                stop=(last and not bias_flags["ff1"]),
                                perf_mode=DR)
                            if last and bias_flags["ff1"]:
                                nc.tensor.matmul(
                                    ps[:], b1_sb[:, ft * P:(ft + 1) * P],
                                    ones1[:, 0:512].bitcast(F32R),
                                    start=False, stop=True)
                    # ffT8 = 16*relu(z); alternate Act/DVE per ft
                    if ft % 2 == 0:
                        nc.scalar.activation(
                            ffT8[:, ft * SQ + qb * 512:
                                 ft * SQ + (qb + 1) * 512],
                            ps[:], AFT.Relu, scale=XS * DS)
                    else:
                        with nc.allow_low_precision("fp8 relu"):
                            nc.vector.tensor_scalar(
                                ffT8[:, ft * SQ + qb * 512:
                                     ft * SQ + (qb + 1) * 512],
                                ps[:], 0.0, XS * DS,
                                op0=ALU.max, op1=ALU.mult)
            for qt in range(4, SQ // P):
                ffn2_qt(qt)

    nc.finalize()
    return nc


_CACHE = {}
LAST_EXEC_NS = None


def kernel(**inputs):
    x = np.asarray(inputs["x"], np.float32)
    enc = np.asarray(inputs["encoder_output"], np.float32)
    src_mask = np.asarray(inputs["src_mask"]).reshape(S)
    tgt_mask = np.asarray(inputs["tgt_mask"]).reshape(S, S)

    def fold(w, g, b, extra_b):
        w = np.asarray(w, np.float32)
        wf = np.asarray(g, np.float32)[:, None] * w
        bf = np.asarray(b, np.float32) @ w + np.asarray(extra_b, np.float32)
        return wf, bf

    def q8(w):
        return np.asarray(w * WS, np.float32).astype(ml_dtypes.float8_e4m3)

    def q8r(w):
        ws = np.asarray(w * WS, np.float32)
        return (ws - ws.astype(ml_dtypes.float8_e4m3)
                .astype(np.float32)).astype(ml_dtypes.float8_e4m3)

    z = np.zeros(D, np.float32)
    sa_wq, bsa_q = fold(inputs["sa_wq"], inputs["ln0_g"], inputs["ln0_b"], z)
    sa_wk, bsa_k = fold(inputs["sa_wk"], inputs["ln0_g"], inputs["ln0_b"], z)
    sa_wv, bsa_v = fold(inputs["sa_wv"], inputs["ln0_g"], inputs["ln0_b"], z)
    sa_wo = np.asarray(inputs["sa_wo"], np.float32)
    bsa_o = np.asarray(inputs["sa_bo"], np.float32)
    ca_wq, bca_q = fold(inputs["ca_wq"], inputs["ln1_g"], inputs["ln1_b"], z)
    ca_wk = np.asarray(inputs["ca_wk"], np.float32)
    bca_k = np.zeros(D, np.float32)
    ca_wv = np.asarray(inputs["ca_wv"], np.float32)
    bca_v = np.zeros(D, np.float32)
    ca_wo = np.asarray(inputs["ca_wo"], np.float32)
    bca_o = np.asarray(inputs["ca_bo"], np.float32)
    w1, b1 = fold(inputs["ff_w1"], inputs["ln2_g"], inputs["ln2_b"],
                  np.asarray(inputs["ff_b1"], np.float32))
    w2 = np.asarray(inputs["ff_w2"], np.float32)
    b2 = np.asarray(inputs["ff_b2"], np.float32)

    bias_flags = {
        "sa_q": bool(np.any(bsa_q)), "sa_k": bool(np.any(bsa_k)),
        "sa_v": bool(np.any(bsa_v)), "sa_o": bool(np.any(bsa_o)),
        "ca_q": bool(np.any(bca_q)), "ca_k": bool(np.any(bca_k)),
        "ca_v": bool(np.any(bca_v)), "ca_o": bool(np.any(bca_o)),
        "ff1": bool(np.any(b1)), "ff2": bool(np.any(b2)),
        "msrc1": bool(np.all(src_mask == 1)),
    }

    key = tuple(sorted(bias_flags.items()))
    if key not in _CACHE:
        _CACHE[key] = build_program(bias_flags)
    nc = _CACHE[key]

    ident = np.eye(P, dtype=np.float32)
    msrc = src_mask.astype(np.float32).reshape(S // P, P).T.copy()
    BS = WS * XS  # bias pre-scale (descaled by DS in the psum copy)

    shared = {
        "ident": ident, "msrc": msrc,
        "sa_wq": q8(sa_wq), "sa_wk": q8(sa_wk), "sa_wv": q8(sa_wv),
        "sa_wo": q8(sa_wo),
        "ca_wq": q8(ca_wq), "ca_wk": q8(ca_wk), "ca_wv": q8(ca_wv),
        "ca_wo": q8(ca_wo),
        "w1a": q8(w1), "w1b": q8r(w1), "w2a": q8(w2), "w2b": q8r(w2),
        "bsa_q": bsa_q[None] * BS, "bsa_k": bsa_k[None] * BS,
        "bsa_v": bsa_v[None] * BS, "bsa_o": bsa_o[None] * BS,
        "bca_q": bca_q[None] * BS, "bca_k": bca_k[None] * BS,
        "bca_v": bca_v[None] * BS, "bca_o": bca_o[None] * BS,
        "b1": b1[None] * BS, "b2": b2[None] * BS,
    }

    in_maps = []
    for c in range(8):
        b, r = divmod(c, 2)
        perm = PERM_BLOCKS[r]
        rows = np.concatenate(
            [np.arange(gb * 512, (gb + 1) * 512) for gb in perm])
        gb0, gb1 = OWN_BLOCKS[r]
        assert perm[0] == gb0 and perm[2] == gb1
        mk = np.zeros((16, P, 512), np.float32)
        for pos, gb in enumerate((gb0, gb1)):
            qs = slice(gb * 512, (gb + 1) * 512)
            mrow = tgt_mask[qs][:, rows]
            for j in range(8):
                kt = j if pos == 0 else 8 + j
                ks = slice(kt * P, (kt + 1) * P)
                mk[pos * 8 + j] = mrow[:, ks].T
            ext = (NKT0 if pos == 0 else NKT1) * P
            assert not np.any(mrow[:, ext:]), "tgt_mask beyond extent"
        im = dict(shared)
        im["xkv"] = np.ascontiguousarray(x[b][rows])
        im["enc"] = np.ascontiguousarray(enc[b])
        im["masks"] = mk.astype(ml_dtypes.bfloat16)
        in_maps.append(im)

    res = run_bass_kernel_spmd(nc, in_maps, core_ids=list(range(8)))
    global LAST_EXEC_NS
    LAST_EXEC_NS = res.exec_time_ns

    out = np.empty((B, S, D), np.float32)
    for c in range(8):
        b, r = divmod(c, 2)
        gb0, gb1 = OWN_BLOCKS[r]
        o = res.results[c]["out"]
        out[b, gb0 * 512:(gb0 + 1) * 512] = o[0:512]
        out[b, gb1 * 512:(gb1 + 1) * 512] = o[512:1024]
    return out


# revision 92
# speedup vs baseline: 1.0504x; 1.0006x over previous
"""Trainium2 Bass kernel for a pre-LN transformer decoder block.

Shapes (hardcoded): B=4, S_TGT=S_SRC=2048, D=512, H=8, DK=64, FF=2048, fp32.

Sharding: 8 cores; core c handles batch c//2. The two cores of a batch split
the 2048 query rows into two causal-balanced groups of 2x512 rows:
  r0: global q-blocks [0:512) and [1536:2048)
  r1: global q-blocks [512:1024) and [1024:1536)
All cores run one identical SPMD program. Keys (and the x rows feeding K/V)
are PERMUTED per core so that the own q-blocks land at canonical positions:
  pi = [own0 | filler0 | own1 | filler1]   (4 blocks of 512 rows)
With this order both ranks see SA extents of 8 k-tiles (pos0) and 16 (pos1),
diagonal mask tiles align, and Q^T is just columns {block0, block2} of the
transposed/normalized x. Per-core visibility is carried by mask DATA built
on the host. Cross-attention is unmasked full-extent.

Precision/layout strategy:
 - Projections / FFN / Wo run as fp8e4m3 DoubleRow matmuls (2 contraction
   rows per pass over e-tile pairs, 4x PE rate vs f32r). Weights are scaled
   x64 and activations x16 into fp8; every PSUM result is descaled by
   2^-10 in its PSUM->SBUF copy (engine-alternated between DVE and Act).
 - K^T/Q^T are bf16 (scores at full PE rate); P is 16*exp(score/8), stored
   fp8 on unmasked k-tile pairs (DoubleRow PV) and bf16 on masked tiles.
   The ones-column of V (=16) carries the softmax denominator; the x16
   cancels in the division.
 - exp alternates between Act (native Exp) and DVE (Schraudolph bit-trick:
   bits = int(A*score + B) reinterpreted as bf16/e4m3), balancing the
   otherwise Act-bound attention spans.
"""

import numpy as np
import ml_dtypes

import concourse.bass as bass
import concourse.bacc as bacc
import concourse.mybir as mybir
import concourse.tile as tile
from concourse.bass_utils import run_bass_kernel_spmd

F32 = mybir.dt.float32
F32R = mybir.dt.float32r
BF16 = mybir.dt.bfloat16
FP8 = mybir.dt.float8e4
I8 = mybir.dt.int8
I16 = mybir.dt.int16
AFT = mybir.ActivationFunctionType
ALU = mybir.AluOpType
AXL = mybir.AxisListType
DR = mybir.MatmulPerfMode.DoubleRow

B, S, D, H, DK, FF = 4, 2048, 512, 8, 64, 2048
P = 128            # partitions
ET = D // P        # 4 e-tiles of 128 over the model dim
EP = ET // 2       # e-tile pairs for DoubleRow
SQ = 1024          # own query rows per core
NKT0, NKT1 = 8, 16  # uniform k-tile extents for SA pos0 / pos1
EPS = 1e-6

WS = 64.0          # fp8 weight scale
XS = 16.0          # fp8 activation scale
DS = 1.0 / (WS * XS)   # descale after a DoubleRow matmul
LN16 = float(np.log(16.0))
LOG2E = 1.4426950408889634
# Schraudolph exp: bits = trunc(score*A + B); B includes the x16 bias
SCH_A_BF = 128.0 * LOG2E / 8.0
SCH_B_BF = (127.0 + 4.0) * 128.0 - 8.0
SCH_A_F8 = 8.0 * LOG2E / 8.0
SCH_B_F8 = (7.0 + 4.0) * 8.0

OWN_BLOCKS = {0: (0, 3), 1: (1, 2)}
PERM_BLOCKS = {0: (0, 1, 3, 2), 1: (1, 0, 2, 3)}
Q_SRC_QBS = (0, 2)
# combined 4-in-1 transpose PSUM (HW-proven); CoreSim's checker rejects it,
# so debugging scripts can flip this off before build.
COMBINED_TP = True
USE_SCH = True     # DVE Schraudolph exp offload
USE_PAIRS = True   # fp8 DoubleRow PV on unmasked k-tile pairs
DEBUG_TAPS = False  # dump intermediates to DRAM for debugging


def _r(ap, pattern, **kw):
    return ap.rearrange(pattern, **kw)


def build_program(bias_flags):
    """Build the SPMD Bass program. bias_flags: dict of bools saying which
    folded biases are nonzero (uniform across cores)."""
    nc = bacc.Bacc("TRN2", target_bir_lowering=False, debug=False, num_devices=8)

    def din(name, shape, dt=F32):
        return nc.dram_tensor(name, shape, dt, kind="ExternalInput").ap()

    xkv_d = din("xkv", [S, D])
    enc_d = din("enc", [S, D])
    masks_d = din("masks", [16, P, 512], BF16)
    msrc_d = din("msrc", [P, S // P])
    ident_d = din("ident", [P, P])
    # fp8 weights, pre-scaled x64
    w_sa = {k: din(f"sa_{k}", [D, D], FP8) for k in ("wq", "wk", "wv", "wo")}
    w_ca = {k: din(f"ca_{k}", [D, D], FP8) for k in ("wq", "wk", "wv", "wo")}
    w1_d = {k: din(f"w1{k}", [D, FF], FP8) for k in "ab"}
    w2_d = {k: din(f"w2{k}", [FF, D], FP8) for k in "ab"}
    # folded biases (pre-scaled x1024), [1, D] / [1, FF]
    b_sa = {k: din(f"bsa_{k}", [1, D]) for k in ("q", "k", "v", "o")}
    b_ca = {k: din(f"bca_{k}", [1, D]) for k in ("q", "k", "v", "o")}
    b1_d = din("b1", [1, FF])
    b2_d = din("b2", [1, D])
    out_d = nc.dram_tensor("out", [SQ, D], F32, kind="ExternalOutput").ap()
    taps = {}
    taps_live = {}
    if DEBUG_TAPS:
        for nm, shape, dt in [
                ("dbg_kvT8", [P, ET * S], FP8),
                ("dbg_ktT", [P, ET * S], BF16),
                ("dbg_qtT", [P, ET * SQ], BF16),
                ("dbg_va", [P, 16 * H * 66], FP8),
                ("dbg_atT8", [P, ET * SQ], FP8),
                ("dbg_x1", [P, (SQ // P) * D], F32),
                ("dbg_x2", [P, (SQ // P) * D], F32)]:
            taps[nm] = nc.dram_tensor(nm, shape, dt,
                                      kind="ExternalOutput").ap()

    with tile.TileContext(nc) as tc:
        with (
            tc.tile_pool(name="persist", bufs=1) as pp,
            tc.tile_pool(name="ln_sb", bufs=6) as lnp,
            tc.tile_pool(name="p_sb", bufs=5) as psb,
            tc.tile_pool(name="ln_st", bufs=4) as stp,
            tc.tile_pool(name="sc_ps", bufs=3, space="PSUM") as scp,
            tc.tile_pool(name="acc_ps", bufs=1, space="PSUM") as accp,
        ):
            ident = pp.tile([P, P], F32R, tag="ident")
            nc.sync.dma_start(ident[:], ident_d[:].bitcast(F32R))
            ones1f = pp.tile([1, P], F32, tag="ones1f")
            nc.vector.memset(ones1f[:], 1.0)
            ones1 = pp.tile([1, P], F32R, tag="ones1")
            nc.vector.tensor_copy(ones1[:], ones1f[:])
            c16_bf = pp.tile([1, P], BF16, tag="c16_bf")
            nc.vector.memset(c16_bf[:], 16.0)
            ln16 = pp.tile([P, 1], F32, tag="ln16")
            nc.vector.memset(ln16[:], LN16)
            msrc = pp.tile([P, S // P], F32, tag="msrc")
            nc.sync.dma_start(msrc[:], msrc_d[:])
            masks_sb = pp.tile([P, 16 * 512], BF16, tag="masks")

            def load_w(dram, name, cols=D):
                # [cin, cols] -> sbuf [128, ET, cols] fp8
                t = pp.tile([P, ET * cols], FP8, tag=name)
                nc.sync.dma_start(
                    _r(t[:], "p (e d) -> p e d", e=ET),
                    _r(dram[:], "(e p) d -> p e d", p=P))
                return t

            def load_bias(dram, name, flag, n=D):
                if not flag:
                    return None
                t = pp.tile([1, n], F32R, tag=name)
                nc.sync.dma_start(t[:], dram[:].bitcast(F32R))
                return t

            def pv8(t8):
                return _r(t8[:], "p (e s) -> p e s", e=ET)

            def copy_scaled(dst, src, c, on_act):
                """PSUM->SBUF copy with scale, engine-balanced."""
                with nc.allow_low_precision("fp8/bf16 staging"):
                    if on_act:
                        nc.scalar.activation(dst, src, AFT.Copy, scale=c)
                    else:
                        nc.vector.tensor_scalar_mul(dst, src, c)

            def ln_stats(x_t, sx_act=False):
                """x_t: [128, 512] f32 sbuf -> (scale, bias) per-row [128,1]."""
                sx = stp.tile([P, 1], F32, tag="sx")
                dump = lnp.tile([P, D], F32, tag="ln_xn")
                sq = stp.tile([P, 1], F32, tag="sq")
                nc.scalar.activation(dump[:], x_t[:], AFT.Square,
                                     accum_out=sq[:])
                if sx_act:
                    dump2 = lnp.tile([P, D], F32, tag="ln_xn", name="dump2")
                    nc.scalar.activation(dump2[:], x_t[:], AFT.Identity,
                                         accum_out=sx[:])
                else:
                    nc.vector.reduce_sum(sx[:], x_t[:], axis=AXL.X)
                mu = stp.tile([P, 1], F32, tag="mu")
                nc.vector.tensor_scalar_mul(mu[:], sx[:], 1.0 / D)
                m2 = stp.tile([P, 1], F32, tag="m2")
                nc.vector.tensor_mul(m2[:], mu[:], mu[:])
                v1 = stp.tile([P, 1], F32, tag="v1")
                nc.vector.tensor_scalar(v1[:], m2[:], -float(D), None,
                                        op0=ALU.mult)
                nc.vector.tensor_add(v1[:], v1[:], sq[:])
                std = stp.tile([P, 1], F32, tag="std")
                nc.scalar.activation(std[:], v1[:], AFT.Sqrt,
                                     scale=1.0 / (D - 1))
                nc.vector.tensor_scalar_add(std[:], std[:], EPS)
                s = stp.tile([P, 1], F32, tag="s")
                nc.vector.reciprocal(s[:], std[:])
                nb = stp.tile([P, 1], F32, tag="nb")
                nc.vector.tensor_mul(nb[:], mu[:], s[:])
                nc.vector.tensor_scalar_mul(nb[:], nb[:], -1.0)
                return s, nb

            def transpose4(xn, dstT8, rows, t, on_act):
                """Transpose [128, 512] f32r sbuf tile t into dstT8
                [128, ET*rows] fp8 (x16): 4 e-transposes, 1 scaled copy."""
                if COMBINED_TP:
                    ps = scp.tile([P, 1024], F32R, tag="score",
                                  name="tp")[:, 0:512]
                    for e in range(ET):
                        nc.tensor.matmul(
                            ps[:, e * P:(e + 1) * P],
                            xn[:, e * P:(e + 1) * P], ident[:],
                            start=(e == 0), stop=(e == ET - 1),
                            is_transpose=True, skip_group_check=(e != 0))
                    copy_scaled(
                        _r(dstT8[:], "p (e s) -> p e s", e=ET)[
                            :, :, t * P:(t + 1) * P],
                        _r(ps[:], "p (e c) -> p e c", e=ET), XS, on_act)
                else:
                    for e in range(ET):
                        ps = scp.tile([P, 1024], F32R, tag="score",
                                      name="tp")[:, 0:P]
                        nc.tensor.matmul(
                            ps[:], xn[:, e * P:(e + 1) * P], ident[:],
                            start=True, stop=True, is_transpose=True)
                        copy_scaled(
                            _r(dstT8[:], "p (e s) -> p e s", e=ET)[
                                :, e, t * P:(t + 1) * P],
                            ps[:], XS, on_act)

            def ln_transpose(src_d, rows, dstT8, do_ln=True):
                nt = rows // P
                for t in range(nt):
                    if do_ln:
                        x_t = lnp.tile([P, D], F32, tag="ln_x")
                        nc.sync.dma_start(x_t[:], src_d[t * P:(t + 1) * P, :])
                        s, nb = ln_stats(x_t, sx_act=(t % 3 != 0))
                        xn = lnp.tile([P, D], F32R, tag="ln_xn")
                        nc.vector.tensor_scalar(xn[:], x_t[:], s[:], nb[:],
                                                op0=ALU.mult, op1=ALU.add)
                        transpose4(xn[:], dstT8, rows, t,
                                   on_act=(t % 2 == 0))
                    else:
                        xn = lnp.tile([P, D], F32R, tag="ln_x")
                        nc.sync.dma_start(
                            xn[:], src_d[t * P:(t + 1) * P, :].bitcast(F32R))
                        transpose4(xn[:], dstT8, rows, t, on_act=(t % 2 == 0))

            def ln_transpose_sbuf(xsb, dstT8, tiles=None):
                for t in (tiles if tiles is not None else range(SQ // P)):
                    x_t = xsb[:, t * D:(t + 1) * D]
                    s, nb = ln_stats(x_t)
                    xn = lnp.tile([P, D], F32R, tag="ln_xn")
                    nc.vector.tensor_scalar(xn[:], x_t, s[:], nb[:],
                                            op0=ALU.mult, op1=ALU.add)
                    transpose4(xn[:], dstT8, SQ, t, on_act=(t % 2 == 0))

            def projT(wt8, bt, has_b, srcT8, src_rows, dstT, src_qbs=None):
                """dstT[:, dt, :] = bf16 (W.T @ xn.T)-slice via DoubleRow."""
                if src_qbs is None:
                    src_qbs = list(range(src_rows // 512))
                nqb = len(src_qbs)
                for dt in range(ET):
                    for dqb, qb in enumerate(src_qbs):
                        ps = scp.tile([P, 1024], F32, tag="score",
                                      name="ps")[:, 0:512]
                        for ep in range(EP):
                            nc.tensor.matmul(
                                ps[:],
                                pv8(wt8)[:, 2 * ep:2 * ep + 2,
                                         dt * P:(dt + 1) * P],
                                pv8(srcT8)[:, 2 * ep:2 * ep + 2,
                                           qb * 512:(qb + 1) * 512],
                                start=(ep == 0),
                                stop=(ep == EP - 1 and not has_b),
                                perf_mode=DR)
                            if ep == EP - 1 and has_b:
                                nc.tensor.matmul(
                                    ps[:], bt[:, dt * P:(dt + 1) * P],
                                    ones1[:, 0:512].bitcast(F32R),
                                    start=False, stop=True)
                        copy_scaled(
                            dstT[:, dt * nqb * 512 + dqb * 512:
                                 dt * nqb * 512 + (dqb + 1) * 512],
                            ps[:], DS, on_act=((dt + dqb) % 3 != 0))

            def proj_va(wt8, bt, has_b, srcT8, src_rows, va):
                """V projection, token-major: va [128, nkt*8*66] fp8 = 16*V
                (+ src-mask row scaling), ones column = 16."""
                nkt = src_rows // P
                for kt in range(nkt):
                    ps = scp.tile([P, 1024], F32, tag="score",
                                  name="ps")[:, 0:512]
                    for ep in range(EP):
                        nc.tensor.matmul(
                            ps[:],
                            pv8(srcT8)[:, 2 * ep:2 * ep + 2,
                                       kt * P:(kt + 1) * P],
                            pv8(wt8)[:, 2 * ep:2 * ep + 2, 0:D],
                            start=(ep == 0),
                            stop=(ep == EP - 1 and not has_b),
                            perf_mode=DR)
                        if ep == EP - 1 and has_b:
                            nc.tensor.matmul(
                                ps[:], ones1[:, 0:P], bt[:],
                                start=False, stop=True)
                    dst = _r(va[:], "p (t h c) -> p t h c", t=nkt, h=H)
                    if bias_flags.get("msrc1"):
                        copy_scaled(dst[:, kt, :, 0:DK],
                                    _r(ps[:], "p (h c) -> p h c", h=H),
                                    XS * DS, on_act=(kt % 2 == 0))
                    else:
                        with nc.allow_low_precision("fp8 va"):
                            nc.vector.tensor_scalar(
                                dst[:, kt, :, 0:DK],
                                _r(ps[:], "p (h c) -> p h c", h=H),
                                msrc[:, kt:kt + 1], XS * DS,
                                op0=ALU.mult, op1=ALU.mult)

            def attention(ktT, va, qtT, nkts, masked, exp_dve, wo8, bo,
                          has_bo, resid, x_out, on_pos_done=None):
                """ktT [128, 4*S_k] bf16; va [128, nkt*8*66] fp8 (16*V);
                qtT [128, 4*1024] bf16; masked: fn(pos,kt)-> mask idx|None;
                exp_dve: fn(pos,kt)->bool; x_out [128,8*512] f32 resid+attn."""
                skmax = max(nkts) * P
                atT8 = pp.tile([P, ET * SQ], FP8, tag="attnT")
                taps_live["atT8"] = atT8
                for pos, nkt in enumerate(nkts):
                    units = []
                    kt = 0
                    while kt < nkt:
                        if (USE_PAIRS and masked(pos, kt) is None
                                and kt + 1 < nkt
                                and masked(pos, kt + 1) is None):
                            units.append((kt, kt + 1))
                            kt += 2
                        else:
                            units.append((kt,))
                            kt += 1
                    for hp in range(H // 2):
                        acc = [accp.tile([P, 512], F32, tag=f"acc{i}",
                                         name=f"acc{i}")
                               for i in range(2)]

                        def emit_score(kt):
                            st = scp.tile([P, 1024], F32, tag="score")
                            for i in range(2):
                                nc.tensor.matmul(
                                    st[:, i * 512:(i + 1) * 512],
                                    ktT[i * DK:(i + 1) * DK,
                                        hp * skmax + kt * P:
                                        hp * skmax + (kt + 1) * P],
                                    qtT[i * DK:(i + 1) * DK,
                                        hp * SQ + pos * 512:
                                        hp * SQ + (pos + 1) * 512],
                                    start=True, stop=True)
                            return st

                        def do_exp(pos, kt, st, pair=None):
                            """pair: ([128,2048] int8 tile, j) for fp8 pair
                            halves; None -> bf16 p_t (masked path)."""
                            on_dve = USE_SCH and exp_dve(pos, kt)
                            if pair is not None:
                                t8, j = pair
                                dst = t8[:, j * 1024:(j + 1) * 1024]
                                if on_dve:
                                    with nc.allow_low_precision("sch exp"):
                                        nc.vector.tensor_scalar(
                                            dst, st[:], SCH_A_F8, SCH_B_F8,
                                            op0=ALU.mult, op1=ALU.add)
                                else:
                                    nc.scalar.activation(
                                        dst.bitcast(FP8), st[:], AFT.Exp,
                                        bias=ln16[:], scale=1.0 / 8.0)
                                return None
                            p_t = psb.tile([P, 1024], I16, tag="p")
                            if on_dve:
                                with nc.allow_low_precision("sch exp"):
                                    nc.vector.tensor_scalar(
                                        p_t[:], st[:], SCH_A_BF, SCH_B_BF,
                                        op0=ALU.mult, op1=ALU.add)
                            else:
                                nc.scalar.activation(
                                    p_t[:].bitcast(BF16), st[:], AFT.Exp,
                                    bias=ln16[:], scale=1.0 / 8.0)
                            return p_t

                        flat = [kt for u in units for kt in u]
                        sts = {flat[0]: emit_score(flat[0])}

                        def prefetch(kt):
                            fi = flat.index(kt) + 1
                            if fi < len(flat):
                                sts[flat[fi]] = emit_score(flat[fi])

                        vat = _r(va[:], "p (t h c) -> p t h c",
                                 t=S // P, h=H)
                        for u in units:
                            if len(u) == 2:
                                k0, k1 = u
                                p2 = psb.tile([P, 2048], I8, tag="p2")
                                for j, kt in enumerate(u):
                                    st = sts.pop(kt)
                                    prefetch(kt)
                                    do_exp(pos, kt, st, pair=(p2, j))
                                p8 = p2[:].bitcast(FP8)
                                for i in range(2):
                                    h = 2 * hp + i
                                    nc.tensor.matmul(
                                        acc[i][0:DK + 2, :],
                                        vat[:, k0:k0 + 2, h, 0:66],
                                        _r(p8, "p (j x) -> p j x", j=2)[
                                            :, :, i * 512:(i + 1) * 512],
                                        start=(k0 == 0),
                                        stop=(k1 == nkt - 1),
                                        perf_mode=DR)
                            else:
                                kt = u[0]
                                st = sts.pop(kt)
                                prefetch(kt)
                                p_t = do_exp(pos, kt, st)
                                pb = p_t[:].bitcast(BF16)
                                mi = masked(pos, kt)
                                if mi is not None:
                                    mt = masks_sb[:, mi * 512:(mi + 1) * 512]
                                    for i in range(2):
                                        nc.vector.tensor_mul(
                                            pb[:, i * 512:(i + 1) * 512],
                                            pb[:, i * 512:(i + 1) * 512],
                                            mt)
                                for i in range(2):
                                    h = 2 * hp + i
                                    nc.tensor.matmul(
                                        acc[i][0:DK + 2, :],
                                        vat[:, kt, h, 0:66],
                                        pb[:, i * 512:(i + 1) * 512],
                                        start=(kt == 0), stop=(kt == nkt - 1))
                        # epilogue: atT8 = 16 * acc/denom (fp8)
                        rcl = []
                        for i in range(2):
                            rc = lnp.tile([1, 512], BF16, tag="ln_xn",
                                          name="rc")
                            with nc.allow_low_precision("softmax denom"):
                                nc.vector.reciprocal(
                                    rc[:], acc[i][DK:DK + 1, :])
                            rcl.append(rc)
                        rbl = []
                        for i in range(2):
                            rbs = lnp.tile([DK, 512], BF16, tag="ln_xn",
                                           name="rbs")
                            nc.gpsimd.partition_broadcast(rbs[:], rcl[i][:])
                            rbl.append(rbs)
                        for i in range(2):
                            with nc.allow_low_precision("fp8 attn out"):
                                nc.vector.tensor_tensor(
                                    atT8[i * DK:(i + 1) * DK,
                                         hp * SQ + pos * 512:
                                         hp * SQ + (pos + 1) * 512],
                                    acc[i][0:DK, :], rbl[i][:],
                                    op=ALU.mult)
                    # output projection + residual for this pos block
                    for qt in range(pos * 4, (pos + 1) * 4):
                        res = resid(qt)
                        ps = scp.tile([P, 1024], F32, tag="score",
                                      name="ps")[:, 0:512]
                        for ep in range(EP):
                            nc.tensor.matmul(
                                ps[:],
                                pv8(atT8)[:, 2 * ep:2 * ep + 2,
                                          qt * P:(qt + 1) * P],
                                pv8(wo8)[:, 2 * ep:2 * ep + 2, 0:D],
                                start=(ep == 0),
                                stop=(ep == EP - 1 and not has_bo),
                                perf_mode=DR)
                            if ep == EP - 1 and has_bo:
                                nc.tensor.matmul(
                                    ps[:], ones1[:, 0:P], bo[:],
                                    start=False, stop=True)
                        tmp = lnp.tile([P, D], F32, tag="ln_xn", name="wot")
                        nc.scalar.activation(tmp[:], ps[:], AFT.Copy,
                                             scale=DS)
                        nc.vector.tensor_tensor(
                            x_out[:, qt * D:(qt + 1) * D], tmp[:],
                            res, op=ALU.add)
                    if on_pos_done is not None:
                        on_pos_done(pos)

            # ---------------- stage A: LN0 + transposes ----------------
            kvT8 = pp.tile([P, ET * S], FP8, tag="kvT")
            ln_transpose(xkv_d, S, kvT8, do_ln=True)

            # ---------------- weights (all upfront, Pool DGE queue) -------
            wk8 = load_w(w_sa["wk"], "w_a")
            wq8 = load_w(w_sa["wq"], "w_b")
            wv8 = load_w(w_sa["wv"], "w_c")
            wo8 = load_w(w_sa["wo"], "w_d")
            ck8 = load_w(w_ca["wk"], "w_e")
            cq8 = load_w(w_ca["wq"], "w_f")
            cv8 = load_w(w_ca["wv"], "w_g")
            co8 = load_w(w_ca["wo"], "w_h")
            w18 = [load_w(w1_d[k], f"w1{k}", cols=FF) for k in "ab"]
            w28 = []
            for k in "ab":
                t = pp.tile([P, (FF // P) * D], FP8, tag=f"w2{k}")
                nc.sync.dma_start(
                    _r(t[:], "p (t d) -> p t d", t=FF // P),
                    _r(w2_d[k][:], "(t p) d -> p t d", p=P))
                w28.append(t)
            bk = load_bias(b_sa["k"], "b_a", bias_flags["sa_k"])
            bq = load_bias(b_sa["q"], "b_b", bias_flags["sa_q"])
            bv = load_bias(b_sa["v"], "b_c", bias_flags["sa_v"])
            bo = load_bias(b_sa["o"], "b_d", bias_flags["sa_o"])
            cbk = load_bias(b_ca["k"], "b_e", bias_flags["ca_k"])
            cbq = load_bias(b_ca["q"], "b_f", bias_flags["ca_q"])
            cbv = load_bias(b_ca["v"], "b_g", bias_flags["ca_v"])
            cbo = load_bias(b_ca["o"], "b_h", bias_flags["ca_o"])
            b1_sb = load_bias(b1_d, "b1", bias_flags["ff1"], n=FF)
            b2_sb = load_bias(b2_d, "b2", bias_flags["ff2"])
            nc.sync.dma_start(
                _r(masks_sb[:], "p (t c) -> p t c", t=16),
                _r(masks_d[:], "t p c -> p t c"))

            # ---------------- stage B: SA projections ----------------
            ktT_sa = pp.tile([P, ET * S], BF16, tag="ktT")
            qtT_sa = pp.tile([P, ET * SQ], BF16, tag="qtT")
            va_sa = pp.tile([P, 16 * H * 66], FP8, tag="va")
            nc.vector.memset(
                _r(va_sa[:], "p (t h c) -> p t h c", t=16, h=H)[:, :, :, DK:DK + 1],
                1.0)
            nc.vector.memset(
                _r(va_sa[:], "p (t h c) -> p t h c", t=16, h=H)[:, :, :, DK + 1:],
                0.0)
            if DEBUG_TAPS:
                nc.sync.dma_start(taps["dbg_kvT8"][:], kvT8[:])
            projT(wk8, bk, bias_flags["sa_k"], kvT8, S, ktT_sa)
            projT(wq8, bq, bias_flags["sa_q"], kvT8, S, qtT_sa,
                  src_qbs=list(Q_SRC_QBS))
            proj_va(wv8, bv, bias_flags["sa_v"], kvT8, S, va_sa)

            if DEBUG_TAPS:
                nc.sync.dma_start(taps["dbg_ktT"][:], ktT_sa[:])
                nc.sync.dma_start(taps["dbg_qtT"][:], qtT_sa[:])
                nc.sync.dma_start(taps["dbg_va"][:], va_sa[:])
            # ---------------- stage C/D: SA attention + Wo ----------------
            x1 = pp.tile([P, (SQ // P) * D], F32, tag="x1")

            def sa_masked(pos, kt):
                return kt if (pos == 0 or kt >= 8) else None

            def sa_exp_dve(pos, kt):
                if sa_masked(pos, kt) is None:
                    return kt % 4 == 1
                return (kt % 8) == 2

            def q_src_row(qt):
                pos, j = divmod(qt, 4)
                return Q_SRC_QBS[pos] * 512 + j * P

            def sa_resid(qt):
                rt = lnp.tile([P, D], F32, tag="ln_x", name="sa_resid")
                r0 = q_src_row(qt)
                nc.sync.dma_start(rt[:], xkv_d[r0:r0 + P, :])
                return rt[:]

            # hoisted CA prep: encoder transpose overlaps SA attention
            encT8 = pp.tile([P, ET * S], FP8, tag="kvT")  # reuse kvT slot
            ln_transpose(enc_d, S, encT8, do_ln=False)

            attention(ktT_sa, va_sa, qtT_sa, [NKT0, NKT1], sa_masked,
                      sa_exp_dve, wo8, bo, bias_flags["sa_o"], sa_resid, x1)

            if DEBUG_TAPS:
                nc.sync.dma_start(taps["dbg_x1"][:], x1[:])
                nc.sync.dma_start(taps["dbg_atT8"][:],
                                  taps_live["atT8"][:])
            # ---------------- stage E/F: CA ----------------
            ktT_ca = pp.tile([P, ET * S], BF16, tag="ktT")
            qtT_ca = pp.tile([P, ET * SQ], BF16, tag="qtT")
            va_ca = pp.tile([P, 16 * H * 66], FP8, tag="va")
            nc.vector.memset(
                _r(va_ca[:], "p (t h c) -> p t h c", t=16, h=H)[:, :, :, DK:DK + 1],
                1.0)
            nc.vector.memset(
                _r(va_ca[:], "p (t h c) -> p t h c", t=16, h=H)[:, :, :, DK + 1:],
                0.0)
            projT(ck8, cbk, bias_flags["ca_k"], encT8, S, ktT_ca)
            proj_va(cv8, cbv, bias_flags["ca_v"], encT8, S, va_ca)

            h1T8 = pp.tile([P, ET * SQ], FP8, tag="hT")
            ln_transpose_sbuf(x1, h1T8)
            projT(cq8, cbq, bias_flags["ca_q"], h1T8, SQ, qtT_ca)

            x2 = pp.tile([P, (SQ // P) * D], F32, tag="x2")

            attention(ktT_ca, va_ca, qtT_ca, [16, 16], lambda p, k: None,
                      lambda p, k: (k % 2 == 1) and (k % 16 != 15), co8,
                      cbo,
                      bias_flags["ca_o"],
                      lambda qt: x1[:, qt * D:(qt + 1) * D], x2)

            if DEBUG_TAPS:
                nc.sync.dma_start(taps["dbg_x2"][:], x2[:])
            # ---------------- stage G: LN2 + FFN ----------------
            h2T8 = pp.tile([P, ET * SQ], FP8, tag="hT")
            ln_transpose_sbuf(x2, h2T8)

            ffT8 = pp.tile([P, (FF // P) * SQ], FP8, tag="ffT")

            def ffn2_qt(qt):
                ps = scp.tile([P, 1024], F32, tag="score",
                              name="ps")[:, 0:512]
                nfp = FF // P // 2
                for wi, wt in enumerate(w28):
                    for fp in range(nfp):
                        nc.tensor.matmul(
                            ps[:],
                            _r(ffT8[:], "p (t s) -> p t s", t=FF // P)[
                                :, 2 * fp:2 * fp + 2, qt * P:(qt + 1) * P],
                            _r(wt[:], "p (t d) -> p t d", t=FF // P)[
                                :, 2 * fp:2 * fp + 2, :],
                            start=(wi == 0 and fp == 0),
                            stop=(wi == 1 and fp == nfp - 1
                                  and not bias_flags["ff2"]),
                            perf_mode=DR)
                if bias_flags["ff2"]:
                    nc.tensor.matmul(
                        ps[:], ones1[:, 0:P], b2_sb[:],
                        start=False, stop=True)
                tmp = lnp.tile([P, D], F32, tag="ln_xn", name="ff2t")
                copy_scaled(tmp[:], ps[:], DS, on_act=(qt % 2 == 0))
                o_t = lnp.tile([P, D], F32, tag="ln_xn", name="o_t")
                nc.vector.tensor_tensor(
                    o_t[:], tmp[:], x2[:, qt * D:(qt + 1) * D], op=ALU.add)
                nc.sync.dma_start(out_d[qt * P:(qt + 1) * P, :], o_t[:])

            for qb in range(SQ // 512):
                if qb == 1:
                    for qt in range(4):
                        ffn2_qt(qt)
                for ft in range(FF // P):
                    ps = scp.tile([P, 1024], F32, tag="score",
                                  name="ps")[:, 0:512]
                    for wi, wt in enumerate(w18):
                        for ep in range(EP):
                            last = (wi == 1 and ep == EP - 1)
                            nc.tensor.matmul(
                                ps[:],
                                _r(wt[:], "p (e f) -> p e f", e=ET)[
                                    :, 2 * ep:2 * ep + 2,
                                    ft * P:(ft + 1) * P],
                                pv8(h2T8)[:, 2 * ep:2 * ep + 2,
                                          qb * 512:(qb + 1) * 512],
                                start=(wi == 0 and ep == 0),
                                stop=(last and not bias_flags["ff1"]),
                                perf_mode=DR)
                            if last and bias_flags["ff1"]:
                                nc.tensor.matmul(
                                    ps[:], b1_sb[:, ft * P:(ft + 1) * P],
                                    ones1[:, 0:512].bitcast(F32R),
                                    start=False, stop=True)
                    # ffT8 = 16*relu(z); alternate Act/DVE per ft
                    if ft % 2 == 0:
                        nc.scalar.activation(
                            ffT8[:, ft * SQ + qb * 512:
                                 ft * SQ + (qb + 1) * 512],
                            ps[:], AFT.Relu, scale=XS * DS)
                    else:
                        with nc.allow_low_precision("fp8 relu"):
                            nc.vector.tensor_scalar(
                                ffT8[:, ft * SQ + qb * 512:
                                     ft * SQ + (qb + 1) * 512],
                                ps[:], 0.0, XS * DS,
                                op0=ALU.max, op1=ALU.mult)
            for qt in range(4, SQ // P):
                ffn2_qt(qt)

    nc.finalize()
    return nc


_CACHE = {}
LAST_EXEC_NS = None


def kernel(**inputs):
    x = np.asarray(inputs["x"], np.float32)
    enc = np.asarray(inputs["encoder_output"], np.float32)
    src_mask = np.asarray(inputs["src_mask"]).reshape(S)
    tgt_mask = np.asarray(inputs["tgt_mask"]).reshape(S, S)

    def fold(w, g, b, extra_b):
        w = np.asarray(w, np.float32)
        wf = np.asarray(g, np.float32)[:, None] * w
        bf = np.asarray(b, np.float32) @ w + np.asarray(extra_b, np.float32)
        return wf, bf

    def q8(w):
        return np.asarray(w * WS, np.float32).astype(ml_dtypes.float8_e4m3)

    def q8r(w):
        ws = np.asarray(w * WS, np.float32)
        return (ws - ws.astype(ml_dtypes.float8_e4m3)
                .astype(np.float32)).astype(ml_dtypes.float8_e4m3)

    z = np.zeros(D, np.float32)
    sa_wq, bsa_q = fold(inputs["sa_wq"], inputs["ln0_g"], inputs["ln0_b"], z)
    sa_wk, bsa_k = fold(inputs["sa_wk"], inputs["ln0_g"], inputs["ln0_b"], z)
    sa_wv, bsa_v = fold(inputs["sa_wv"], inputs["ln0_g"], inputs["ln0_b"], z)
    sa_wo = np.asarray(inputs["sa_wo"], np.float32)
    bsa_o = np.asarray(inputs["sa_bo"], np.float32)
    ca_wq, bca_q = fold(inputs["ca_wq"], inputs["ln1_g"], inputs["ln1_b"], z)
    ca_wk = np.asarray(inputs["ca_wk"], np.float32)
    bca_k = np.zeros(D, np.float32)
    ca_wv = np.asarray(inputs["ca_wv"], np.float32)
    bca_v = np.zeros(D, np.float32)
    ca_wo = np.asarray(inputs["ca_wo"], np.float32)
    bca_o = np.asarray(inputs["ca_bo"], np.float32)
    w1, b1 = fold(inputs["ff_w1"], inputs["ln2_g"], inputs["ln2_b"],
                  np.asarray(inputs["ff_b1"], np.float32))
    w2 = np.asarray(inputs["ff_w2"], np.float32)
    b2 = np.asarray(inputs["ff_b2"], np.float32)

    bias_flags = {
        "sa_q": bool(np.any(bsa_q)), "sa_k": bool(np.any(bsa_k)),
        "sa_v": bool(np.any(bsa_v)), "sa_o": bool(np.any(bsa_o)),
        "ca_q": bool(np.any(bca_q)), "ca_k": bool(np.any(bca_k)),
        "ca_v": bool(np.any(bca_v)), "ca_o": bool(np.any(bca_o)),
        "ff1": bool(np.any(b1)), "ff2": bool(np.any(b2)),
        "msrc1": bool(np.all(src_mask == 1)),
    }

    key = tuple(sorted(bias_flags.items()))
    if key not in _CACHE:
        _CACHE[key] = build_program(bias_flags)
    nc = _CACHE[key]

    ident = np.eye(P, dtype=np.float32)
    msrc = src_mask.astype(np.float32).reshape(S // P, P).T.copy()
    BS = WS * XS  # bias pre-scale (descaled by DS in the psum copy)

    shared = {
        "ident": ident, "msrc": msrc,
        "sa_wq": q8(sa_wq), "sa_wk": q8(sa_wk), "sa_wv": q8(sa_wv),
        "sa_wo": q8(sa_wo),
        "ca_wq": q8(ca_wq), "ca_wk": q8(ca_wk), "ca_wv": q8(ca_wv),
        "ca_wo": q8(ca_wo),
        "w1a": q8(w1), "w1b": q8r(w1), "w2a": q8(w2), "w2b": q8r(w2),
        "bsa_q": bsa_q[None] * BS, "bsa_k": bsa_k[None] * BS,
        "bsa_v": bsa_v[None] * BS, "bsa_o": bsa_o[None] * BS,
        "bca_q": bca_q[None] * BS, "bca_k": bca_k[None] * BS,
        "bca_v": bca_v[None] * BS, "bca_o": bca_o[None] * BS,
        "b1": b1[None] * BS, "b2": b2[None] * BS,
    }

    in_maps = []
    for c in range(8):
        b, r = divmod(c, 2)
        perm = PERM_BLOCKS[r]
        rows = np.concatenate(
            [np.arange(gb * 512, (gb + 1) * 512) for gb in perm])
        gb0, gb1 = OWN_BLOCKS[r]
        assert perm[0] == gb0 and perm[2] == gb1
        mk = np.zeros((16, P, 512), np.float32)
        for pos, gb in enumerate((gb0, gb1)):
            qs = slice(gb * 512, (gb + 1) * 512)
            mrow = tgt_mask[qs][:, rows]
            for j in range(8):
                kt = j if pos == 0 else 8 + j
                ks = slice(kt * P, (kt + 1) * P)
                mk[pos * 8 + j] = mrow[:, ks].T
            ext = (NKT0 if pos == 0 else NKT1) * P
            assert not np.any(mrow[:, ext:]), "tgt_mask beyond extent"
        im = dict(shared)
        im["xkv"] = np.ascontiguousarray(x[b][rows])
        im["enc"] = np.ascontiguousarray(enc[b])
        im["masks"] = mk.astype(ml_dtypes.bfloat16)
        in_maps.append(im)

    res = run_bass_kernel_spmd(nc, in_maps, core_ids=list(range(8)))
    global LAST_EXEC_NS
    LAST_EXEC_NS = res.exec_time_ns

    out = np.empty((B, S, D), np.float32)
    for c in range(8):
        b, r = divmod(c, 2)
        gb0, gb1 = OWN_BLOCKS[r]
        o = res.results[c]["out"]
        out[b, gb0 * 512:(gb0 + 1) * 512] = o[0:512]
        out[b, gb1 * 512:(gb1 + 1) * 512] = o[512:1024]
    return out


# revision 99
# speedup vs baseline: 1.0750x; 1.0235x over previous
"""Trainium2 Bass kernel for a pre-LN transformer decoder block.

Shapes (hardcoded): B=4, S_TGT=S_SRC=2048, D=512, H=8, DK=64, FF=2048, fp32.

Sharding: 8 cores; core c handles batch c//2. The two cores of a batch split
the 2048 query rows into two causal-balanced groups of 2x512 rows:
  r0: global q-blocks [0:512) and [1536:2048)
  r1: global q-blocks [512:1024) and [1024:1536)
All cores run one identical SPMD program. Keys (and the x rows feeding K/V)
are PERMUTED per core so that the own q-blocks land at canonical positions:
  pi = [own0 | filler0 | own1 | filler1]   (4 blocks of 512 rows)
With this order both ranks see SA extents of 8 k-tiles (pos0) and 16 (pos1),
diagonal mask tiles align, and Q^T is just columns {block0, block2} of the
transposed/normalized x. Per-core visibility is carried by mask DATA built
on the host. Cross-attention is unmasked full-extent.

Precision/layout strategy:
 - Projections / FFN / Wo run as fp8e4m3 DoubleRow matmuls (2 contraction
   rows per pass over e-tile pairs, 4x PE rate vs f32r). Weights are scaled
   x64 and activations x16 into fp8; every PSUM result is descaled by
   2^-10 in its PSUM->SBUF copy (engine-alternated between DVE and Act).
 - K^T/Q^T are bf16 (scores at full PE rate); P is 16*exp(score/8), stored
   fp8 on unmasked k-tile pairs (DoubleRow PV) and bf16 on masked tiles.
   The ones-column of V (=16) carries the softmax denominator; the x16
   cancels in the division.
 - exp alternates between Act (native Exp) and DVE (Schraudolph bit-trick:
   bits = int(A*score + B) reinterpreted as bf16/e4m3), balancing the
   otherwise Act-bound attention spans.
"""

import numpy as np
import ml_dtypes

import concourse.bass as bass
import concourse.bacc as bacc
import concourse.mybir as mybir
import concourse.tile as tile
from concourse.bass_utils import run_bass_kernel_spmd

F32 = mybir.dt.float32
F32R = mybir.dt.float32r
BF16 = mybir.dt.bfloat16
FP8 = mybir.dt.float8e4
I8 = mybir.dt.int8
I16 = mybir.dt.int16
AFT = mybir.ActivationFunctionType
ALU = mybir.AluOpType
AXL = mybir.AxisListType
DR = mybir.MatmulPerfMode.DoubleRow

B, S, D, H, DK, FF = 4, 2048, 512, 8, 64, 2048
P = 128            # partitions
ET = D // P        # 4 e-tiles of 128 over the model dim
EP = ET // 2       # e-tile pairs for DoubleRow
SQ = 1024          # own query rows per core
NKT0, NKT1 = 8, 16  # uniform k-tile extents for SA pos0 / pos1
EPS = 1e-6

WS = 64.0          # fp8 weight scale
XS = 16.0          # fp8 activation scale
DS = 1.0 / (WS * XS)   # descale after a DoubleRow matmul
LN16 = float(np.log(16.0))
LOG2E = 1.4426950408889634
# Schraudolph exp: bits = trunc(score*A + B); B includes the x16 bias
SCH_A_BF = 128.0 * LOG2E / 8.0
SCH_B_BF = (127.0 + 4.0) * 128.0 - 8.0
SCH_A_F8 = 8.0 * LOG2E / 8.0
SCH_B_F8 = (7.0 + 4.0) * 8.0

OWN_BLOCKS = {0: (0, 3), 1: (1, 2)}
PERM_BLOCKS = {0: (0, 1, 3, 2), 1: (1, 0, 2, 3)}
Q_SRC_QBS = (0, 2)
# combined 4-in-1 transpose PSUM (HW-proven); CoreSim's checker rejects it,
# so debugging scripts can flip this off before build.
COMBINED_TP = True
USE_SCH = True     # DVE Schraudolph exp offload
USE_PAIRS = True   # fp8 DoubleRow PV on unmasked k-tile pairs
DEBUG_TAPS = False  # dump intermediates to DRAM for debugging


def _r(ap, pattern, **kw):
    return ap.rearrange(pattern, **kw)


def build_program(bias_flags):
    """Build the SPMD Bass program. bias_flags: dict of bools saying which
    folded biases are nonzero (uniform across cores)."""
    nc = bacc.Bacc("TRN2", target_bir_lowering=False, debug=False, num_devices=8)

    def din(name, shape, dt=F32):
        return nc.dram_tensor(name, shape, dt, kind="ExternalInput").ap()

    xkv_d = din("xkv", [S, D])
    enc_d = din("enc", [S, D])
    masks_d = din("masks", [16, P, 512], BF16)
    msrc_d = din("msrc", [P, S // P])
    ident_d = din("ident", [P, P])
    # fp8 weights, pre-scaled x64
    w_sa = {k: din(f"sa_{k}", [D, D], FP8) for k in ("wq", "wk", "wv", "wo")}
    w_ca = {k: din(f"ca_{k}", [D, D], FP8) for k in ("wq", "wk", "wv", "wo")}
    w1_d = {k: din(f"w1{k}", [D, FF], FP8) for k in "ab"}
    w2_d = {k: din(f"w2{k}", [FF, D], FP8) for k in "ab"}
    # folded biases (pre-scaled x1024), [1, D] / [1, FF]
    b_sa = {k: din(f"bsa_{k}", [1, D]) for k in ("q", "k", "v", "o")}
    b_ca = {k: din(f"bca_{k}", [1, D]) for k in ("q", "k", "v", "o")}
    b1_d = din("b1", [1, FF])
    b2_d = din("b2", [1, D])
    out_d = nc.dram_tensor("out", [SQ, D], F32, kind="ExternalOutput").ap()
    taps = {}
    taps_live = {}
    if DEBUG_TAPS:
        for nm, shape, dt in [
                ("dbg_kvT8", [P, ET * S], FP8),
                ("dbg_ktT", [P, ET * S], BF16),
                ("dbg_qtT", [P, ET * SQ], BF16),
                ("dbg_va", [P, 16 * H * 66], FP8),
                ("dbg_atT8", [P, ET * SQ], FP8),
                ("dbg_x1", [P, (SQ // P) * D], F32),
                ("dbg_x2", [P, (SQ // P) * D], F32)]:
            taps[nm] = nc.dram_tensor(nm, shape, dt,
                                      kind="ExternalOutput").ap()

    with tile.TileContext(nc) as tc:
        with (
            tc.tile_pool(name="persist", bufs=1) as pp,
            tc.tile_pool(name="ln_sb", bufs=6) as lnp,
            tc.tile_pool(name="p_sb", bufs=5) as psb,
            tc.tile_pool(name="ln_st", bufs=4) as stp,
            tc.tile_pool(name="sc_ps", bufs=3, space="PSUM") as scp,
            tc.tile_pool(name="acc_ps", bufs=1, space="PSUM") as accp,
        ):
            ident = pp.tile([P, P], F32R, tag="ident")
            nc.sync.dma_start(ident[:], ident_d[:].bitcast(F32R))
            ones1f = pp.tile([1, P], F32, tag="ones1f")
            nc.vector.memset(ones1f[:], 1.0)
            ones1 = pp.tile([1, P], F32R, tag="ones1")
            nc.vector.tensor_copy(ones1[:], ones1f[:])
            c16_bf = pp.tile([1, P], BF16, tag="c16_bf")
            nc.vector.memset(c16_bf[:], 16.0)
            ln16 = pp.tile([P, 1], F32, tag="ln16")
            nc.vector.memset(ln16[:], LN16)
            msrc = pp.tile([P, S // P], F32, tag="msrc")
            nc.sync.dma_start(msrc[:], msrc_d[:])
            masks_sb = pp.tile([P, 16 * 512], BF16, tag="masks")

            def load_w(dram, name, cols=D):
                # [cin, cols] -> sbuf [128, ET, cols] fp8
                t = pp.tile([P, ET * cols], FP8, tag=name)
                nc.sync.dma_start(
                    _r(t[:], "p (e d) -> p e d", e=ET),
                    _r(dram[:], "(e p) d -> p e d", p=P))
                return t

            def load_bias(dram, name, flag, n=D):
                if not flag:
                    return None
                t = pp.tile([1, n], F32R, tag=name)
                nc.sync.dma_start(t[:], dram[:].bitcast(F32R))
                return t

            def pv8(t8):
                return _r(t8[:], "p (e s) -> p e s", e=ET)

            def copy_scaled(dst, src, c, on_act):
                """PSUM->SBUF copy with scale, engine-balanced."""
                with nc.allow_low_precision("fp8/bf16 staging"):
                    if on_act:
                        nc.scalar.activation(dst, src, AFT.Copy, scale=c)
                    else:
                        nc.vector.tensor_scalar_mul(dst, src, c)

            def ln_stats(x_t, sx_act=False):
                """x_t: [128, 512] f32 sbuf -> (scale, bias) per-row [128,1]."""
                sx = stp.tile([P, 1], F32, tag="sx")
                dump = lnp.tile([P, D], F32, tag="ln_xn")
                sq = stp.tile([P, 1], F32, tag="sq")
                nc.scalar.activation(dump[:], x_t[:], AFT.Square,
                                     accum_out=sq[:])
                if sx_act:
                    dump2 = lnp.tile([P, D], F32, tag="ln_xn", name="dump2")
                    nc.scalar.activation(dump2[:], x_t[:], AFT.Identity,
                                         accum_out=sx[:])
                else:
                    nc.vector.reduce_sum(sx[:], x_t[:], axis=AXL.X)
                mu = stp.tile([P, 1], F32, tag="mu")
                nc.vector.tensor_scalar_mul(mu[:], sx[:], 1.0 / D)
                m2 = stp.tile([P, 1], F32, tag="m2")
                nc.vector.tensor_mul(m2[:], mu[:], mu[:])
                v1 = stp.tile([P, 1], F32, tag="v1")
                nc.vector.tensor_scalar(v1[:], m2[:], -float(D), None,
                                        op0=ALU.mult)
                nc.vector.tensor_add(v1[:], v1[:], sq[:])
                std = stp.tile([P, 1], F32, tag="std")
                nc.scalar.activation(std[:], v1[:], AFT.Sqrt,
                                     scale=1.0 / (D - 1))
                nc.vector.tensor_scalar_add(std[:], std[:], EPS)
                s = stp.tile([P, 1], F32, tag="s")
                nc.vector.reciprocal(s[:], std[:])
                nb = stp.tile([P, 1], F32, tag="nb")
                nc.vector.tensor_mul(nb[:], mu[:], s[:])
                nc.vector.tensor_scalar_mul(nb[:], nb[:], -1.0)
                return s, nb

            def transpose4(xn, dstT8, rows, t, on_act):
                """Transpose [128, 512] f32r sbuf tile t into dstT8
                [128, ET*rows] fp8 (x16): 4 e-transposes, 1 scaled copy."""
                if COMBINED_TP:
                    ps = scp.tile([P, 1024], F32R, tag="score",
                                  name="tp")[:, 0:512]
                    for e in range(ET):
                        nc.tensor.matmul(
                            ps[:, e * P:(e + 1) * P],
                            xn[:, e * P:(e + 1) * P], ident[:],
                            start=(e == 0), stop=(e == ET - 1),
                            is_transpose=True, skip_group_check=(e != 0))
                    copy_scaled(
                        _r(dstT8[:], "p (e s) -> p e s", e=ET)[
                            :, :, t * P:(t + 1) * P],
                        _r(ps[:], "p (e c) -> p e c", e=ET), XS, on_act)
                else:
                    for e in range(ET):
                        ps = scp.tile([P, 1024], F32R, tag="score",
                                      name="tp")[:, 0:P]
                        nc.tensor.matmul(
                            ps[:], xn[:, e * P:(e + 1) * P], ident[:],
                            start=True, stop=True, is_transpose=True)
                        copy_scaled(
                            _r(dstT8[:], "p (e s) -> p e s", e=ET)[
                                :, e, t * P:(t + 1) * P],
                            ps[:], XS, on_act)

            def ln_transpose(src_d, rows, dstT8, do_ln=True):
                nt = rows // P
                for t in range(nt):
                    if do_ln:
                        x_t = lnp.tile([P, D], F32, tag="ln_x")
                        nc.sync.dma_start(x_t[:], src_d[t * P:(t + 1) * P, :])
                        s, nb = ln_stats(x_t, sx_act=(t % 3 != 0))
                        xn = lnp.tile([P, D], F32R, tag="ln_xn")
                        nc.vector.tensor_scalar(xn[:], x_t[:], s[:], nb[:],
                                                op0=ALU.mult, op1=ALU.add)
                        transpose4(xn[:], dstT8, rows, t,
                                   on_act=(t % 2 == 0))
                    else:
                        xn = lnp.tile([P, D], F32R, tag="ln_x")
                        nc.sync.dma_start(
                            xn[:], src_d[t * P:(t + 1) * P, :].bitcast(F32R))
                        transpose4(xn[:], dstT8, rows, t, on_act=(t % 2 == 0))

            def ln_transpose_sbuf(xsb, dstT8, tiles=None):
                for t in (tiles if tiles is not None else range(SQ // P)):
                    x_t = xsb[:, t * D:(t + 1) * D]
                    s, nb = ln_stats(x_t)
                    xn = lnp.tile([P, D], F32R, tag="ln_xn")
                    nc.vector.tensor_scalar(xn[:], x_t, s[:], nb[:],
                                            op0=ALU.mult, op1=ALU.add)
                    transpose4(xn[:], dstT8, SQ, t, on_act=(t % 2 == 0))

            def projT(wt8, bt, has_b, srcT8, src_rows, dstT, src_qbs=None):
                """dstT[:, dt, :] = bf16 (W.T @ xn.T)-slice via DoubleRow."""
                if src_qbs is None:
                    src_qbs = list(range(src_rows // 512))
                nqb = len(src_qbs)
                for dt in range(ET):
                    for dqb, qb in enumerate(src_qbs):
                        ps = scp.tile([P, 1024], F32, tag="score",
                                      name="ps")[:, 0:512]
                        for ep in range(EP):
                            nc.tensor.matmul(
                                ps[:],
                                pv8(wt8)[:, 2 * ep:2 * ep + 2,
                                         dt * P:(dt + 1) * P],
                                pv8(srcT8)[:, 2 * ep:2 * ep + 2,
                                           qb * 512:(qb + 1) * 512],
                                start=(ep == 0),
                                stop=(ep == EP - 1 and not has_b),
                                perf_mode=DR)
                            if ep == EP - 1 and has_b:
                                nc.tensor.matmul(
                                    ps[:], bt[:, dt * P:(dt + 1) * P],
                                    ones1[:, 0:512].bitcast(F32R),
                                    start=False, stop=True)
                        copy_scaled(
                            dstT[:, dt * nqb * 512 + dqb * 512:
                                 dt * nqb * 512 + (dqb + 1) * 512],
                            ps[:], DS, on_act=((dt + dqb) % 3 != 0))

            def proj_va(wt8, bt, has_b, srcT8, src_rows, va):
                """V projection, token-major: va [128, nkt*8*66] fp8 = 16*V
                (+ src-mask row scaling), ones column = 16."""
                nkt = src_rows // P
                for kt in range(nkt):
                    ps = scp.tile([P, 1024], F32, tag="score",
                                  name="ps")[:, 0:512]
                    for ep in range(EP):
                        nc.tensor.matmul(
                            ps[:],
                            pv8(srcT8)[:, 2 * ep:2 * ep + 2,
                                       kt * P:(kt + 1) * P],
                            pv8(wt8)[:, 2 * ep:2 * ep + 2, 0:D],
                            start=(ep == 0),
                            stop=(ep == EP - 1 and not has_b),
                            perf_mode=DR)
                        if ep == EP - 1 and has_b:
                            nc.tensor.matmul(
                                ps[:], ones1[:, 0:P], bt[:],
                                start=False, stop=True)
                    dst = _r(va[:], "p (t h c) -> p t h c", t=nkt, h=H)
                    if bias_flags.get("msrc1"):
                        copy_scaled(dst[:, kt, :, 0:DK],
                                    _r(ps[:], "p (h c) -> p h c", h=H),
                                    XS * DS, on_act=(kt % 2 == 0))
                    else:
                        with nc.allow_low_precision("fp8 va"):
                            nc.vector.tensor_scalar(
                                dst[:, kt, :, 0:DK],
                                _r(ps[:], "p (h c) -> p h c", h=H),
                                msrc[:, kt:kt + 1], XS * DS,
                                op0=ALU.mult, op1=ALU.mult)

            def attention(ktT, va, qtT, nkts, masked, exp_dve, wo8, bo,
                          has_bo, resid, x_out, on_pos_done=None):
                """ktT [128, 4*S_k] bf16; va [128, nkt*8*66] fp8 (16*V);
                qtT [128, 4*1024] bf16; masked: fn(pos,kt)-> mask idx|None;
                exp_dve: fn(pos,kt)->bool; x_out [128,8*512] f32 resid+attn."""
                skmax = max(nkts) * P
                atT8 = pp.tile([P, ET * SQ], FP8, tag="attnT")
                taps_live["atT8"] = atT8
                for pos, nkt in enumerate(nkts):
                    units = []
                    kt = 0
                    while kt < nkt:
                        if (USE_PAIRS and masked(pos, kt) is None
                                and kt + 1 < nkt
                                and masked(pos, kt + 1) is None):
                            units.append((kt, kt + 1))
                            kt += 2
                        else:
                            units.append((kt,))
                            kt += 1
                    for hp in range(H // 2):
                        acc = [accp.tile([P, 512], F32, tag=f"acc{i}",
                                         name=f"acc{i}")
                               for i in range(2)]

                        def emit_score(kt):
                            st = scp.tile([P, 1024], F32, tag="score")
                            for i in range(2):
                                nc.tensor.matmul(
                                    st[:, i * 512:(i + 1) * 512],
                                    ktT[i * DK:(i + 1) * DK,
                                        hp * skmax + kt * P:
                                        hp * skmax + (kt + 1) * P],
                                    qtT[i * DK:(i + 1) * DK,
                                        hp * SQ + pos * 512:
                                        hp * SQ + (pos + 1) * 512],
                                    start=True, stop=True)
                            return st

                        def do_exp(pos, kt, st, pair=None):
                            """pair: ([128,2048] int8 tile, j) for fp8 pair
                            halves; None -> bf16 p_t (masked path)."""
                            on_dve = USE_SCH and exp_dve(pos, kt)
                            if pair is not None:
                                t8, j = pair
                                dst = t8[:, j * 1024:(j + 1) * 1024]
                                if on_dve:
                                    with nc.allow_low_precision("sch exp"):
                                        nc.vector.tensor_scalar(
                                            dst, st[:], SCH_A_F8, SCH_B_F8,
                                            op0=ALU.mult, op1=ALU.add)
                                else:
                                    nc.scalar.activation(
                                        dst.bitcast(FP8), st[:], AFT.Exp,
                                        bias=ln16[:], scale=1.0 / 8.0)
                                return None
                            p_t = psb.tile([P, 1024], I16, tag="p")
                            if on_dve:
                                with nc.allow_low_precision("sch exp"):
                                    nc.vector.tensor_scalar(
                                        p_t[:], st[:], SCH_A_BF, SCH_B_BF,
                                        op0=ALU.mult, op1=ALU.add)
                            else:
                                nc.scalar.activation(
                                    p_t[:].bitcast(BF16), st[:], AFT.Exp,
                                    bias=ln16[:], scale=1.0 / 8.0)
                            return p_t

                        flat = [kt for u in units for kt in u]
                        sts = {flat[0]: emit_score(flat[0])}

                        def prefetch(kt):
                            fi = flat.index(kt) + 1
                            if fi < len(flat):
                                sts[flat[fi]] = emit_score(flat[fi])

                        vat = _r(va[:], "p (t h c) -> p t h c",
                                 t=S // P, h=H)
                        for u in units:
                            if len(u) == 2:
                                k0, k1 = u
                                p2 = psb.tile([P, 2048], I8, tag="p2")
                                for j, kt in enumerate(u):
                                    st = sts.pop(kt)
                                    prefetch(kt)
                                    do_exp(pos, kt, st, pair=(p2, j))
                                p8 = p2[:].bitcast(FP8)
                                for i in range(2):
                                    h = 2 * hp + i
                                    nc.tensor.matmul(
                                        acc[i][0:DK + 2, :],
                                        vat[:, k0:k0 + 2, h, 0:66],
                                        _r(p8, "p (j x) -> p j x", j=2)[
                                            :, :, i * 512:(i + 1) * 512],
                                        start=(k0 == 0),
                                        stop=(k1 == nkt - 1),
                                        perf_mode=DR)
                            else:
                                kt = u[0]
                                st = sts.pop(kt)
                                prefetch(kt)
                                p_t = do_exp(pos, kt, st)
                                pb = p_t[:].bitcast(BF16)
                                mi = masked(pos, kt)
                                if mi is not None:
                                    mt = masks_sb[:, mi * 512:(mi + 1) * 512]
                                    for i in range(2):
                                        nc.vector.tensor_mul(
                                            pb[:, i * 512:(i + 1) * 512],
                                            pb[:, i * 512:(i + 1) * 512],
                                            mt)
                                for i in range(2):
                                    h = 2 * hp + i
                                    nc.tensor.matmul(
                                        acc[i][0:DK + 2, :],
                                        vat[:, kt, h, 0:66],
                                        pb[:, i * 512:(i + 1) * 512],
                                        start=(kt == 0), stop=(kt == nkt - 1))
                        # epilogue: atT8 = 16 * acc/denom (fp8)
                        rcl = []
                        for i in range(2):
                            rc = lnp.tile([1, 512], BF16, tag="ln_xn",
                                          name="rc")
                            with nc.allow_low_precision("softmax denom"):
                                nc.vector.reciprocal(
                                    rc[:], acc[i][DK:DK + 1, :])
                            rcl.append(rc)
                        rbl = []
                        for i in range(2):
                            rbs = lnp.tile([DK, 512], BF16, tag="ln_xn",
                                           name="rbs")
                            nc.gpsimd.partition_broadcast(rbs[:], rcl[i][:])
                            rbl.append(rbs)
                        for i in range(2):
                            with nc.allow_low_precision("fp8 attn out"):
                                nc.vector.tensor_tensor(
                                    atT8[i * DK:(i + 1) * DK,
                                         hp * SQ + pos * 512:
                                         hp * SQ + (pos + 1) * 512],
                                    acc[i][0:DK, :], rbl[i][:],
                                    op=ALU.mult)
                    # output projection + residual for this pos block
                    for qt in range(pos * 4, (pos + 1) * 4):
                        res = resid(qt)
                        ps = scp.tile([P, 1024], F32, tag="score",
                                      name="ps")[:, 0:512]
                        for ep in range(EP):
                            nc.tensor.matmul(
                                ps[:],
                                pv8(atT8)[:, 2 * ep:2 * ep + 2,
                                          qt * P:(qt + 1) * P],
                                pv8(wo8)[:, 2 * ep:2 * ep + 2, 0:D],
                                start=(ep == 0),
                                stop=(ep == EP - 1 and not has_bo),
                                perf_mode=DR)
                            if ep == EP - 1 and has_bo:
                                nc.tensor.matmul(
                                    ps[:], ones1[:, 0:P], bo[:],
                                    start=False, stop=True)
                        tmp = lnp.tile([P, D], F32, tag="ln_xn", name="wot")
                        nc.scalar.activation(tmp[:], ps[:], AFT.Copy,
                                             scale=DS)
                        nc.vector.tensor_tensor(
                            x_out[:, qt * D:(qt + 1) * D], tmp[:],
                            res, op=ALU.add)
                    if on_pos_done is not None:
                        on_pos_done(pos)

            # ---------------- stage A: LN0 + transposes ----------------
            kvT8 = pp.tile([P, ET * S], FP8, tag="kvT")
            ln_transpose(xkv_d, S, kvT8, do_ln=True)

            # ---------------- weights (all upfront, Pool DGE queue) -------
            wk8 = load_w(w_sa["wk"], "w_a")
            wq8 = load_w(w_sa["wq"], "w_b")
            wv8 = load_w(w_sa["wv"], "w_c")
            wo8 = load_w(w_sa["wo"], "w_d")
            ck8 = load_w(w_ca["wk"], "w_e")
            cq8 = load_w(w_ca["wq"], "w_f")
            cv8 = load_w(w_ca["wv"], "w_g")
            co8 = load_w(w_ca["wo"], "w_h")
            w18 = [load_w(w1_d[k], f"w1{k}", cols=FF) for k in "ab"]
            w28 = []
            for k in "ab":
                t = pp.tile([P, (FF // P) * D], FP8, tag=f"w2{k}")
                nc.sync.dma_start(
                    _r(t[:], "p (t d) -> p t d", t=FF // P),
                    _r(w2_d[k][:], "(t p) d -> p t d", p=P))
                w28.append(t)
            bk = load_bias(b_sa["k"], "b_a", bias_flags["sa_k"])
            bq = load_bias(b_sa["q"], "b_b", bias_flags["sa_q"])
            bv = load_bias(b_sa["v"], "b_c", bias_flags["sa_v"])
            bo = load_bias(b_sa["o"], "b_d", bias_flags["sa_o"])
            cbk = load_bias(b_ca["k"], "b_e", bias_flags["ca_k"])
            cbq = load_bias(b_ca["q"], "b_f", bias_flags["ca_q"])
            cbv = load_bias(b_ca["v"], "b_g", bias_flags["ca_v"])
            cbo = load_bias(b_ca["o"], "b_h", bias_flags["ca_o"])
            b1_sb = load_bias(b1_d, "b1", bias_flags["ff1"], n=FF)
            b2_sb = load_bias(b2_d, "b2", bias_flags["ff2"])
            nc.sync.dma_start(
                _r(masks_sb[:], "p (t c) -> p t c", t=16),
                _r(masks_d[:], "t p c -> p t c"))

            # ---------------- stage B: SA projections ----------------
            ktT_sa = pp.tile([P, ET * S], BF16, tag="ktT")
            qtT_sa = pp.tile([P, ET * SQ], BF16, tag="qtT")
            va_sa = pp.tile([P, 16 * H * 66], FP8, tag="va")
            nc.vector.memset(
                _r(va_sa[:], "p (t h c) -> p t h c", t=16, h=H)[:, :, :, DK:DK + 1],
                1.0)
            nc.vector.memset(
                _r(va_sa[:], "p (t h c) -> p t h c", t=16, h=H)[:, :, :, DK + 1:],
                0.0)
            if DEBUG_TAPS:
                nc.sync.dma_start(taps["dbg_kvT8"][:], kvT8[:])
            projT(wk8, bk, bias_flags["sa_k"], kvT8, S, ktT_sa)
            projT(wq8, bq, bias_flags["sa_q"], kvT8, S, qtT_sa,
                  src_qbs=list(Q_SRC_QBS))
            proj_va(wv8, bv, bias_flags["sa_v"], kvT8, S, va_sa)

            if DEBUG_TAPS:
                nc.sync.dma_start(taps["dbg_ktT"][:], ktT_sa[:])
                nc.sync.dma_start(taps["dbg_qtT"][:], qtT_sa[:])
                nc.sync.dma_start(taps["dbg_va"][:], va_sa[:])
            # ---------------- stage C/D: SA attention + Wo ----------------
            x1 = pp.tile([P, (SQ // P) * D], F32, tag="x1")

            def sa_masked(pos, kt):
                return kt if (pos == 0 or kt >= 8) else None

            def sa_exp_dve(pos, kt):
                if sa_masked(pos, kt) is None:
                    return kt % 4 == 1
                return (kt % 8) == 2

            def q_src_row(qt):
                pos, j = divmod(qt, 4)
                return Q_SRC_QBS[pos] * 512 + j * P

            def sa_resid(qt):
                rt = lnp.tile([P, D], F32, tag="ln_x", name="sa_resid")
                r0 = q_src_row(qt)
                nc.sync.dma_start(rt[:], xkv_d[r0:r0 + P, :])
                return rt[:]

            # hoisted CA prep: encoder transpose overlaps SA attention
            encT8 = pp.tile([P, ET * S], FP8, tag="kvT")  # reuse kvT slot
            ln_transpose(enc_d, S, encT8, do_ln=False)

            attention(ktT_sa, va_sa, qtT_sa, [NKT0, NKT1], sa_masked,
                      sa_exp_dve, wo8, bo, bias_flags["sa_o"], sa_resid, x1)

            if DEBUG_TAPS:
                nc.sync.dma_start(taps["dbg_x1"][:], x1[:])
                nc.sync.dma_start(taps["dbg_atT8"][:],
                                  taps_live["atT8"][:])
            # ---------------- stage E/F: CA ----------------
            ktT_ca = pp.tile([P, ET * S], BF16, tag="ktT")
            qtT_ca = pp.tile([P, ET * SQ], BF16, tag="qtT")
            va_ca = pp.tile([P, 16 * H * 66], FP8, tag="va")
            nc.vector.memset(
                _r(va_ca[:], "p (t h c) -> p t h c", t=16, h=H)[:, :, :, DK:DK + 1],
                1.0)
            nc.vector.memset(
                _r(va_ca[:], "p (t h c) -> p t h c", t=16, h=H)[:, :, :, DK + 1:],
                0.0)
            projT(ck8, cbk, bias_flags["ca_k"], encT8, S, ktT_ca)
            proj_va(cv8, cbv, bias_flags["ca_v"], encT8, S, va_ca)

            h1T8 = pp.tile([P, ET * SQ], FP8, tag="hT")
            ln_transpose_sbuf(x1, h1T8)
            projT(cq8, cbq, bias_flags["ca_q"], h1T8, SQ, qtT_ca)

            x2 = pp.tile([P, (SQ // P) * D], F32, tag="x2")

            attention(ktT_ca, va_ca, qtT_ca, [16, 16], lambda p, k: None,
                      lambda p, k: (k % 2 == 1) and (k % 16 != 3), co8,
                      cbo,
                      bias_flags["ca_o"],
                      lambda qt: x1[:, qt * D:(qt + 1) * D], x2)

            if DEBUG_TAPS:
                nc.sync.dma_start(taps["dbg_x2"][:], x2[:])
            # ---------------- stage G: LN2 + FFN ----------------
            h2T8 = pp.tile([P, ET * SQ], FP8, tag="hT")
            ln_transpose_sbuf(x2, h2T8)

            ffT8 = pp.tile([P, (FF // P) * SQ], FP8, tag="ffT")

            def ffn2_qt(qt):
                ps = scp.tile([P, 1024], F32, tag="score",
                              name="ps")[:, 0:512]
                nfp = FF // P // 2
                for wi, wt in enumerate(w28):
                    for fp in range(nfp):
                        nc.tensor.matmul(
                            ps[:],
                            _r(ffT8[:], "p (t s) -> p t s", t=FF // P)[
                                :, 2 * fp:2 * fp + 2, qt * P:(qt + 1) * P],
                            _r(wt[:], "p (t d) -> p t d", t=FF // P)[
                                :, 2 * fp:2 * fp + 2, :],
                            start=(wi == 0 and fp == 0),
                            stop=(wi == 1 and fp == nfp - 1
                                  and not bias_flags["ff2"]),
                            perf_mode=DR)
                if bias_flags["ff2"]:
                    nc.tensor.matmul(
                        ps[:], ones1[:, 0:P], b2_sb[:],
                        start=False, stop=True)
                tmp = lnp.tile([P, D], F32, tag="ln_xn", name="ff2t")
                copy_scaled(tmp[:], ps[:], DS, on_act=(qt % 2 == 0))
                o_t = lnp.tile([P, D], F32, tag="ln_xn", name="o_t")
                nc.vector.tensor_tensor(
                    o_t[:], tmp[:], x2[:, qt * D:(qt + 1) * D], op=ALU.add)
                nc.sync.dma_start(out_d[qt * P:(qt + 1) * P, :], o_t[:])

            for qb in range(SQ // 512):
                if qb == 1:
                    for qt in range(4):
                        ffn2_qt(qt)
                for ft in range(FF // P):
                    ps = scp.tile([P, 1024], F32, tag="score",
                                  name="ps")[:, 0:512]
                    for wi, wt in enumerate(w18):
                        for ep in range(EP):
                            last = (wi == 1 and ep == EP - 1)
                            nc.tensor.matmul(
                                ps[:],
                                _r(wt[:], "p (e f) -> p e f", e=ET)[
                                    :, 2 * ep:2 * ep + 2,
                                    ft * P:(ft + 1) * P],
                                pv8(h2T8)[:, 2 * ep:2 * ep + 2,
                                          qb * 512:(qb + 1) * 512],
                                start=(wi == 0 and ep == 0),
                                stop=(last and not bias_flags["ff1"]),
                                perf_mode=DR)
                            if last and bias_flags["ff1"]:
                                nc.tensor.matmul(
                                    ps[:], b1_sb[:, ft * P:(ft + 1) * P],
                                    ones1[:, 0:512].bitcast(F32R),
                                    start=False, stop=True)
                    # ffT8 = 16*relu(z); alternate Act/DVE per ft
                    if ft % 2 == 0:
                        nc.scalar.activation(
                            ffT8[:, ft * SQ + qb * 512:
                                 ft * SQ + (qb + 1) * 512],
                            ps[:], AFT.Relu, scale=XS * DS)
                    else:
                        with nc.allow_low_precision("fp8 relu"):
                            nc.vector.tensor_scalar(
                                ffT8[:, ft * SQ + qb * 512:
                                     ft * SQ + (qb + 1) * 512],
                                ps[:], 0.0, XS * DS,
                                op0=ALU.max, op1=ALU.mult)
            for qt in range(4, SQ // P):
                ffn2_qt(qt)

    nc.finalize()
    return nc


_CACHE = {}
LAST_EXEC_NS = None


def kernel(**inputs):
    x = np.asarray(inputs["x"], np.float32)
    enc = np.asarray(inputs["encoder_output"], np.float32)
    src_mask = np.asarray(inputs["src_mask"]).reshape(S)
    tgt_mask = np.asarray(inputs["tgt_mask"]).reshape(S, S)

    def fold(w, g, b, extra_b):
        w = np.asarray(w, np.float32)
        wf = np.asarray(g, np.float32)[:, None] * w
        bf = np.asarray(b, np.float32) @ w + np.asarray(extra_b, np.float32)
        return wf, bf

    def q8(w):
        return np.asarray(w * WS, np.float32).astype(ml_dtypes.float8_e4m3)

    def q8r(w):
        ws = np.asarray(w * WS, np.float32)
        return (ws - ws.astype(ml_dtypes.float8_e4m3)
                .astype(np.float32)).astype(ml_dtypes.float8_e4m3)

    z = np.zeros(D, np.float32)
    sa_wq, bsa_q = fold(inputs["sa_wq"], inputs["ln0_g"], inputs["ln0_b"], z)
    sa_wk, bsa_k = fold(inputs["sa_wk"], inputs["ln0_g"], inputs["ln0_b"], z)
    sa_wv, bsa_v = fold(inputs["sa_wv"], inputs["ln0_g"], inputs["ln0_b"], z)
    sa_wo = np.asarray(inputs["sa_wo"], np.float32)
    bsa_o = np.asarray(inputs["sa_bo"], np.float32)
    ca_wq, bca_q = fold(inputs["ca_wq"], inputs["ln1_g"], inputs["ln1_b"], z)
    ca_wk = np.asarray(inputs["ca_wk"], np.float32)
    bca_k = np.zeros(D, np.float32)
    ca_wv = np.asarray(inputs["ca_wv"], np.float32)
    bca_v = np.zeros(D, np.float32)
    ca_wo = np.asarray(inputs["ca_wo"], np.float32)
    bca_o = np.asarray(inputs["ca_bo"], np.float32)
    w1, b1 = fold(inputs["ff_w1"], inputs["ln2_g"], inputs["ln2_b"],
                  np.asarray(inputs["ff_b1"], np.float32))
    w2 = np.asarray(inputs["ff_w2"], np.float32)
    b2 = np.asarray(inputs["ff_b2"], np.float32)

    bias_flags = {
        "sa_q": bool(np.any(bsa_q)), "sa_k": bool(np.any(bsa_k)),
        "sa_v": bool(np.any(bsa_v)), "sa_o": bool(np.any(bsa_o)),
        "ca_q": bool(np.any(bca_q)), "ca_k": bool(np.any(bca_k)),
        "ca_v": bool(np.any(bca_v)), "ca_o": bool(np.any(bca_o)),
        "ff1": bool(np.any(b1)), "ff2": bool(np.any(b2)),
        "msrc1": bool(np.all(src_mask == 1)),
    }

    key = tuple(sorted(bias_flags.items()))
    if key not in _CACHE:
        _CACHE[key] = build_program(bias_flags)
    nc = _CACHE[key]

    ident = np.eye(P, dtype=np.float32)
    msrc = src_mask.astype(np.float32).reshape(S // P, P).T.copy()
    BS = WS * XS  # bias pre-scale (descaled by DS in the psum copy)

    shared = {
        "ident": ident, "msrc": msrc,
        "sa_wq": q8(sa_wq), "sa_wk": q8(sa_wk), "sa_wv": q8(sa_wv),
        "sa_wo": q8(sa_wo),
        "ca_wq": q8(ca_wq), "ca_wk": q8(ca_wk), "ca_wv": q8(ca_wv),
        "ca_wo": q8(ca_wo),
        "w1a": q8(w1), "w1b": q8r(w1), "w2a": q8(w2), "w2b": q8r(w2),
        "bsa_q": bsa_q[None] * BS, "bsa_k": bsa_k[None] * BS,
        "bsa_v": bsa_v[None] * BS, "bsa_o": bsa_o[None] * BS,
        "bca_q": bca_q[None] * BS, "bca_k": bca_k[None] * BS,
        "bca_v": bca_v[None] * BS, "bca_o": bca_o[None] * BS,
        "b1": b1[None] * BS, "b2": b2[None] * BS,
    }

    in_maps = []
    for c in range(8):
        b, r = divmod(c, 2)
        perm = PERM_BLOCKS[r]
        rows = np.concatenate(
            [np.arange(gb * 512, (gb + 1) * 512) for gb in perm])
        gb0, gb1 = OWN_BLOCKS[r]
        assert perm[0] == gb0 and perm[2] == gb1
        mk = np.zeros((16, P, 512), np.float32)
        for pos, gb in enumerate((gb0, gb1)):
            qs = slice(gb * 512, (gb + 1) * 512)
            mrow = tgt_mask[qs][:, rows]
            for j in range(8):
                kt = j if pos == 0 else 8 + j
                ks = slice(kt * P, (kt + 1) * P)
                mk[pos * 8 + j] = mrow[:, ks].T
            ext = (NKT0 if pos == 0 else NKT1) * P
            assert not np.any(mrow[:, ext:]), "tgt_mask beyond extent"
        im = dict(shared)
        im["xkv"] = np.ascontiguousarray(x[b][rows])
        im["enc"] = np.ascontiguousarray(enc[b])
        im["masks"] = mk.astype(ml_dtypes.bfloat16)
        in_maps.append(im)

    res = run_bass_kernel_spmd(nc, in_maps, core_ids=list(range(8)))
    global LAST_EXEC_NS
    LAST_EXEC_NS = res.exec_time_ns

    out = np.empty((B, S, D), np.float32)
    for c in range(8):
        b, r = divmod(c, 2)
        gb0, gb1 = OWN_BLOCKS[r]
        o = res.results[c]["out"]
        out[b, gb0 * 512:(gb0 + 1) * 512] = o[0:512]
        out[b, gb1 * 512:(gb1 + 1) * 512] = o[512:1024]
    return out


# revision 106
# speedup vs baseline: 1.0801x; 1.0047x over previous
"""Trainium2 Bass kernel for a pre-LN transformer decoder block.

Shapes (hardcoded): B=4, S_TGT=S_SRC=2048, D=512, H=8, DK=64, FF=2048, fp32.

Sharding: 8 cores; core c handles batch c//2. The two cores of a batch split
the 2048 query rows into two causal-balanced groups of 2x512 rows:
  r0: global q-blocks [0:512) and [1536:2048)
  r1: global q-blocks [512:1024) and [1024:1536)
All cores run one identical SPMD program. Keys (and the x rows feeding K/V)
are PERMUTED per core so that the own q-blocks land at canonical positions:
  pi = [own0 | filler0 | own1 | filler1]   (4 blocks of 512 rows)
With this order both ranks see SA extents of 8 k-tiles (pos0) and 16 (pos1),
diagonal mask tiles align, and Q^T is just columns {block0, block2} of the
transposed/normalized x. Per-core visibility is carried by mask DATA built
on the host. Cross-attention is unmasked full-extent.

Precision/layout strategy:
 - Projections / FFN / Wo run as fp8e4m3 DoubleRow matmuls (2 contraction
   rows per pass over e-tile pairs, 4x PE rate vs f32r). Weights are scaled
   x64 and activations x16 into fp8; every PSUM result is descaled by
   2^-10 in its PSUM->SBUF copy (engine-alternated between DVE and Act).
 - K^T/Q^T are bf16 (scores at full PE rate); P is 16*exp(score/8), stored
   fp8 on unmasked k-tile pairs (DoubleRow PV) and bf16 on masked tiles.
   The ones-column of V (=16) carries the softmax denominator; the x16
   cancels in the division.
 - exp alternates between Act (native Exp) and DVE (Schraudolph bit-trick:
   bits = int(A*score + B) reinterpreted as bf16/e4m3), balancing the
   otherwise Act-bound attention spans.
"""

import numpy as np
import ml_dtypes

import concourse.bass as bass
import concourse.bacc as bacc
import concourse.mybir as mybir
import concourse.tile as tile
from concourse.bass_utils import run_bass_kernel_spmd

F32 = mybir.dt.float32
F32R = mybir.dt.float32r
BF16 = mybir.dt.bfloat16
FP8 = mybir.dt.float8e4
I8 = mybir.dt.int8
I16 = mybir.dt.int16
AFT = mybir.ActivationFunctionType
ALU = mybir.AluOpType
AXL = mybir.AxisListType
DR = mybir.MatmulPerfMode.DoubleRow

B, S, D, H, DK, FF = 4, 2048, 512, 8, 64, 2048
P = 128            # partitions
ET = D // P        # 4 e-tiles of 128 over the model dim
EP = ET // 2       # e-tile pairs for DoubleRow
SQ = 1024          # own query rows per core
NKT0, NKT1 = 8, 16  # uniform k-tile extents for SA pos0 / pos1
EPS = 1e-6

WS = 64.0          # fp8 weight scale
XS = 16.0          # fp8 activation scale
DS = 1.0 / (WS * XS)   # descale after a DoubleRow matmul
LN16 = float(np.log(16.0))
LOG2E = 1.4426950408889634
# Schraudolph exp: bits = trunc(score*A + B); B includes the x16 bias
SCH_A_BF = 128.0 * LOG2E / 8.0
SCH_B_BF = (127.0 + 4.0) * 128.0 - 8.0
SCH_A_F8 = 8.0 * LOG2E / 8.0
SCH_B_F8 = (7.0 + 4.0) * 8.0

OWN_BLOCKS = {0: (0, 3), 1: (1, 2)}
PERM_BLOCKS = {0: (0, 1, 3, 2), 1: (1, 0, 2, 3)}
Q_SRC_QBS = (0, 2)
# combined 4-in-1 transpose PSUM (HW-proven); CoreSim's checker rejects it,
# so debugging scripts can flip this off before build.
COMBINED_TP = True
USE_SCH = True     # DVE Schraudolph exp offload
USE_PAIRS = True   # fp8 DoubleRow PV on unmasked k-tile pairs
DEBUG_TAPS = False  # dump intermediates to DRAM for debugging


def _r(ap, pattern, **kw):
    return ap.rearrange(pattern, **kw)


def build_program(bias_flags):
    """Build the SPMD Bass program. bias_flags: dict of bools saying which
    folded biases are nonzero (uniform across cores)."""
    nc = bacc.Bacc("TRN2", target_bir_lowering=False, debug=False, num_devices=8)

    def din(name, shape, dt=F32):
        return nc.dram_tensor(name, shape, dt, kind="ExternalInput").ap()

    xkv_d = din("xkv", [S, D])
    enc_d = din("enc", [S, D])
    masks_d = din("masks", [16, P, 512], BF16)
    msrc_d = din("msrc", [P, S // P])
    ident_d = din("ident", [P, P])
    # fp8 weights, pre-scaled x64
    w_sa = {k: din(f"sa_{k}", [D, D], FP8) for k in ("wq", "wk", "wv", "wo")}
    w_ca = {k: din(f"ca_{k}", [D, D], FP8) for k in ("wq", "wk", "wv", "wo")}
    w1_d = {k: din(f"w1{k}", [D, FF], FP8) for k in "ab"}
    w2_d = {k: din(f"w2{k}", [FF, D], FP8) for k in "ab"}
    # folded biases (pre-scaled x1024), [1, D] / [1, FF]
    b_sa = {k: din(f"bsa_{k}", [1, D]) for k in ("q", "k", "v", "o")}
    b_ca = {k: din(f"bca_{k}", [1, D]) for k in ("q", "k", "v", "o")}
    b1_d = din("b1", [1, FF])
    b2_d = din("b2", [1, D])
    out_d = nc.dram_tensor("out", [SQ, D], F32, kind="ExternalOutput").ap()
    taps = {}
    taps_live = {}
    if DEBUG_TAPS:
        for nm, shape, dt in [
                ("dbg_kvT8", [P, ET * S], FP8),
                ("dbg_ktT", [P, ET * S], BF16),
                ("dbg_qtT", [P, ET * SQ], BF16),
                ("dbg_va", [P, 16 * H * 66], FP8),
                ("dbg_atT8", [P, ET * SQ], FP8),
                ("dbg_x1", [P, (SQ // P) * D], F32),
                ("dbg_x2", [P, (SQ // P) * D], F32)]:
            taps[nm] = nc.dram_tensor(nm, shape, dt,
                                      kind="ExternalOutput").ap()

    with tile.TileContext(nc) as tc:
        with (
            tc.tile_pool(name="persist", bufs=1) as pp,
            tc.tile_pool(name="ln_sb", bufs=6) as lnp,
            tc.tile_pool(name="p_sb", bufs=5) as psb,
            tc.tile_pool(name="ln_st", bufs=4) as stp,
            tc.tile_pool(name="sc_ps", bufs=3, space="PSUM") as scp,
            tc.tile_pool(name="acc_ps", bufs=1, space="PSUM") as accp,
        ):
            ident = pp.tile([P, P], F32R, tag="ident")
            nc.sync.dma_start(ident[:], ident_d[:].bitcast(F32R))
            ones1f = pp.tile([1, P], F32, tag="ones1f")
            nc.vector.memset(ones1f[:], 1.0)
            ones1 = pp.tile([1, P], F32R, tag="ones1")
            nc.vector.tensor_copy(ones1[:], ones1f[:])
            c16_bf = pp.tile([1, P], BF16, tag="c16_bf")
            nc.vector.memset(c16_bf[:], 16.0)
            ln16 = pp.tile([P, 1], F32, tag="ln16")
            nc.vector.memset(ln16[:], LN16)
            msrc = pp.tile([P, S // P], F32, tag="msrc")
            nc.sync.dma_start(msrc[:], msrc_d[:])
            masks_sb = pp.tile([P, 16 * 512], BF16, tag="masks")

            def load_w(dram, name, cols=D):
                # [cin, cols] -> sbuf [128, ET, cols] fp8
                t = pp.tile([P, ET * cols], FP8, tag=name)
                nc.sync.dma_start(
                    _r(t[:], "p (e d) -> p e d", e=ET),
                    _r(dram[:], "(e p) d -> p e d", p=P))
                return t

            def load_bias(dram, name, flag, n=D):
                if not flag:
                    return None
                t = pp.tile([1, n], F32R, tag=name)
                nc.sync.dma_start(t[:], dram[:].bitcast(F32R))
                return t

            def pv8(t8):
                return _r(t8[:], "p (e s) -> p e s", e=ET)

            def copy_scaled(dst, src, c, on_act):
                """PSUM->SBUF copy with scale, engine-balanced."""
                with nc.allow_low_precision("fp8/bf16 staging"):
                    if on_act:
                        nc.scalar.activation(dst, src, AFT.Copy, scale=c)
                    else:
                        nc.vector.tensor_scalar_mul(dst, src, c)

            def ln_stats(x_t, sx_act=False):
                """x_t: [128, 512] f32 sbuf -> (scale, bias) per-row [128,1]."""
                sx = stp.tile([P, 1], F32, tag="sx")
                dump = lnp.tile([P, D], F32, tag="ln_xn")
                sq = stp.tile([P, 1], F32, tag="sq")
                nc.scalar.activation(dump[:], x_t[:], AFT.Square,
                                     accum_out=sq[:])
                if sx_act:
                    dump2 = lnp.tile([P, D], F32, tag="ln_xn", name="dump2")
                    nc.scalar.activation(dump2[:], x_t[:], AFT.Identity,
                                         accum_out=sx[:])
                else:
                    nc.vector.reduce_sum(sx[:], x_t[:], axis=AXL.X)
                mu = stp.tile([P, 1], F32, tag="mu")
                nc.vector.tensor_scalar_mul(mu[:], sx[:], 1.0 / D)
                m2 = stp.tile([P, 1], F32, tag="m2")
                nc.vector.tensor_mul(m2[:], mu[:], mu[:])
                v1 = stp.tile([P, 1], F32, tag="v1")
                nc.vector.tensor_scalar(v1[:], m2[:], -float(D), None,
                                        op0=ALU.mult)
                nc.vector.tensor_add(v1[:], v1[:], sq[:])
                std = stp.tile([P, 1], F32, tag="std")
                nc.scalar.activation(std[:], v1[:], AFT.Sqrt,
                                     scale=1.0 / (D - 1))
                nc.vector.tensor_scalar_add(std[:], std[:], EPS)
                s = stp.tile([P, 1], F32, tag="s")
                nc.vector.reciprocal(s[:], std[:])
                nb = stp.tile([P, 1], F32, tag="nb")
                nc.vector.tensor_mul(nb[:], mu[:], s[:])
                nc.vector.tensor_scalar_mul(nb[:], nb[:], -1.0)
                return s, nb

            def transpose4(xn, dstT8, rows, t, on_act):
                """Transpose [128, 512] f32r sbuf tile t into dstT8
                [128, ET*rows] fp8 (x16): 4 e-transposes, 1 scaled copy."""
                if COMBINED_TP:
                    ps = scp.tile([P, 1024], F32R, tag="score",
                                  name="tp")[:, 0:512]
                    for e in range(ET):
                        nc.tensor.matmul(
                            ps[:, e * P:(e + 1) * P],
                            xn[:, e * P:(e + 1) * P], ident[:],
                            start=(e == 0), stop=(e == ET - 1),
                            is_transpose=True, skip_group_check=(e != 0))
                    copy_scaled(
                        _r(dstT8[:], "p (e s) -> p e s", e=ET)[
                            :, :, t * P:(t + 1) * P],
                        _r(ps[:], "p (e c) -> p e c", e=ET), XS, on_act)
                else:
                    for e in range(ET):
                        ps = scp.tile([P, 1024], F32R, tag="score",
                                      name="tp")[:, 0:P]
                        nc.tensor.matmul(
                            ps[:], xn[:, e * P:(e + 1) * P], ident[:],
                            start=True, stop=True, is_transpose=True)
                        copy_scaled(
                            _r(dstT8[:], "p (e s) -> p e s", e=ET)[
                                :, e, t * P:(t + 1) * P],
                            ps[:], XS, on_act)

            def ln_transpose(src_d, rows, dstT8, do_ln=True):
                nt = rows // P
                for t in range(nt):
                    if do_ln:
                        x_t = lnp.tile([P, D], F32, tag="ln_x")
                        nc.sync.dma_start(x_t[:], src_d[t * P:(t + 1) * P, :])
                        s, nb = ln_stats(x_t, sx_act=(t % 3 != 0))
                        xn = lnp.tile([P, D], F32R, tag="ln_xn")
                        nc.vector.tensor_scalar(xn[:], x_t[:], s[:], nb[:],
                                                op0=ALU.mult, op1=ALU.add)
                        transpose4(xn[:], dstT8, rows, t,
                                   on_act=(t % 2 == 0))
                    else:
                        xn = lnp.tile([P, D], F32R, tag="ln_x")
                        nc.sync.dma_start(
                            xn[:], src_d[t * P:(t + 1) * P, :].bitcast(F32R))
                        transpose4(xn[:], dstT8, rows, t, on_act=(t % 2 == 0))

            def ln_transpose_sbuf(xsb, dstT8, tiles=None):
                for t in (tiles if tiles is not None else range(SQ // P)):
                    x_t = xsb[:, t * D:(t + 1) * D]
                    s, nb = ln_stats(x_t)
                    xn = lnp.tile([P, D], F32R, tag="ln_xn")
                    nc.vector.tensor_scalar(xn[:], x_t, s[:], nb[:],
                                            op0=ALU.mult, op1=ALU.add)
                    transpose4(xn[:], dstT8, SQ, t, on_act=(t % 2 == 0))

            def projT(wt8, bt, has_b, srcT8, src_rows, dstT, src_qbs=None):
                """dstT[:, dt, :] = bf16 (W.T @ xn.T)-slice via DoubleRow."""
                if src_qbs is None:
                    src_qbs = list(range(src_rows // 512))
                nqb = len(src_qbs)
                for dt in range(ET):
                    for dqb, qb in enumerate(src_qbs):
                        ps = scp.tile([P, 1024], F32, tag="score",
                                      name="ps")[:, 0:512]
                        for ep in range(EP):
                            nc.tensor.matmul(
                                ps[:],
                                pv8(wt8)[:, 2 * ep:2 * ep + 2,
                                         dt * P:(dt + 1) * P],
                                pv8(srcT8)[:, 2 * ep:2 * ep + 2,
                                           qb * 512:(qb + 1) * 512],
                                start=(ep == 0),
                                stop=(ep == EP - 1 and not has_b),
                                perf_mode=DR)
                            if ep == EP - 1 and has_b:
                                nc.tensor.matmul(
                                    ps[:], bt[:, dt * P:(dt + 1) * P],
                                    ones1[:, 0:512].bitcast(F32R),
                                    start=False, stop=True)
                        copy_scaled(
                            dstT[:, dt * nqb * 512 + dqb * 512:
                                 dt * nqb * 512 + (dqb + 1) * 512],
                            ps[:], DS, on_act=((dt + dqb) % 3 != 0))

            def proj_va(wt8, bt, has_b, srcT8, src_rows, va):
                """V projection, token-major: va [128, nkt*8*66] fp8 = 16*V
                (+ src-mask row scaling), ones column = 16."""
                nkt = src_rows // P
                for kt in range(nkt):
                    ps = scp.tile([P, 1024], F32, tag="score",
                                  name="ps")[:, 0:512]
                    for ep in range(EP):
                        nc.tensor.matmul(
                            ps[:],
                            pv8(srcT8)[:, 2 * ep:2 * ep + 2,
                                       kt * P:(kt + 1) * P],
                            pv8(wt8)[:, 2 * ep:2 * ep + 2, 0:D],
                            start=(ep == 0),
                            stop=(ep == EP - 1 and not has_b),
                            perf_mode=DR)
                        if ep == EP - 1 and has_b:
                            nc.tensor.matmul(
                                ps[:], ones1[:, 0:P], bt[:],
                                start=False, stop=True)
                    dst = _r(va[:], "p (t h c) -> p t h c", t=nkt, h=H)
                    if bias_flags.get("msrc1"):
                        copy_scaled(dst[:, kt, :, 0:DK],
                                    _r(ps[:], "p (h c) -> p h c", h=H),
                                    XS * DS, on_act=(kt % 2 == 0))
                    else:
                        with nc.allow_low_precision("fp8 va"):
                            nc.vector.tensor_scalar(
                                dst[:, kt, :, 0:DK],
                                _r(ps[:], "p (h c) -> p h c", h=H),
                                msrc[:, kt:kt + 1], XS * DS,
                                op0=ALU.mult, op1=ALU.mult)

            def attention(ktT, va, qtT, nkts, masked, exp_dve, wo8, bo,
                          has_bo, resid, x_out, on_pos_done=None):
                """ktT [128, 4*S_k] bf16; va [128, nkt*8*66] fp8 (16*V);
                qtT [128, 4*1024] bf16; masked: fn(pos,kt)-> mask idx|None;
                exp_dve: fn(pos,kt)->bool; x_out [128,8*512] f32 resid+attn."""
                skmax = max(nkts) * P
                atT8 = pp.tile([P, ET * SQ], FP8, tag="attnT")
                taps_live["atT8"] = atT8
                for pos, nkt in enumerate(nkts):
                    units = []
                    kt = 0
                    while kt < nkt:
                        if (USE_PAIRS and masked(pos, kt) is None
                                and kt + 1 < nkt
                                and masked(pos, kt + 1) is None):
                            units.append((kt, kt + 1))
                            kt += 2
                        else:
                            units.append((kt,))
                            kt += 1
                    for hp in range(H // 2):
                        acc = [accp.tile([P, 512], F32, tag=f"acc{i}",
                                         name=f"acc{i}")
                               for i in range(2)]

                        def emit_score(kt):
                            st = scp.tile([P, 1024], F32, tag="score")
                            for i in range(2):
                                nc.tensor.matmul(
                                    st[:, i * 512:(i + 1) * 512],
                                    ktT[i * DK:(i + 1) * DK,
                                        hp * skmax + kt * P:
                                        hp * skmax + (kt + 1) * P],
                                    qtT[i * DK:(i + 1) * DK,
                                        hp * SQ + pos * 512:
                                        hp * SQ + (pos + 1) * 512],
                                    start=True, stop=True)
                            return st

                        def do_exp(pos, kt, st, pair=None):
                            """pair: ([128,2048] int8 tile, j) for fp8 pair
                            halves; None -> bf16 p_t (masked path)."""
                            on_dve = USE_SCH and exp_dve(pos, kt)
                            if pair is not None:
                                t8, j = pair
                                dst = t8[:, j * 1024:(j + 1) * 1024]
                                if on_dve:
                                    with nc.allow_low_precision("sch exp"):
                                        nc.vector.tensor_scalar(
                                            dst, st[:], SCH_A_F8, SCH_B_F8,
                                            op0=ALU.mult, op1=ALU.add)
                                else:
                                    nc.scalar.activation(
                                        dst.bitcast(FP8), st[:], AFT.Exp,
                                        bias=ln16[:], scale=1.0 / 8.0)
                                return None
                            p_t = psb.tile([P, 1024], I16, tag="p")
                            if on_dve:
                                with nc.allow_low_precision("sch exp"):
                                    nc.vector.tensor_scalar(
                                        p_t[:], st[:], SCH_A_BF, SCH_B_BF,
                                        op0=ALU.mult, op1=ALU.add)
                            else:
                                nc.scalar.activation(
                                    p_t[:].bitcast(BF16), st[:], AFT.Exp,
                                    bias=ln16[:], scale=1.0 / 8.0)
                            return p_t

                        flat = [kt for u in units for kt in u]
                        sts = {flat[0]: emit_score(flat[0])}

                        def prefetch(kt):
                            fi = flat.index(kt) + 1
                            if fi < len(flat):
                                sts[flat[fi]] = emit_score(flat[fi])

                        vat = _r(va[:], "p (t h c) -> p t h c",
                                 t=S // P, h=H)
                        for u in units:
                            if len(u) == 2:
                                k0, k1 = u
                                p2 = psb.tile([P, 2048], I8, tag="p2")
                                for j, kt in enumerate(u):
                                    st = sts.pop(kt)
                                    prefetch(kt)
                                    do_exp(pos, kt, st, pair=(p2, j))
                                p8 = p2[:].bitcast(FP8)
                                for i in range(2):
                                    h = 2 * hp + i
                                    nc.tensor.matmul(
                                        acc[i][0:DK + 2, :],
                                        vat[:, k0:k0 + 2, h, 0:66],
                                        _r(p8, "p (j x) -> p j x", j=2)[
                                            :, :, i * 512:(i + 1) * 512],
                                        start=(k0 == 0),
                                        stop=(k1 == nkt - 1),
                                        perf_mode=DR)
                            else:
                                kt = u[0]
                                st = sts.pop(kt)
                                prefetch(kt)
                                p_t = do_exp(pos, kt, st)
                                pb = p_t[:].bitcast(BF16)
                                mi = masked(pos, kt)
                                if mi is not None:
                                    mt = masks_sb[:, mi * 512:(mi + 1) * 512]
                                    for i in range(2):
                                        nc.vector.tensor_mul(
                                            pb[:, i * 512:(i + 1) * 512],
                                            pb[:, i * 512:(i + 1) * 512],
                                            mt)
                                for i in range(2):
                                    h = 2 * hp + i
                                    nc.tensor.matmul(
                                        acc[i][0:DK + 2, :],
                                        vat[:, kt, h, 0:66],
                                        pb[:, i * 512:(i + 1) * 512],
                                        start=(kt == 0), stop=(kt == nkt - 1))
                        # epilogue: atT8 = 16 * acc/denom (fp8)
                        rcl = []
                        for i in range(2):
                            rc = lnp.tile([1, 512], BF16, tag="ln_xn",
                                          name="rc")
                            with nc.allow_low_precision("softmax denom"):
                                nc.vector.reciprocal(
                                    rc[:], acc[i][DK:DK + 1, :])
                            rcl.append(rc)
                        rbl = []
                        for i in range(2):
                            rbs = lnp.tile([DK, 512], BF16, tag="ln_xn",
                                           name="rbs")
                            nc.gpsimd.partition_broadcast(rbs[:], rcl[i][:])
                            rbl.append(rbs)
                        for i in range(2):
                            with nc.allow_low_precision("fp8 attn out"):
                                nc.vector.tensor_tensor(
                                    atT8[i * DK:(i + 1) * DK,
                                         hp * SQ + pos * 512:
                                         hp * SQ + (pos + 1) * 512],
                                    acc[i][0:DK, :], rbl[i][:],
                                    op=ALU.mult)
                    # output projection + residual for this pos block
                    for qt in range(pos * 4, (pos + 1) * 4):
                        res = resid(qt)
                        ps = scp.tile([P, 1024], F32, tag="score",
                                      name="ps")[:, 0:512]
                        for ep in range(EP):
                            nc.tensor.matmul(
                                ps[:],
                                pv8(atT8)[:, 2 * ep:2 * ep + 2,
                                          qt * P:(qt + 1) * P],
                                pv8(wo8)[:, 2 * ep:2 * ep + 2, 0:D],
                                start=(ep == 0),
                                stop=(ep == EP - 1 and not has_bo),
                                perf_mode=DR)
                            if ep == EP - 1 and has_bo:
                                nc.tensor.matmul(
                                    ps[:], ones1[:, 0:P], bo[:],
                                    start=False, stop=True)
                        tmp = lnp.tile([P, D], F32, tag="ln_xn", name="wot")
                        nc.scalar.activation(tmp[:], ps[:], AFT.Copy,
                                             scale=DS)
                        nc.vector.tensor_tensor(
                            x_out[:, qt * D:(qt + 1) * D], tmp[:],
                            res, op=ALU.add)
                    if on_pos_done is not None:
                        on_pos_done(pos)

            # ---------------- stage A: LN0 + transposes ----------------
            kvT8 = pp.tile([P, ET * S], FP8, tag="kvT")
            ln_transpose(xkv_d, S, kvT8, do_ln=True)

            # ---------------- weights (all upfront, Pool DGE queue) -------
            wk8 = load_w(w_sa["wk"], "w_a")
            wq8 = load_w(w_sa["wq"], "w_b")
            wv8 = load_w(w_sa["wv"], "w_c")
            wo8 = load_w(w_sa["wo"], "w_d")
            ck8 = load_w(w_ca["wk"], "w_e")
            cq8 = load_w(w_ca["wq"], "w_f")
            cv8 = load_w(w_ca["wv"], "w_g")
            co8 = load_w(w_ca["wo"], "w_h")
            w18 = [load_w(w1_d[k], f"w1{k}", cols=FF) for k in "ab"]
            w28 = []
            for k in "ab":
                t = pp.tile([P, (FF // P) * D], FP8, tag=f"w2{k}")
                nc.sync.dma_start(
                    _r(t[:], "p (t d) -> p t d", t=FF // P),
                    _r(w2_d[k][:], "(t p) d -> p t d", p=P))
                w28.append(t)
            bk = load_bias(b_sa["k"], "b_a", bias_flags["sa_k"])
            bq = load_bias(b_sa["q"], "b_b", bias_flags["sa_q"])
            bv = load_bias(b_sa["v"], "b_c", bias_flags["sa_v"])
            bo = load_bias(b_sa["o"], "b_d", bias_flags["sa_o"])
            cbk = load_bias(b_ca["k"], "b_e", bias_flags["ca_k"])
            cbq = load_bias(b_ca["q"], "b_f", bias_flags["ca_q"])
            cbv = load_bias(b_ca["v"], "b_g", bias_flags["ca_v"])
            cbo = load_bias(b_ca["o"], "b_h", bias_flags["ca_o"])
            b1_sb = load_bias(b1_d, "b1", bias_flags["ff1"], n=FF)
            b2_sb = load_bias(b2_d, "b2", bias_flags["ff2"])
            nc.sync.dma_start(
                _r(masks_sb[:], "p (t c) -> p t c", t=16),
                _r(masks_d[:], "t p c -> p t c"))

            # ---------------- stage B: SA projections ----------------
            ktT_sa = pp.tile([P, ET * S], BF16, tag="ktT")
            qtT_sa = pp.tile([P, ET * SQ], BF16, tag="qtT")
            va_sa = pp.tile([P, 16 * H * 66], FP8, tag="va")
            nc.vector.memset(
                _r(va_sa[:], "p (t h c) -> p t h c", t=16, h=H)[:, :, :, DK:DK + 1],
                1.0)
            nc.vector.memset(
                _r(va_sa[:], "p (t h c) -> p t h c", t=16, h=H)[:, :, :, DK + 1:],
                0.0)
            if DEBUG_TAPS:
                nc.sync.dma_start(taps["dbg_kvT8"][:], kvT8[:])
            projT(wk8, bk, bias_flags["sa_k"], kvT8, S, ktT_sa)
            projT(wq8, bq, bias_flags["sa_q"], kvT8, S, qtT_sa,
                  src_qbs=list(Q_SRC_QBS))
            proj_va(wv8, bv, bias_flags["sa_v"], kvT8, S, va_sa)

            if DEBUG_TAPS:
                nc.sync.dma_start(taps["dbg_ktT"][:], ktT_sa[:])
                nc.sync.dma_start(taps["dbg_qtT"][:], qtT_sa[:])
                nc.sync.dma_start(taps["dbg_va"][:], va_sa[:])
            # ---------------- stage C/D: SA attention + Wo ----------------
            x1 = pp.tile([P, (SQ // P) * D], F32, tag="x1")

            def sa_masked(pos, kt):
                return kt if (pos == 0 or kt >= 8) else None

            def sa_exp_dve(pos, kt):
                if sa_masked(pos, kt) is None:
                    return kt % 4 == 1
                return (kt % 8) == 5

            def q_src_row(qt):
                pos, j = divmod(qt, 4)
                return Q_SRC_QBS[pos] * 512 + j * P

            def sa_resid(qt):
                rt = lnp.tile([P, D], F32, tag="ln_x", name="sa_resid")
                r0 = q_src_row(qt)
                nc.sync.dma_start(rt[:], xkv_d[r0:r0 + P, :])
                return rt[:]

            # hoisted CA prep: encoder transpose overlaps SA attention
            encT8 = pp.tile([P, ET * S], FP8, tag="kvT")  # reuse kvT slot
            ln_transpose(enc_d, S, encT8, do_ln=False)

            attention(ktT_sa, va_sa, qtT_sa, [NKT0, NKT1], sa_masked,
                      sa_exp_dve, wo8, bo, bias_flags["sa_o"], sa_resid, x1)

            if DEBUG_TAPS:
                nc.sync.dma_start(taps["dbg_x1"][:], x1[:])
                nc.sync.dma_start(taps["dbg_atT8"][:],
                                  taps_live["atT8"][:])
            # ---------------- stage E/F: CA ----------------
            ktT_ca = pp.tile([P, ET * S], BF16, tag="ktT")
            qtT_ca = pp.tile([P, ET * SQ], BF16, tag="qtT")
            va_ca = pp.tile([P, 16 * H * 66], FP8, tag="va")
            nc.vector.memset(
                _r(va_ca[:], "p (t h c) -> p t h c", t=16, h=H)[:, :, :, DK:DK + 1],
                1.0)
            nc.vector.memset(
                _r(va_ca[:], "p (t h c) -> p t h c", t=16, h=H)[:, :, :, DK + 1:],
                0.0)
            projT(ck8, cbk, bias_flags["ca_k"], encT8, S, ktT_ca)
            proj_va(cv8, cbv, bias_flags["ca_v"], encT8, S, va_ca)

            h1T8 = pp.tile([P, ET * SQ], FP8, tag="hT")
            ln_transpose_sbuf(x1, h1T8)
            projT(cq8, cbq, bias_flags["ca_q"], h1T8, SQ, qtT_ca)

            x2 = pp.tile([P, (SQ // P) * D], F32, tag="x2")

            attention(ktT_ca, va_ca, qtT_ca, [16, 16], lambda p, k: None,
                      lambda p, k: (k % 2 == 1) and (k % 16 != 3), co8,
                      cbo,
                      bias_flags["ca_o"],
                      lambda qt: x1[:, qt * D:(qt + 1) * D], x2)

            if DEBUG_TAPS:
                nc.sync.dma_start(taps["dbg_x2"][:], x2[:])
            # ---------------- stage G: LN2 + FFN ----------------
            h2T8 = pp.tile([P, ET * SQ], FP8, tag="hT")
            ln_transpose_sbuf(x2, h2T8)

            ffT8 = pp.tile([P, (FF // P) * SQ], FP8, tag="ffT")

            def ffn2_qt(qt):
                ps = scp.tile([P, 1024], F32, tag="score",
                              name="ps")[:, 0:512]
                nfp = FF // P // 2
                for wi, wt in enumerate(w28):
                    for fp in range(nfp):
                        nc.tensor.matmul(
                            ps[:],
                            _r(ffT8[:], "p (t s) -> p t s", t=FF // P)[
                                :, 2 * fp:2 * fp + 2, qt * P:(qt + 1) * P],
                            _r(wt[:], "p (t d) -> p t d", t=FF // P)[
                                :, 2 * fp:2 * fp + 2, :],
                            start=(wi == 0 and fp == 0),
                            stop=(wi == 1 and fp == nfp - 1
                                  and not bias_flags["ff2"]),
                            perf_mode=DR)
                if bias_flags["ff2"]:
                    nc.tensor.matmul(
                        ps[:], ones1[:, 0:P], b2_sb[:],
                        start=False, stop=True)
                tmp = lnp.tile([P, D], F32, tag="ln_xn", name="ff2t")
                copy_scaled(tmp[:], ps[:], DS, on_act=(qt % 2 == 0))
                o_t = lnp.tile([P, D], F32, tag="ln_xn", name="o_t")
                nc.vector.tensor_tensor(
                    o_t[:], tmp[:], x2[:, qt * D:(qt + 1) * D], op=ALU.add)
                nc.sync.dma_start(out_d[qt * P:(qt + 1) * P, :], o_t[:])

            for qb in range(SQ // 512):
                if qb == 1:
                    for qt in range(4):
                        ffn2_qt(qt)
                for ft in range(FF // P):
                    ps = scp.tile([P, 1024], F32, tag="score",
                                  name="ps")[:, 0:512]
                    for wi, wt in enumerate(w18):
                        for ep in range(EP):
                            last = (wi == 1 and ep == EP - 1)
                            nc.tensor.matmul(
                                ps[:],
                                _r(wt[:], "p (e f) -> p e f", e=ET)[
                                    :, 2 * ep:2 * ep + 2,
                                    ft * P:(ft + 1) * P],
                                pv8(h2T8)[:, 2 * ep:2 * ep + 2,
                                          qb * 512:(qb + 1) * 512],
                                start=(wi == 0 and ep == 0),
                                stop=(last and not bias_flags["ff1"]),
                                perf_mode=DR)
                            if last and bias_flags["ff1"]:
                                nc.tensor.matmul(
                                    ps[:], b1_sb[:, ft * P:(ft + 1) * P],
                                    ones1[:, 0:512].bitcast(F32R),
                                    start=False, stop=True)
                    # ffT8 = 16*relu(z); alternate Act/DVE per ft
                    if ft % 2 == 0:
                        nc.scalar.activation(
                            ffT8[:, ft * SQ + qb * 512:
                                 ft * SQ + (qb + 1) * 512],
                            ps[:], AFT.Relu, scale=XS * DS)
                    else:
                        with nc.allow_low_precision("fp8 relu"):
                            nc.vector.tensor_scalar(
                                ffT8[:, ft * SQ + qb * 512:
                                     ft * SQ + (qb + 1) * 512],
                                ps[:], 0.0, XS * DS,
                                op0=ALU.max, op1=ALU.mult)
            for qt in range(4, SQ // P):
                ffn2_qt(qt)

    nc.finalize()
    return nc


_CACHE = {}
LAST_EXEC_NS = None


def kernel(**inputs):
    x = np.asarray(inputs["x"], np.float32)
    enc = np.asarray(inputs["encoder_output"], np.float32)
    src_mask = np.asarray(inputs["src_mask"]).reshape(S)
    tgt_mask = np.asarray(inputs["tgt_mask"]).reshape(S, S)

    def fold(w, g, b, extra_b):
        w = np.asarray(w, np.float32)
        wf = np.asarray(g, np.float32)[:, None] * w
        bf = np.asarray(b, np.float32) @ w + np.asarray(extra_b, np.float32)
        return wf, bf

    def q8(w):
        return np.asarray(w * WS, np.float32).astype(ml_dtypes.float8_e4m3)

    def q8r(w):
        ws = np.asarray(w * WS, np.float32)
        return (ws - ws.astype(ml_dtypes.float8_e4m3)
                .astype(np.float32)).astype(ml_dtypes.float8_e4m3)

    z = np.zeros(D, np.float32)
    sa_wq, bsa_q = fold(inputs["sa_wq"], inputs["ln0_g"], inputs["ln0_b"], z)
    sa_wk, bsa_k = fold(inputs["sa_wk"], inputs["ln0_g"], inputs["ln0_b"], z)
    sa_wv, bsa_v = fold(inputs["sa_wv"], inputs["ln0_g"], inputs["ln0_b"], z)
    sa_wo = np.asarray(inputs["sa_wo"], np.float32)
    bsa_o = np.asarray(inputs["sa_bo"], np.float32)
    ca_wq, bca_q = fold(inputs["ca_wq"], inputs["ln1_g"], inputs["ln1_b"], z)
    ca_wk = np.asarray(inputs["ca_wk"], np.float32)
    bca_k = np.zeros(D, np.float32)
    ca_wv = np.asarray(inputs["ca_wv"], np.float32)
    bca_v = np.zeros(D, np.float32)
    ca_wo = np.asarray(inputs["ca_wo"], np.float32)
    bca_o = np.asarray(inputs["ca_bo"], np.float32)
    w1, b1 = fold(inputs["ff_w1"], inputs["ln2_g"], inputs["ln2_b"],
                  np.asarray(inputs["ff_b1"], np.float32))
    w2 = np.asarray(inputs["ff_w2"], np.float32)
    b2 = np.asarray(inputs["ff_b2"], np.float32)

    bias_flags = {
        "sa_q": bool(np.any(bsa_q)), "sa_k": bool(np.any(bsa_k)),
        "sa_v": bool(np.any(bsa_v)), "sa_o": bool(np.any(bsa_o)),
        "ca_q": bool(np.any(bca_q)), "ca_k": bool(np.any(bca_k)),
        "ca_v": bool(np.any(bca_v)), "ca_o": bool(np.any(bca_o)),
        "ff1": bool(np.any(b1)), "ff2": bool(np.any(b2)),
        "msrc1": bool(np.all(src_mask == 1)),
    }

    key = tuple(sorted(bias_flags.items()))
    if key not in _CACHE:
        _CACHE[key] = build_program(bias_flags)
    nc = _CACHE[key]

    ident = np.eye(P, dtype=np.float32)
    msrc = src_mask.astype(np.float32).reshape(S // P, P).T.copy()
    BS = WS * XS  # bias pre-scale (descaled by DS in the psum copy)

    shared = {
        "ident": ident, "msrc": msrc,
        "sa_wq": q8(sa_wq), "sa_wk": q8(sa_wk), "sa_wv": q8(sa_wv),
        "sa_wo": q8(sa_wo),
        "ca_wq": q8(ca_wq), "ca_wk": q8(ca_wk), "ca_wv": q8(ca_wv),
        "ca_wo": q8(ca_wo),
        "w1a": q8(w1), "w1b": q8r(w1), "w2a": q8(w2), "w2b": q8r(w2),
        "bsa_q": bsa_q[None] * BS, "bsa_k": bsa_k[None] * BS,
        "bsa_v": bsa_v[None] * BS, "bsa_o": bsa_o[None] * BS,
        "bca_q": bca_q[None] * BS, "bca_k": bca_k[None] * BS,
        "bca_v": bca_v[None] * BS, "bca_o": bca_o[None] * BS,
        "b1": b1[None] * BS, "b2": b2[None] * BS,
    }

    in_maps = []
    for c in range(8):
        b, r = divmod(c, 2)
        perm = PERM_BLOCKS[r]
        rows = np.concatenate(
            [np.arange(gb * 512, (gb + 1) * 512) for gb in perm])
        gb0, gb1 = OWN_BLOCKS[r]
        assert perm[0] == gb0 and perm[2] == gb1
        mk = np.zeros((16, P, 512), np.float32)
        for pos, gb in enumerate((gb0, gb1)):
            qs = slice(gb * 512, (gb + 1) * 512)
            mrow = tgt_mask[qs][:, rows]
            for j in range(8):
                kt = j if pos == 0 else 8 + j
                ks = slice(kt * P, (kt + 1) * P)
                mk[pos * 8 + j] = mrow[:, ks].T
            ext = (NKT0 if pos == 0 else NKT1) * P
            assert not np.any(mrow[:, ext:]), "tgt_mask beyond extent"
        im = dict(shared)
        im["xkv"] = np.ascontiguousarray(x[b][rows])
        im["enc"] = np.ascontiguousarray(enc[b])
        im["masks"] = mk.astype(ml_dtypes.bfloat16)
        in_maps.append(im)

    res = run_bass_kernel_spmd(nc, in_maps, core_ids=list(range(8)))
    global LAST_EXEC_NS
    LAST_EXEC_NS = res.exec_time_ns

    out = np.empty((B, S, D), np.float32)
    for c in range(8):
        b, r = divmod(c, 2)
        gb0, gb1 = OWN_BLOCKS[r]
        o = res.results[c]["out"]
        out[b, gb0 * 512:(gb0 + 1) * 512] = o[0:512]
        out[b, gb1 * 512:(gb1 + 1) * 512] = o[512:1024]
    return out


# revision 112
# speedup vs baseline: 1.0840x; 1.0037x over previous
"""Trainium2 Bass kernel for a pre-LN transformer decoder block.

Shapes (hardcoded): B=4, S_TGT=S_SRC=2048, D=512, H=8, DK=64, FF=2048, fp32.

Sharding: 8 cores; core c handles batch c//2. The two cores of a batch split
the 2048 query rows into two causal-balanced groups of 2x512 rows:
  r0: global q-blocks [0:512) and [1536:2048)
  r1: global q-blocks [512:1024) and [1024:1536)
All cores run one identical SPMD program. Keys (and the x rows feeding K/V)
are PERMUTED per core so that the own q-blocks land at canonical positions:
  pi = [own0 | filler0 | own1 | filler1]   (4 blocks of 512 rows)
With this order both ranks see SA extents of 8 k-tiles (pos0) and 16 (pos1),
diagonal mask tiles align, and Q^T is just columns {block0, block2} of the
transposed/normalized x. Per-core visibility is carried by mask DATA built
on the host. Cross-attention is unmasked full-extent.

Precision/layout strategy:
 - Projections / FFN / Wo run as fp8e4m3 DoubleRow matmuls (2 contraction
   rows per pass over e-tile pairs, 4x PE rate vs f32r). Weights are scaled
   x64 and activations x16 into fp8; every PSUM result is descaled by
   2^-10 in its PSUM->SBUF copy (engine-alternated between DVE and Act).
 - K^T/Q^T are bf16 (scores at full PE rate); P is 16*exp(score/8), stored
   fp8 on unmasked k-tile pairs (DoubleRow PV) and bf16 on masked tiles.
   The ones-column of V (=16) carries the softmax denominator; the x16
   cancels in the division.
 - exp alternates between Act (native Exp) and DVE (Schraudolph bit-trick:
   bits = int(A*score + B) reinterpreted as bf16/e4m3), balancing the
   otherwise Act-bound attention spans.
"""

import numpy as np
import ml_dtypes

import concourse.bass as bass
import concourse.bacc as bacc
import concourse.mybir as mybir
import concourse.tile as tile
from concourse.bass_utils import run_bass_kernel_spmd

F32 = mybir.dt.float32
F32R = mybir.dt.float32r
BF16 = mybir.dt.bfloat16
FP8 = mybir.dt.float8e4
I8 = mybir.dt.int8
I16 = mybir.dt.int16
AFT = mybir.ActivationFunctionType
ALU = mybir.AluOpType
AXL = mybir.AxisListType
DR = mybir.MatmulPerfMode.DoubleRow

B, S, D, H, DK, FF = 4, 2048, 512, 8, 64, 2048
P = 128            # partitions
ET = D // P        # 4 e-tiles of 128 over the model dim
EP = ET // 2       # e-tile pairs for DoubleRow
SQ = 1024          # own query rows per core
NKT0, NKT1 = 8, 16  # uniform k-tile extents for SA pos0 / pos1
EPS = 1e-6

WS = 64.0          # fp8 weight scale
XS = 16.0          # fp8 activation scale
DS = 1.0 / (WS * XS)   # descale after a DoubleRow matmul
LN16 = float(np.log(16.0))
LOG2E = 1.4426950408889634
# Schraudolph exp: bits = trunc(score*A + B); B includes the x16 bias
SCH_A_BF = 128.0 * LOG2E / 8.0
SCH_B_BF = (127.0 + 4.0) * 128.0 - 8.0
SCH_A_F8 = 8.0 * LOG2E / 8.0
SCH_B_F8 = (7.0 + 4.0) * 8.0

OWN_BLOCKS = {0: (0, 3), 1: (1, 2)}
PERM_BLOCKS = {0: (0, 1, 3, 2), 1: (1, 0, 2, 3)}
Q_SRC_QBS = (0, 2)
# combined 4-in-1 transpose PSUM (HW-proven); CoreSim's checker rejects it,
# so debugging scripts can flip this off before build.
COMBINED_TP = True
USE_SCH = True     # DVE Schraudolph exp offload
USE_PAIRS = True   # fp8 DoubleRow PV on unmasked k-tile pairs
DEBUG_TAPS = False  # dump intermediates to DRAM for debugging


def _r(ap, pattern, **kw):
    return ap.rearrange(pattern, **kw)


def build_program(bias_flags):
    """Build the SPMD Bass program. bias_flags: dict of bools saying which
    folded biases are nonzero (uniform across cores)."""
    nc = bacc.Bacc("TRN2", target_bir_lowering=False, debug=False, num_devices=8)

    def din(name, shape, dt=F32):
        return nc.dram_tensor(name, shape, dt, kind="ExternalInput").ap()

    xkv_d = din("xkv", [S, D])
    enc_d = din("enc", [S, D])
    masks_d = din("masks", [16, P, 512], BF16)
    msrc_d = din("msrc", [P, S // P])
    ident_d = din("ident", [P, P])
    # fp8 weights, pre-scaled x64
    w_sa = {k: din(f"sa_{k}", [D, D], FP8) for k in ("wq", "wk", "wv", "wo")}
    w_ca = {k: din(f"ca_{k}", [D, D], FP8) for k in ("wq", "wk", "wv", "wo")}
    w1_d = {k: din(f"w1{k}", [D, FF], FP8) for k in "ab"}
    w2_d = {k: din(f"w2{k}", [FF, D], FP8) for k in "ab"}
    # folded biases (pre-scaled x1024), [1, D] / [1, FF]
    b_sa = {k: din(f"bsa_{k}", [1, D]) for k in ("q", "k", "v", "o")}
    b_ca = {k: din(f"bca_{k}", [1, D]) for k in ("q", "k", "v", "o")}
    b1_d = din("b1", [1, FF])
    b2_d = din("b2", [1, D])
    out_d = nc.dram_tensor("out", [SQ, D], F32, kind="ExternalOutput").ap()
    taps = {}
    taps_live = {}
    if DEBUG_TAPS:
        for nm, shape, dt in [
                ("dbg_kvT8", [P, ET * S], FP8),
                ("dbg_ktT", [P, ET * S], BF16),
                ("dbg_qtT", [P, ET * SQ], BF16),
                ("dbg_va", [P, 16 * H * 66], FP8),
                ("dbg_atT8", [P, ET * SQ], FP8),
                ("dbg_x1", [P, (SQ // P) * D], F32),
                ("dbg_x2", [P, (SQ // P) * D], F32)]:
            taps[nm] = nc.dram_tensor(nm, shape, dt,
                                      kind="ExternalOutput").ap()

    with tile.TileContext(nc) as tc:
        with (
            tc.tile_pool(name="persist", bufs=1) as pp,
            tc.tile_pool(name="ln_sb", bufs=6) as lnp,
            tc.tile_pool(name="p_sb", bufs=5) as psb,
            tc.tile_pool(name="ln_st", bufs=4) as stp,
            tc.tile_pool(name="sc_ps", bufs=3, space="PSUM") as scp,
            tc.tile_pool(name="acc_ps", bufs=1, space="PSUM") as accp,
        ):
            ident = pp.tile([P, P], F32R, tag="ident")
            nc.sync.dma_start(ident[:], ident_d[:].bitcast(F32R))
            ones1f = pp.tile([1, P], F32, tag="ones1f")
            nc.vector.memset(ones1f[:], 1.0)
            ones1 = pp.tile([1, P], F32R, tag="ones1")
            nc.vector.tensor_copy(ones1[:], ones1f[:])
            c16_bf = pp.tile([1, P], BF16, tag="c16_bf")
            nc.vector.memset(c16_bf[:], 16.0)
            ln16 = pp.tile([P, 1], F32, tag="ln16")
            nc.vector.memset(ln16[:], LN16)
            msrc = pp.tile([P, S // P], F32, tag="msrc")
            nc.sync.dma_start(msrc[:], msrc_d[:])
            masks_sb = pp.tile([P, 16 * 512], BF16, tag="masks")

            def load_w(dram, name, cols=D):
                # [cin, cols] -> sbuf [128, ET, cols] fp8
                t = pp.tile([P, ET * cols], FP8, tag=name)
                nc.sync.dma_start(
                    _r(t[:], "p (e d) -> p e d", e=ET),
                    _r(dram[:], "(e p) d -> p e d", p=P))
                return t

            def load_bias(dram, name, flag, n=D):
                if not flag:
                    return None
                t = pp.tile([1, n], F32R, tag=name)
                nc.sync.dma_start(t[:], dram[:].bitcast(F32R))
                return t

            def pv8(t8):
                return _r(t8[:], "p (e s) -> p e s", e=ET)

            def copy_scaled(dst, src, c, on_act):
                """PSUM->SBUF copy with scale, engine-balanced."""
                with nc.allow_low_precision("fp8/bf16 staging"):
                    if on_act:
                        nc.scalar.activation(dst, src, AFT.Copy, scale=c)
                    else:
                        nc.vector.tensor_scalar_mul(dst, src, c)

            def ln_stats(x_t, sx_act=False):
                """x_t: [128, 512] f32 sbuf -> (scale, bias) per-row [128,1]."""
                sx = stp.tile([P, 1], F32, tag="sx")
                dump = lnp.tile([P, D], F32, tag="ln_xn")
                sq = stp.tile([P, 1], F32, tag="sq")
                nc.scalar.activation(dump[:], x_t[:], AFT.Square,
                                     accum_out=sq[:])
                if sx_act:
                    dump2 = lnp.tile([P, D], F32, tag="ln_xn", name="dump2")
                    nc.scalar.activation(dump2[:], x_t[:], AFT.Identity,
                                         accum_out=sx[:])
                else:
                    nc.vector.reduce_sum(sx[:], x_t[:], axis=AXL.X)
                mu = stp.tile([P, 1], F32, tag="mu")
                nc.vector.tensor_scalar_mul(mu[:], sx[:], 1.0 / D)
                m2 = stp.tile([P, 1], F32, tag="m2")
                nc.vector.tensor_mul(m2[:], mu[:], mu[:])
                v1 = stp.tile([P, 1], F32, tag="v1")
                nc.vector.tensor_scalar(v1[:], m2[:], -float(D), None,
                                        op0=ALU.mult)
                nc.vector.tensor_add(v1[:], v1[:], sq[:])
                std = stp.tile([P, 1], F32, tag="std")
                nc.scalar.activation(std[:], v1[:], AFT.Sqrt,
                                     scale=1.0 / (D - 1))
                nc.vector.tensor_scalar_add(std[:], std[:], EPS)
                s = stp.tile([P, 1], F32, tag="s")
                nc.vector.reciprocal(s[:], std[:])
                nb = stp.tile([P, 1], F32, tag="nb")
                nc.vector.tensor_mul(nb[:], mu[:], s[:])
                nc.vector.tensor_scalar_mul(nb[:], nb[:], -1.0)
                return s, nb

            def transpose4(xn, dstT8, rows, t, on_act):
                """Transpose [128, 512] f32r sbuf tile t into dstT8
                [128, ET*rows] fp8 (x16): 4 e-transposes, 1 scaled copy."""
                if COMBINED_TP:
                    ps = scp.tile([P, 1024], F32R, tag="score",
                                  name="tp")[:, 0:512]
                    for e in range(ET):
                        nc.tensor.matmul(
                            ps[:, e * P:(e + 1) * P],
                            xn[:, e * P:(e + 1) * P], ident[:],
                            start=(e == 0), stop=(e == ET - 1),
                            is_transpose=True, skip_group_check=(e != 0))
                    copy_scaled(
                        _r(dstT8[:], "p (e s) -> p e s", e=ET)[
                            :, :, t * P:(t + 1) * P],
                        _r(ps[:], "p (e c) -> p e c", e=ET), XS, on_act)
                else:
                    for e in range(ET):
                        ps = scp.tile([P, 1024], F32R, tag="score",
                                      name="tp")[:, 0:P]
                        nc.tensor.matmul(
                            ps[:], xn[:, e * P:(e + 1) * P], ident[:],
                            start=True, stop=True, is_transpose=True)
                        copy_scaled(
                            _r(dstT8[:], "p (e s) -> p e s", e=ET)[
                                :, e, t * P:(t + 1) * P],
                            ps[:], XS, on_act)

            def ln_transpose(src_d, rows, dstT8, do_ln=True):
                nt = rows // P
                for t in range(nt):
                    if do_ln:
                        x_t = lnp.tile([P, D], F32, tag="ln_x")
                        nc.sync.dma_start(x_t[:], src_d[t * P:(t + 1) * P, :])
                        s, nb = ln_stats(x_t, sx_act=(t % 3 != 1))
                        xn = lnp.tile([P, D], F32R, tag="ln_xn")
                        nc.vector.tensor_scalar(xn[:], x_t[:], s[:], nb[:],
                                                op0=ALU.mult, op1=ALU.add)
                        transpose4(xn[:], dstT8, rows, t,
                                   on_act=(t % 2 == 0))
                    else:
                        xn = lnp.tile([P, D], F32R, tag="ln_x")
                        nc.sync.dma_start(
                            xn[:], src_d[t * P:(t + 1) * P, :].bitcast(F32R))
                        transpose4(xn[:], dstT8, rows, t, on_act=(t % 2 == 0))

            def ln_transpose_sbuf(xsb, dstT8, tiles=None):
                for t in (tiles if tiles is not None else range(SQ // P)):
                    x_t = xsb[:, t * D:(t + 1) * D]
                    s, nb = ln_stats(x_t)
                    xn = lnp.tile([P, D], F32R, tag="ln_xn")
                    nc.vector.tensor_scalar(xn[:], x_t, s[:], nb[:],
                                            op0=ALU.mult, op1=ALU.add)
                    transpose4(xn[:], dstT8, SQ, t, on_act=(t % 2 == 0))

            def projT(wt8, bt, has_b, srcT8, src_rows, dstT, src_qbs=None):
                """dstT[:, dt, :] = bf16 (W.T @ xn.T)-slice via DoubleRow."""
                if src_qbs is None:
                    src_qbs = list(range(src_rows // 512))
                nqb = len(src_qbs)
                for dt in range(ET):
                    for dqb, qb in enumerate(src_qbs):
                        ps = scp.tile([P, 1024], F32, tag="score",
                                      name="ps")[:, 0:512]
                        for ep in range(EP):
                            nc.tensor.matmul(
                                ps[:],
                                pv8(wt8)[:, 2 * ep:2 * ep + 2,
                                         dt * P:(dt + 1) * P],
                                pv8(srcT8)[:, 2 * ep:2 * ep + 2,
                                           qb * 512:(qb + 1) * 512],
                                start=(ep == 0),
                                stop=(ep == EP - 1 and not has_b),
                                perf_mode=DR)
                            if ep == EP - 1 and has_b:
                                nc.tensor.matmul(
                                    ps[:], bt[:, dt * P:(dt + 1) * P],
                                    ones1[:, 0:512].bitcast(F32R),
                                    start=False, stop=True)
                        copy_scaled(
                            dstT[:, dt * nqb * 512 + dqb * 512:
                                 dt * nqb * 512 + (dqb + 1) * 512],
                            ps[:], DS, on_act=((dt + dqb) % 3 != 0))

            def proj_va(wt8, bt, has_b, srcT8, src_rows, va):
                """V projection, token-major: va [128, nkt*8*66] fp8 = 16*V
                (+ src-mask row scaling), ones column = 16."""
                nkt = src_rows // P
                for kt in range(nkt):
                    ps = scp.tile([P, 1024], F32, tag="score",
                                  name="ps")[:, 0:512]
                    for ep in range(EP):
                        nc.tensor.matmul(
                            ps[:],
                            pv8(srcT8)[:, 2 * ep:2 * ep + 2,
                                       kt * P:(kt + 1) * P],
                            pv8(wt8)[:, 2 * ep:2 * ep + 2, 0:D],
                            start=(ep == 0),
                            stop=(ep == EP - 1 and not has_b),
                            perf_mode=DR)
                        if ep == EP - 1 and has_b:
                            nc.tensor.matmul(
                                ps[:], ones1[:, 0:P], bt[:],
                                start=False, stop=True)
                    dst = _r(va[:], "p (t h c) -> p t h c", t=nkt, h=H)
                    if bias_flags.get("msrc1"):
                        copy_scaled(dst[:, kt, :, 0:DK],
                                    _r(ps[:], "p (h c) -> p h c", h=H),
                                    XS * DS, on_act=(kt % 2 == 1))
                    else:
                        with nc.allow_low_precision("fp8 va"):
                            nc.vector.tensor_scalar(
                                dst[:, kt, :, 0:DK],
                                _r(ps[:], "p (h c) -> p h c", h=H),
                                msrc[:, kt:kt + 1], XS * DS,
                                op0=ALU.mult, op1=ALU.mult)

            def attention(ktT, va, qtT, nkts, masked, exp_dve, wo8, bo,
                          has_bo, resid, x_out, on_pos_done=None):
                """ktT [128, 4*S_k] bf16; va [128, nkt*8*66] fp8 (16*V);
                qtT [128, 4*1024] bf16; masked: fn(pos,kt)-> mask idx|None;
                exp_dve: fn(pos,kt)->bool; x_out [128,8*512] f32 resid+attn."""
                skmax = max(nkts) * P
                atT8 = pp.tile([P, ET * SQ], FP8, tag="attnT")
                taps_live["atT8"] = atT8
                for pos, nkt in enumerate(nkts):
                    units = []
                    kt = 0
                    while kt < nkt:
                        if (USE_PAIRS and masked(pos, kt) is None
                                and kt + 1 < nkt
                                and masked(pos, kt + 1) is None):
                            units.append((kt, kt + 1))
                            kt += 2
                        else:
                            units.append((kt,))
                            kt += 1
                    for hp in range(H // 2):
                        acc = [accp.tile([P, 512], F32, tag=f"acc{i}",
                                         name=f"acc{i}")
                               for i in range(2)]

                        def emit_score(kt):
                            st = scp.tile([P, 1024], F32, tag="score")
                            for i in range(2):
                                nc.tensor.matmul(
                                    st[:, i * 512:(i + 1) * 512],
                                    ktT[i * DK:(i + 1) * DK,
                                        hp * skmax + kt * P:
                                        hp * skmax + (kt + 1) * P],
                                    qtT[i * DK:(i + 1) * DK,
                                        hp * SQ + pos * 512:
                                        hp * SQ + (pos + 1) * 512],
                                    start=True, stop=True)
                            return st

                        def do_exp(pos, kt, st, pair=None):
                            """pair: ([128,2048] int8 tile, j) for fp8 pair
                            halves; None -> bf16 p_t (masked path)."""
                            on_dve = USE_SCH and exp_dve(pos, kt)
                            if pair is not None:
                                t8, j = pair
                                dst = t8[:, j * 1024:(j + 1) * 1024]
                                if on_dve:
                                    with nc.allow_low_precision("sch exp"):
                                        nc.vector.tensor_scalar(
                                            dst, st[:], SCH_A_F8, SCH_B_F8,
                                            op0=ALU.mult, op1=ALU.add)
                                else:
                                    nc.scalar.activation(
                                        dst.bitcast(FP8), st[:], AFT.Exp,
                                        bias=ln16[:], scale=1.0 / 8.0)
                                return None
                            p_t = psb.tile([P, 1024], I16, tag="p")
                            if on_dve:
                                with nc.allow_low_precision("sch exp"):
                                    nc.vector.tensor_scalar(
                                        p_t[:], st[:], SCH_A_BF, SCH_B_BF,
                                        op0=ALU.mult, op1=ALU.add)
                            else:
                                nc.scalar.activation(
                                    p_t[:].bitcast(BF16), st[:], AFT.Exp,
                                    bias=ln16[:], scale=1.0 / 8.0)
                            return p_t

                        flat = [kt for u in units for kt in u]
                        sts = {flat[0]: emit_score(flat[0])}

                        def prefetch(kt):
                            fi = flat.index(kt) + 1
                            if fi < len(flat):
                                sts[flat[fi]] = emit_score(flat[fi])

                        vat = _r(va[:], "p (t h c) -> p t h c",
                                 t=S // P, h=H)
                        for u in units:
                            if len(u) == 2:
                                k0, k1 = u
                                p2 = psb.tile([P, 2048], I8, tag="p2")
                                for j, kt in enumerate(u):
                                    st = sts.pop(kt)
                                    prefetch(kt)
                                    do_exp(pos, kt, st, pair=(p2, j))
                                p8 = p2[:].bitcast(FP8)
                                for i in range(2):
                                    h = 2 * hp + i
                                    nc.tensor.matmul(
                                        acc[i][0:DK + 2, :],
                                        vat[:, k0:k0 + 2, h, 0:66],
                                        _r(p8, "p (j x) -> p j x", j=2)[
                                            :, :, i * 512:(i + 1) * 512],
                                        start=(k0 == 0),
                                        stop=(k1 == nkt - 1),
                                        perf_mode=DR)
                            else:
                                kt = u[0]
                                st = sts.pop(kt)
                                prefetch(kt)
                                p_t = do_exp(pos, kt, st)
                                pb = p_t[:].bitcast(BF16)
                                mi = masked(pos, kt)
                                if mi is not None:
                                    mt = masks_sb[:, mi * 512:(mi + 1) * 512]
                                    for i in range(2):
                                        nc.vector.tensor_mul(
                                            pb[:, i * 512:(i + 1) * 512],
                                            pb[:, i * 512:(i + 1) * 512],
                                            mt)
                                for i in range(2):
                                    h = 2 * hp + i
                                    nc.tensor.matmul(
                                        acc[i][0:DK + 2, :],
                                        vat[:, kt, h, 0:66],
                                        pb[:, i * 512:(i + 1) * 512],
                                        start=(kt == 0), stop=(kt == nkt - 1))
                        # epilogue: atT8 = 16 * acc/denom (fp8)
                        rcl = []
                        for i in range(2):
                            rc = lnp.tile([1, 512], BF16, tag="ln_xn",
                                          name="rc")
                            with nc.allow_low_precision("softmax denom"):
                                nc.vector.reciprocal(
                                    rc[:], acc[i][DK:DK + 1, :])
                            rcl.append(rc)
                        rbl = []
                        for i in range(2):
                            rbs = lnp.tile([DK, 512], BF16, tag="ln_xn",
                                           name="rbs")
                            nc.gpsimd.partition_broadcast(rbs[:], rcl[i][:])
                            rbl.append(rbs)
                        for i in range(2):
                            with nc.allow_low_precision("fp8 attn out"):
                                nc.vector.tensor_tensor(
                                    atT8[i * DK:(i + 1) * DK,
                                         hp * SQ + pos * 512:
                                         hp * SQ + (pos + 1) * 512],
                                    acc[i][0:DK, :], rbl[i][:],
                                    op=ALU.mult)
                    # output projection + residual for this pos block
                    for qt in range(pos * 4, (pos + 1) * 4):
                        res = resid(qt)
                        ps = scp.tile([P, 1024], F32, tag="score",
                                      name="ps")[:, 0:512]
                        for ep in range(EP):
                            nc.tensor.matmul(
                                ps[:],
                                pv8(atT8)[:, 2 * ep:2 * ep + 2,
                                          qt * P:(qt + 1) * P],
                                pv8(wo8)[:, 2 * ep:2 * ep + 2, 0:D],
                                start=(ep == 0),
                                stop=(ep == EP - 1 and not has_bo),
                                perf_mode=DR)
                            if ep == EP - 1 and has_bo:
                                nc.tensor.matmul(
                                    ps[:], ones1[:, 0:P], bo[:],
                                    start=False, stop=True)
                        tmp = lnp.tile([P, D], F32, tag="ln_xn", name="wot")
                        nc.scalar.activation(tmp[:], ps[:], AFT.Copy,
                                             scale=DS)
                        nc.vector.tensor_tensor(
                            x_out[:, qt * D:(qt + 1) * D], tmp[:],
                            res, op=ALU.add)
                    if on_pos_done is not None:
                        on_pos_done(pos)

            # ---------------- stage A: LN0 + transposes ----------------
            kvT8 = pp.tile([P, ET * S], FP8, tag="kvT")
            ln_transpose(xkv_d, S, kvT8, do_ln=True)

            # ---------------- weights (all upfront, Pool DGE queue) -------
            wk8 = load_w(w_sa["wk"], "w_a")
            wq8 = load_w(w_sa["wq"], "w_b")
            wv8 = load_w(w_sa["wv"], "w_c")
            wo8 = load_w(w_sa["wo"], "w_d")
            ck8 = load_w(w_ca["wk"], "w_e")
            cq8 = load_w(w_ca["wq"], "w_f")
            cv8 = load_w(w_ca["wv"], "w_g")
            co8 = load_w(w_ca["wo"], "w_h")
            w18 = [load_w(w1_d[k], f"w1{k}", cols=FF) for k in "ab"]
            w28 = []
            for k in "ab":
                t = pp.tile([P, (FF // P) * D], FP8, tag=f"w2{k}")
                nc.sync.dma_start(
                    _r(t[:], "p (t d) -> p t d", t=FF // P),
                    _r(w2_d[k][:], "(t p) d -> p t d", p=P))
                w28.append(t)
            bk = load_bias(b_sa["k"], "b_a", bias_flags["sa_k"])
            bq = load_bias(b_sa["q"], "b_b", bias_flags["sa_q"])
            bv = load_bias(b_sa["v"], "b_c", bias_flags["sa_v"])
            bo = load_bias(b_sa["o"], "b_d", bias_flags["sa_o"])
            cbk = load_bias(b_ca["k"], "b_e", bias_flags["ca_k"])
            cbq = load_bias(b_ca["q"], "b_f", bias_flags["ca_q"])
            cbv = load_bias(b_ca["v"], "b_g", bias_flags["ca_v"])
            cbo = load_bias(b_ca["o"], "b_h", bias_flags["ca_o"])
            b1_sb = load_bias(b1_d, "b1", bias_flags["ff1"], n=FF)
            b2_sb = load_bias(b2_d, "b2", bias_flags["ff2"])
            nc.sync.dma_start(
                _r(masks_sb[:], "p (t c) -> p t c", t=16),
                _r(masks_d[:], "t p c -> p t c"))

            # ---------------- stage B: SA projections ----------------
            ktT_sa = pp.tile([P, ET * S], BF16, tag="ktT")
            qtT_sa = pp.tile([P, ET * SQ], BF16, tag="qtT")
            va_sa = pp.tile([P, 16 * H * 66], FP8, tag="va")
            nc.vector.memset(
                _r(va_sa[:], "p (t h c) -> p t h c", t=16, h=H)[:, :, :, DK:DK + 1],
                1.0)
            nc.vector.memset(
                _r(va_sa[:], "p (t h c) -> p t h c", t=16, h=H)[:, :, :, DK + 1:],
                0.0)
            if DEBUG_TAPS:
                nc.sync.dma_start(taps["dbg_kvT8"][:], kvT8[:])
            projT(wk8, bk, bias_flags["sa_k"], kvT8, S, ktT_sa)
            projT(wq8, bq, bias_flags["sa_q"], kvT8, S, qtT_sa,
                  src_qbs=list(Q_SRC_QBS))
            proj_va(wv8, bv, bias_flags["sa_v"], kvT8, S, va_sa)

            if DEBUG_TAPS:
                nc.sync.dma_start(taps["dbg_ktT"][:], ktT_sa[:])
                nc.sync.dma_start(taps["dbg_qtT"][:], qtT_sa[:])
                nc.sync.dma_start(taps["dbg_va"][:], va_sa[:])
            # ---------------- stage C/D: SA attention + Wo ----------------
            x1 = pp.tile([P, (SQ // P) * D], F32, tag="x1")

            def sa_masked(pos, kt):
                return kt if (pos == 0 or kt >= 8) else None

            def sa_exp_dve(pos, kt):
                if sa_masked(pos, kt) is None:
                    return kt % 4 == 1
                return (kt % 8) == 5

            def q_src_row(qt):
                pos, j = divmod(qt, 4)
                return Q_SRC_QBS[pos] * 512 + j * P

            def sa_resid(qt):
                rt = lnp.tile([P, D], F32, tag="ln_x", name="sa_resid")
                r0 = q_src_row(qt)
                nc.sync.dma_start(rt[:], xkv_d[r0:r0 + P, :])
                return rt[:]

            # hoisted CA prep: encoder transpose overlaps SA attention
            encT8 = pp.tile([P, ET * S], FP8, tag="kvT")  # reuse kvT slot
            ln_transpose(enc_d, S, encT8, do_ln=False)

            attention(ktT_sa, va_sa, qtT_sa, [NKT0, NKT1], sa_masked,
                      sa_exp_dve, wo8, bo, bias_flags["sa_o"], sa_resid, x1)

            if DEBUG_TAPS:
                nc.sync.dma_start(taps["dbg_x1"][:], x1[:])
                nc.sync.dma_start(taps["dbg_atT8"][:],
                                  taps_live["atT8"][:])
            # ---------------- stage E/F: CA ----------------
            ktT_ca = pp.tile([P, ET * S], BF16, tag="ktT")
            qtT_ca = pp.tile([P, ET * SQ], BF16, tag="qtT")
            va_ca = pp.tile([P, 16 * H * 66], FP8, tag="va")
            nc.vector.memset(
                _r(va_ca[:], "p (t h c) -> p t h c", t=16, h=H)[:, :, :, DK:DK + 1],
                1.0)
            nc.vector.memset(
                _r(va_ca[:], "p (t h c) -> p t h c", t=16, h=H)[:, :, :, DK + 1:],
                0.0)
            projT(ck8, cbk, bias_flags["ca_k"], encT8, S, ktT_ca)
            proj_va(cv8, cbv, bias_flags["ca_v"], encT8, S, va_ca)

            h1T8 = pp.tile([P, ET * SQ], FP8, tag="hT")
            ln_transpose_sbuf(x1, h1T8)
            projT(cq8, cbq, bias_flags["ca_q"], h1T8, SQ, qtT_ca)

            x2 = pp.tile([P, (SQ // P) * D], F32, tag="x2")

            attention(ktT_ca, va_ca, qtT_ca, [16, 16], lambda p, k: None,
                      lambda p, k: (k % 2 == 1) and (k % 16 != 3), co8,
                      cbo,
                      bias_flags["ca_o"],
                      lambda qt: x1[:, qt * D:(qt + 1) * D], x2)

            if DEBUG_TAPS:
                nc.sync.dma_start(taps["dbg_x2"][:], x2[:])
            # ---------------- stage G: LN2 + FFN ----------------
            h2T8 = pp.tile([P, ET * SQ], FP8, tag="hT")
            ln_transpose_sbuf(x2, h2T8)

            ffT8 = pp.tile([P, (FF // P) * SQ], FP8, tag="ffT")

            def ffn2_qt(qt):
                ps = scp.tile([P, 1024], F32, tag="score",
                              name="ps")[:, 0:512]
                nfp = FF // P // 2
                for wi, wt in enumerate(w28):
                    for fp in range(nfp):
                        nc.tensor.matmul(
                            ps[:],
                            _r(ffT8[:], "p (t s) -> p t s", t=FF // P)[
                                :, 2 * fp:2 * fp + 2, qt * P:(qt + 1) * P],
                            _r(wt[:], "p (t d) -> p t d", t=FF // P)[
                                :, 2 * fp:2 * fp + 2, :],
                            start=(wi == 0 and fp == 0),
                            stop=(wi == 1 and fp == nfp - 1
                                  and not bias_flags["ff2"]),
                            perf_mode=DR)
                if bias_flags["ff2"]:
                    nc.tensor.matmul(
                        ps[:], ones1[:, 0:P], b2_sb[:],
                        start=False, stop=True)
                tmp = lnp.tile([P, D], F32, tag="ln_xn", name="ff2t")
                copy_scaled(tmp[:], ps[:], DS, on_act=(qt % 2 == 0))
                o_t = lnp.tile([P, D], F32, tag="ln_xn", name="o_t")
                nc.vector.tensor_tensor(
                    o_t[:], tmp[:], x2[:, qt * D:(qt + 1) * D], op=ALU.add)
                nc.sync.dma_start(out_d[qt * P:(qt + 1) * P, :], o_t[:])

            for qb in range(SQ // 512):
                if qb == 1:
                    for qt in range(4):
                        ffn2_qt(qt)
                for ft in range(FF // P):
                    ps = scp.tile([P, 1024], F32, tag="score",
                                  name="ps")[:, 0:512]
                    for wi, wt in enumerate(w18):
                        for ep in range(EP):
                            last = (wi == 1 and ep == EP - 1)
                            nc.tensor.matmul(
                                ps[:],
                                _r(wt[:], "p (e f) -> p e f", e=ET)[
                                    :, 2 * ep:2 * ep + 2,
                                    ft * P:(ft + 1) * P],
                                pv8(h2T8)[:, 2 * ep:2 * ep + 2,
                                          qb * 512:(qb + 1) * 512],
                                start=(wi == 0 and ep == 0),
                                stop=(last and not bias_flags["ff1"]),
                                perf_mode=DR)
                            if last and bias_flags["ff1"]:
                                nc.tensor.matmul(
                                    ps[:], b1_sb[:, ft * P:(ft + 1) * P],
                                    ones1[:, 0:512].bitcast(F32R),
                                    start=False, stop=True)
                    # ffT8 = 16*relu(z); alternate Act/DVE per ft
                    if ft % 2 == 0:
                        nc.scalar.activation(
                            ffT8[:, ft * SQ + qb * 512:
                                 ft * SQ + (qb + 1) * 512],
                            ps[:], AFT.Relu, scale=XS * DS)
                    else:
                        with nc.allow_low_precision("fp8 relu"):
                            nc.vector.tensor_scalar(
                                ffT8[:, ft * SQ + qb * 512:
                                     ft * SQ + (qb + 1) * 512],
                                ps[:], 0.0, XS * DS,
                                op0=ALU.max, op1=ALU.mult)
            for qt in range(4, SQ // P):
                ffn2_qt(qt)

    nc.finalize()
    return nc


_CACHE = {}
LAST_EXEC_NS = None


def kernel(**inputs):
    x = np.asarray(inputs["x"], np.float32)
    enc = np.asarray(inputs["encoder_output"], np.float32)
    src_mask = np.asarray(inputs["src_mask"]).reshape(S)
    tgt_mask = np.asarray(inputs["tgt_mask"]).reshape(S, S)

    def fold(w, g, b, extra_b):
        w = np.asarray(w, np.float32)
        wf = np.asarray(g, np.float32)[:, None] * w
        bf = np.asarray(b, np.float32) @ w + np.asarray(extra_b, np.float32)
        return wf, bf

    def q8(w):
        return np.asarray(w * WS, np.float32).astype(ml_dtypes.float8_e4m3)

    def q8r(w):
        ws = np.asarray(w * WS, np.float32)
        return (ws - ws.astype(ml_dtypes.float8_e4m3)
                .astype(np.float32)).astype(ml_dtypes.float8_e4m3)

    z = np.zeros(D, np.float32)
    sa_wq, bsa_q = fold(inputs["sa_wq"], inputs["ln0_g"], inputs["ln0_b"], z)
    sa_wk, bsa_k = fold(inputs["sa_wk"], inputs["ln0_g"], inputs["ln0_b"], z)
    sa_wv, bsa_v = fold(inputs["sa_wv"], inputs["ln0_g"], inputs["ln0_b"], z)
    sa_wo = np.asarray(inputs["sa_wo"], np.float32)
    bsa_o = np.asarray(inputs["sa_bo"], np.float32)
    ca_wq, bca_q = fold(inputs["ca_wq"], inputs["ln1_g"], inputs["ln1_b"], z)
    ca_wk = np.asarray(inputs["ca_wk"], np.float32)
    bca_k = np.zeros(D, np.float32)
    ca_wv = np.asarray(inputs["ca_wv"], np.float32)
    bca_v = np.zeros(D, np.float32)
    ca_wo = np.asarray(inputs["ca_wo"], np.float32)
    bca_o = np.asarray(inputs["ca_bo"], np.float32)
    w1, b1 = fold(inputs["ff_w1"], inputs["ln2_g"], inputs["ln2_b"],
                  np.asarray(inputs["ff_b1"], np.float32))
    w2 = np.asarray(inputs["ff_w2"], np.float32)
    b2 = np.asarray(inputs["ff_b2"], np.float32)

    bias_flags = {
        "sa_q": bool(np.any(bsa_q)), "sa_k": bool(np.any(bsa_k)),
        "sa_v": bool(np.any(bsa_v)), "sa_o": bool(np.any(bsa_o)),
        "ca_q": bool(np.any(bca_q)), "ca_k": bool(np.any(bca_k)),
        "ca_v": bool(np.any(bca_v)), "ca_o": bool(np.any(bca_o)),
        "ff1": bool(np.any(b1)), "ff2": bool(np.any(b2)),
        "msrc1": bool(np.all(src_mask == 1)),
    }

    key = tuple(sorted(bias_flags.items()))
    if key not in _CACHE:
        _CACHE[key] = build_program(bias_flags)
    nc = _CACHE[key]

    ident = np.eye(P, dtype=np.float32)
    msrc = src_mask.astype(np.float32).reshape(S // P, P).T.copy()
    BS = WS * XS  # bias pre-scale (descaled by DS in the psum copy)

    shared = {
        "ident": ident, "msrc": msrc,
        "sa_wq": q8(sa_wq), "sa_wk": q8(sa_wk), "sa_wv": q8(sa_wv),
        "sa_wo": q8(sa_wo),
        "ca_wq": q8(ca_wq), "ca_wk": q8(ca_wk), "ca_wv": q8(ca_wv),
        "ca_wo": q8(ca_wo),
        "w1a": q8(w1), "w1b": q8r(w1), "w2a": q8(w2), "w2b": q8r(w2),
        "bsa_q": bsa_q[None] * BS, "bsa_k": bsa_k[None] * BS,
        "bsa_v": bsa_v[None] * BS, "bsa_o": bsa_o[None] * BS,
        "bca_q": bca_q[None] * BS, "bca_k": bca_k[None] * BS,
        "bca_v": bca_v[None] * BS, "bca_o": bca_o[None] * BS,
        "b1": b1[None] * BS, "b2": b2[None] * BS,
    }

    in_maps = []
    for c in range(8):
        b, r = divmod(c, 2)
        perm = PERM_BLOCKS[r]
        rows = np.concatenate(
            [np.arange(gb * 512, (gb + 1) * 512) for gb in perm])
        gb0, gb1 = OWN_BLOCKS[r]
        assert perm[0] == gb0 and perm[2] == gb1
        mk = np.zeros((16, P, 512), np.float32)
        for pos, gb in enumerate((gb0, gb1)):
            qs = slice(gb * 512, (gb + 1) * 512)
            mrow = tgt_mask[qs][:, rows]
            for j in range(8):
                kt = j if pos == 0 else 8 + j
                ks = slice(kt * P, (kt + 1) * P)
                mk[pos * 8 + j] = mrow[:, ks].T
            ext = (NKT0 if pos == 0 else NKT1) * P
            assert not np.any(mrow[:, ext:]), "tgt_mask beyond extent"
        im = dict(shared)
        im["xkv"] = np.ascontiguousarray(x[b][rows])
        im["enc"] = np.ascontiguousarray(enc[b])
        im["masks"] = mk.astype(ml_dtypes.bfloat16)
        in_maps.append(im)

    res = run_bass_kernel_spmd(nc, in_maps, core_ids=list(range(8)))
    global LAST_EXEC_NS
    LAST_EXEC_NS = res.exec_time_ns

    out = np.empty((B, S, D), np.float32)
    for c in range(8):
        b, r = divmod(c, 2)
        gb0, gb1 = OWN_BLOCKS[r]
        o = res.results[c]["out"]
        out[b, gb0 * 512:(gb0 + 1) * 512] = o[0:512]
        out[b, gb1 * 512:(gb1 + 1) * 512] = o[512:1024]
    return out
